# revision 32
# baseline (speedup 1.0000x reference)
"""Bass/Trainium2 kernel for nn_CSEM sparse_attention problem (v3).

Sharding: 8 cores = 4 samples x 2 spatial halves (bottom half vertically
flipped on host so all cores run an identical NEFF).

v3 vs v2: fp8e4 DoubleRow matmuls for conv0 / conv1 / P@V (one DR matmul
accumulates two (weights-plane, ifmap-plane) K-tiles at 0.5 cycles/row).
Activations and weights for those convs are fp8; conv taps are addressed as
column/row offsets into padded flat slabs so tap pairs become stride-`d`
plane pairs of a single AP. Softmax is transpose-free: S'^T stays key-major,
q-norms are broadcast along partitions, exp goes straight to fp8 P^T planes,
and the softmax denominator (from an ones-matmul colsum) is folded into the
depthwise-conv evacuation scale. Depthwise conv + q/k transposes + S' stay
bf16 for accuracy.
"""

import numpy as np
import ml_dtypes

import concourse.bass as bass
import concourse.mybir as mybir
import concourse.tile as tile
from concourse.bass_utils import run_bass_kernel_spmd
from concourse.masks import make_identity

F8 = mybir.dt.float8e4
BF16 = mybir.dt.bfloat16
F32 = mybir.dt.float32
BN_EPS = 1e-5
f8np = ml_dtypes.float8_e4m3

CIN, C, C3 = 64, 192, 576
W, WPD = 96, 98
XR = 54          # x slab rows (1 zero pad + 53 data)
YR = 52          # y rows computed locally (0..51)
TR = 50          # t rows 0..49
QR = 49          # conv1 v rows 0..48
SR = 48          # rows feeding S partial
OR_ = 48         # final output rows per core
NCH = SR * W // 128   # 36 transpose chunks
VS = QR * W      # v plane stride (4704)
SL = 4908        # Tall slab stride (1 lead + 50*98 data + slack)
XLEN = 1 + XR * WPD + 8
W0S, W1S = 32.0, 64.0
GROUPS = [[0, 1], [2, 3], [4, 5], [6, 7]]

# conv1 DoubleRow pairs as ((slab, dx), (slab, dx)); None = zero-weight dummy
# plane (rhs offset +2 -- a dim1 stride equal to the element stride wedges the
# device). Pairs stay within one slab so each block's read footprint is
# row-local and conv1 pipelines with the bilinear-branch writes.
PAIRS_C1 = [((0, 0), (0, 2)), ((0, 1), None), ((1, 0), (1, 2)),
            ((1, 1), None), ((2, 0), (2, 2)), ((2, 1), None),
            ((3, 0), (3, 2)), ((3, 1), None)]
PAIRS_T4 = [((0, 0), (0, 2)), ((0, 1), None)]   # slab idx relative to T4


def _split_waits(nc, limit=1):
    """This walrus build rejects instructions carrying more than one sem-wait
    command. Spread extra waits onto same-engine ENGINE_NOPs inserted just
    before the offending instruction (semantically identical: the engine
    blocks on each wait in program order)."""
    ctr = [0]
    for f in nc.m.functions:
        for blk in f.blocks:
            il = blk.instructions
            new = []
            for inst in il:
                si = inst.sync_info
                waits = list(si.on_wait) if (si and si.on_wait) else []
                if len(waits) > limit:
                    for w in waits[:-limit]:
                        ctr[0] += 1
                        nop = mybir.InstNoOp(name=f"WNOP-{ctr[0]}")
                        nop.engine = inst.engine
                        nop.sync_info = mybir.SyncInfo(on_wait=[w], on_update=[])
                        new.append(nop)
                    si.on_wait = waits[-limit:]
                new.append(inst)
            il[:] = new


def _row_blocks(nrows, per=5):
    out, r = [], 0
    while r < nrows:
        n = min(per, nrows - r)
        out.append((r, n))
        r += n
    return out


def build_nc():
    nc = bass.Bass()

    xs_d = nc.declare_dram_parameter("xs", [64, XR * WPD], F8, isOutput=False)
    w0_d = nc.declare_dram_parameter("w0p", [128, 3 * 2 * C], F8, isOutput=False)
    sb0_d = nc.declare_dram_parameter("sb0p", [C, 2], F32, isOutput=False)
    w1_d = nc.declare_dram_parameter("w1p", [128, 8 * 2 * C3], F8, isOutput=False)
    w14_d = nc.declare_dram_parameter("w1p4", [64, 2 * 2 * C3], F8, isOutput=False)
    sb1_d = nc.declare_dram_parameter("sb1", [C3, 2], F32, isOutput=False)
    w2a_d = nc.declare_dram_parameter("w2da", [128, 9 * 128], BF16, isOutput=False)
    w2f_d = nc.declare_dram_parameter("w2fb", [128, 3 * 64], BF16, isOutput=False)
    w2g_d = nc.declare_dram_parameter("w2gb", [128, 3 * 64], BF16, isOutput=False)
    b2_d = nc.declare_dram_parameter("b2v", [C, 1], F32, isOutput=False)
    tmp_d = nc.declare_dram_parameter("tempv", [1, 1], F32, isOutput=False)
    z8_d = nc.declare_dram_parameter("zpad8", [1, 512], F8, isOutput=False)
    z16_d = nc.declare_dram_parameter("zpad16", [1, 512], BF16, isOutput=False)
    yout = nc.declare_dram_parameter("yout", [C, OR_ * W], F32, isOutput=True)

    cc2i = nc.dram_tensor("cc2i", [C, 194], BF16)
    cc2o = nc.dram_tensor("cc2o", [C, 194], BF16)

    with tile.TileContext(nc) as tc:
        _body(nc, tc, xs_d, w0_d, sb0_d, w1_d, w14_d, sb1_d, w2a_d, w2f_d,
              w2g_d, b2_d, tmp_d, z8_d, z16_d, yout, cc2i, cc2o)
    _split_waits(nc)
    return nc


def _body(nc, tc, xs_d, w0_d, sb0_d, w1_d, w14_d, sb1_d, w2a_d, w2f_d,
          w2g_d, b2_d, tmp_d, z8_d, z16_d, yout, cc2i, cc2o):
    import contextlib
    ctx = contextlib.ExitStack()
    P = ctx.enter_context(tc.tile_pool(name="persist", bufs=1))
    ev = ctx.enter_context(tc.tile_pool(name="evac", bufs=3))

    # ---- persistent SBUF ----
    xfold = P.tile([128, XLEN], F8, tag="xfold")
    w0s = P.tile([128, 3, 2, C], F8, tag="w0s")
    w1s = P.tile([128, 8, 2, C3], F8, tag="w1s")
    w1s4 = P.tile([64, 2, 2, C3], F8, tag="w1s4")
    sc0a = P.tile([128, 2], F32, tag="sc0a")
    sc0b = P.tile([64, 2], F32, tag="sc0b")
    scp = [128, 128, 128, 128, 64]
    sc1 = [P.tile([scp[i], 2], F32, tag=f"sc1_{i}", name=f"sc1_{i}")
           for i in range(5)]
    w2da = P.tile([128, 9 * 128], BF16, tag="w2da")
    w2fb = P.tile([128, 3 * 64], BF16, tag="w2fb")
    w2gb = P.tile([128, 3 * 64], BF16, tag="w2gb")  # data at partitions 64..
    b2a = P.tile([128, 1], F32, tag="b2a")
    b2b = P.tile([64, 1], F32, tag="b2b")
    tmps = P.tile([128, 1], F32, tag="tmps")

    Tall = P.tile([128, 4 * SL], F8, tag="Tall")
    T4 = P.tile([64, SL], F8, tag="T4")
    xpool = P.tile([128, YR * WPD], BF16, tag="xpool")   # later reused as oa
    plh = P.tile([128, YR, 48], BF16, tag="plh")
    pl = P.tile([128, 26, 48], BF16, tag="pl")
    vint = P.tile([128, TR, 48], BF16, tag="vint")
    tscr = P.tile([128, TR, 48], BF16, tag="tscr")
    brs = P.tile([128, 51 * WPD], F8, tag="brs")
    ob = P.tile([128, TR * WPD], BF16, tag="ob")
    qk = [P.tile([128, SR * W], BF16, tag=f"qk{i}", name=f"qk{i}")
          for i in range(3)]
    vall = P.tile([128, 2 * VS], F8, tag="vall")
    sqac = [P.tile([128, 10], F32, tag=f"sqac{i}", name=f"sqac{i}")
            for i in range(3)]
    accs = [P.tile([128, 1], F32, tag=f"accs{i}", name=f"accs{i}")
            for i in range(3)]
    ssa = P.tile([128, 194], BF16, tag="ssa")
    ssb = P.tile([64, 194], BF16, tag="ssb")
    sfa = P.tile([128, 194], BF16, tag="sfa")
    sfb = P.tile([64, 194], BF16, tag="sfb")
    accb = P.tile([128, 2], BF16, tag="accb")
    rska = P.tile([128, 1], F32, tag="rska")
    rskb = P.tile([64, 1], F32, tag="rskb")
    qs = P.tile([1, C], F32, tag="qs")
    rqbc = P.tile([128, C], F32, tag="rqbc")
    ea = P.tile([128, C], F32, tag="ea")
    eb = P.tile([64, C], F32, tag="eb")
    ptall = P.tile([128, 2 * C], F8, tag="ptall")
    isr = P.tile([1, C], F32, tag="isr")
    rsA = P.tile([128, 1], F32, tag="rsA")
    rsB = P.tile([64, 1], F32, tag="rsB")
    ones8 = P.tile([128, 1], F8, tag="ones8")
    ones1 = P.tile([1, 128], F32, tag="ones1")

    ident = P.tile([128, 128], BF16, tag="ident")
    make_identity(nc, ident[:])
    identf = P.tile([128, 128], F32, tag="identf")
    make_identity(nc, identf[:])
    nc.vector.memset(ones8[:], 1.0)
    nc.vector.memset(ones1[:], 1.0)

    # ---- input DMAs (x chunked so conv0 starts early) ----
    nc.sync.dma_start(out=w0s[:], in_=w0_d.rearrange("p (a b m) -> p a b m",
                                                     a=3, b=2))
    nc.gpsimd.dma_start(out=sc0a[:], in_=sb0_d[0:128, :])
    nc.gpsimd.dma_start(out=sc0b[:], in_=sb0_d[128:192, :])
    for (a, b) in ((0, 8), (8, 32), (32, XR)):
        nc.sync.dma_start(out=xfold[0:64, 1 + a * WPD:1 + b * WPD],
                          in_=xs_d[:, a * WPD:b * WPD])
        bb = min(b, XR - 1)
        nc.sync.dma_start(out=xfold[64:128, 1 + a * WPD:1 + bb * WPD],
                          in_=xs_d[:, (a + 1) * WPD:(bb + 1) * WPD])
    nc.gpsimd.dma_start(out=w1s[:], in_=w1_d.rearrange("p (a b m) -> p a b m",
                                                     a=8, b=2))
    nc.gpsimd.dma_start(out=w1s4[:], in_=w14_d.rearrange(
        "p (a b m) -> p a b m", a=2, b=2))
    for i, (lo, hi) in enumerate([(0, 128), (128, 256), (256, 384),
                                  (384, 512), (512, 576)]):
        nc.gpsimd.dma_start(out=sc1[i][:], in_=sb1_d[lo:hi, :])
    nc.gpsimd.dma_start(out=w2da[:], in_=w2a_d[:])
    nc.gpsimd.dma_start(out=w2fb[:], in_=w2f_d[:])
    nc.gpsimd.dma_start(out=w2gb[:], in_=w2g_d[:])
    nc.gpsimd.dma_start(out=b2a[:], in_=b2_d[0:128, :])
    nc.gpsimd.dma_start(out=b2b[:], in_=b2_d[128:192, :])
    nc.gpsimd.dma_start(
        out=tmps[:],
        in_=bass.AP(tensor=tmp_d, offset=0, ap=[[0, 128], [1, 1]]))

    # ---- border memsets (DVE; only the load-bearing zeros) ----
    # xfold lead/tail and Tall slab leads/tails are read only by discarded
    # junk output columns -- no clears needed there. T4 row 49 IS needed:
    # the zero-weight dummy plane reads it (0 x NaN = NaN).
    tall_b = Tall[:]
    t4_b = T4[:]

    def slabv(s, plo, phi, r0, r1, c0=0, c1=WPD):
        """[phi-plo, r1-r0, c1-c0] view of slab s rows r0..r1."""
        base = tall_b if s < 4 else t4_b
        off = (s % 4) * SL if s < 4 else 0
        ap0 = base.ap[0][0]
        return bass.AP(tensor=base.tensor,
                       offset=base.offset + plo * ap0 + off + 1 + r0 * WPD + c0,
                       ap=[[ap0, phi - plo], [WPD, r1 - r0], [1, c1 - c0]])

    for s in range(4):
        nc.vector.memset(slabv(s, 0, 128, 0, 1), 0.0)              # row 0
        nc.vector.memset(slabv(s, 0, 128, 0, TR, 0, 1), 0.0)       # col 0
        nc.vector.memset(slabv(s, 0, 128, 0, TR, 97, 98), 0.0)     # col 97
    nc.vector.memset(T4[:, 1 + 49 * WPD:SL], 0.0)   # row 49 + slack
    nc.vector.memset(slabv(4, 0, 64, 0, 1), 0.0)
    nc.vector.memset(slabv(4, 0, 64, 0, 49, 0, 1), 0.0)
    nc.vector.memset(slabv(4, 0, 64, 0, 49, 97, 98), 0.0)

    brv = brs.rearrange("p (r w) -> p r w", w=WPD)
    nc.vector.memset(brv[:, 0:1, :], 0.0)
    nc.vector.memset(brv[:, :, 0:1], 0.0)
    nc.vector.memset(brv[:, :, 97:98], 0.0)

    xpv = xpool.rearrange("p (r w) -> p r w", w=WPD)
    obv = ob.rearrange("p (r w) -> p r w", w=WPD)
    qkv = [t.rearrange("p (r w) -> p r w", w=W) for t in qk]
    w2av = w2da.rearrange("p (t m) -> p t m", t=9)
    w2fv = w2fb.rearrange("p (t m) -> p t m", t=3)
    w2gv = w2gb.rearrange("p (t m) -> p t m", t=3)

    xf_b = xfold[:]

    def c0_rhs(s0, d0, s1, d1, nr):
        o0 = 1 + s0 * WPD + d0 - 1
        o1 = 1 + s1 * WPD + d1 - 1
        return bass.AP(tensor=xf_b.tensor, offset=xf_b.offset + o0,
                       ap=[[xf_b.ap[0][0], 128], [o1 - o0, 2], [1, nr * WPD]])

    def c1_rhs(pairs_base, p0, p1, r0, nr):
        (s0, d0) = p0
        if p1 is None:
            s1, d1 = s0, d0 + 2
        else:
            s1, d1 = p1
        off = (lambda s, d: s * SL + 1 + r0 * WPD + d - 1)
        o0, o1 = off(s0, d0), off(s1, d1)
        return bass.AP(tensor=pairs_base.tensor, offset=pairs_base.offset + o0,
                       ap=[[pairs_base.ap[0][0], pairs_base.ap[0][1]],
                           [o1 - o0, 2], [1, nr * WPD]])

    # ---------------- conv0 (fp8 DoubleRow) ----------------
    # out-tile A (m 0:128): x1|x3 -> xpool (bf16); out-tile B (m 128:192):
    # x2 -> Tall slab2 lower partitions (fp8)
    blocksA = _row_blocks(YR)
    blocksB = _row_blocks(TR)
    with tc.tile_pool(name="ps_c0", bufs=4, space="PSUM") as pp0:
        def conv0_block(r0, nr, m0, mw):
            ps = pp0.tile([128, 5 * WPD], F32, tag="c0ps")
            specs = ((r0, 0, r0, 2), (r0, 1, r0 + 1, 1),
                     (r0 + 1, 0, r0 + 1, 2))
            for pi, (s0, d0, s1, d1) in enumerate(specs):
                nc.tensor.matmul(
                    ps[0:mw, 0:nr * WPD], lhsT=w0s[:, pi, :, m0:m0 + mw],
                    rhs=c0_rhs(s0, d0, s1, d1, nr),
                    start=(pi == 0), stop=(pi == 2),
                    perf_mode=mybir.MatmulPerfMode.DoubleRow)
            return ps

        for (r0, nr) in blocksA:
            ps = conv0_block(r0, nr, 0, 128)
            psv = ps.rearrange("p (r x) -> p r x", x=WPD)
            nc.scalar.activation(
                out=xpv[:, r0:r0 + nr, 1:97], in_=psv[:, 0:nr, 1:97],
                func=mybir.ActivationFunctionType.Relu,
                bias=sc0a[:, 1:2], scale=sc0a[:, 0:1])
        for (r0, nr) in blocksB:
            ps = conv0_block(r0, nr, 128, 64)
            psv = ps.rearrange("p (r x) -> p r x", x=WPD)
            # x2 -> ky1 slot = slab2 partitions 0..63 (t rows at tile rows)
            nc.scalar.activation(
                out=slabv(2, 0, 64, r0, r0 + nr, 1, 97),
                in_=psv[0:64, 0:nr, 1:97],
                func=mybir.ActivationFunctionType.Relu,
                bias=sc0b[:, 1:2], scale=sc0b[:, 0:1])

    # x2 ky-shifted slot copies (slab2 col pads are zero so full width)
    for (a, b) in ((0, 10), (10, 30), (30, 49)):
        nc.sync.dma_start(out=slabv(0, 64, 128, a + 1, b + 1),
                          in_=slabv(2, 0, 64, a, b))
        nc.sync.dma_start(out=slabv(3, 64, 128, a, b),
                          in_=slabv(2, 0, 64, a + 1, b + 1))

    # ---------------- pools + bilinear (DVE; final writes fp8 brs) --------
    cA = P.tile([128, 2], F32, tag="cA")
    nc.vector.memset(cA[0:64, 0:1], 0.75)
    nc.vector.memset(cA[0:64, 1:2], 0.25)
    nc.vector.memset(cA[64:128, 0:1], 0.1875)
    nc.vector.memset(cA[64:128, 1:2], 0.0625)
    cC = P.tile([128, 1], F32, tag="cC")
    nc.vector.memset(cC[0:64, :], 1.0)
    nc.vector.memset(cC[64:128, :], 0.25)

    pl_chunks = [(0, 5), (5, 10), (10, 15), (15, 20), (20, 25), (25, 26)]
    kv_o = 0   # next odd-row k (vint[2k+1], k<=24)
    kv_e = 0   # next even-row k (vint[2k+2], k<=23)
    hv = 0     # next t-row for the horizontal pass

    def hpass(a, b):
        if b <= a:
            return
        nc.vector.tensor_scalar(out=brv[:, 1 + a:1 + b, 1:2],
                                in0=vint[:, a:b, 0:1], scalar1=cC[:, 0:1],
                                scalar2=None, op0=mybir.AluOpType.mult)
        nc.vector.tensor_scalar(out=brv[:, 1 + a:1 + b, 96:97],
                                in0=vint[:, a:b, 47:48], scalar1=cC[:, 0:1],
                                scalar2=None, op0=mybir.AluOpType.mult)
        nc.vector.tensor_scalar(out=tscr[:, a:b, 0:47], in0=vint[:, a:b, 1:48],
                                scalar1=cA[:, 1:2], scalar2=None,
                                op0=mybir.AluOpType.mult)
        nc.vector.scalar_tensor_tensor(
            out=brv[:, 1 + a:1 + b, 2:96:2], in0=vint[:, a:b, 0:47],
            scalar=cA[:, 0:1], in1=tscr[:, a:b, 0:47],
            op0=mybir.AluOpType.mult, op1=mybir.AluOpType.add)
        nc.vector.tensor_scalar(out=tscr[:, a:b, 0:47], in0=vint[:, a:b, 1:48],
                                scalar1=cA[:, 0:1], scalar2=None,
                                op0=mybir.AluOpType.mult)
        nc.vector.scalar_tensor_tensor(
            out=brv[:, 1 + a:1 + b, 3:96:2], in0=vint[:, a:b, 0:47],
            scalar=cA[:, 1:2], in1=tscr[:, a:b, 0:47],
            op0=mybir.AluOpType.mult, op1=mybir.AluOpType.add)

    for (k0, k1) in pl_chunks:
        # horizontal pool pairs for y rows 2k0..2k1-1
        nc.vector.tensor_tensor(out=plh[0:64, 2 * k0:2 * k1, :],
                                in0=xpv[0:64, 2 * k0:2 * k1, 1:97:2],
                                in1=xpv[0:64, 2 * k0:2 * k1, 2:98:2],
                                op=mybir.AluOpType.max)
        nc.vector.tensor_tensor(out=plh[64:128, 2 * k0:2 * k1, :],
                                in0=xpv[64:128, 2 * k0:2 * k1, 1:97:2],
                                in1=xpv[64:128, 2 * k0:2 * k1, 2:98:2],
                                op=mybir.AluOpType.add)
        # vertical pool pairs -> pl rows k0..k1-1
        nc.vector.tensor_tensor(out=pl[0:64, k0:k1, :],
                                in0=plh[0:64, 2 * k0:2 * k1:2, :],
                                in1=plh[0:64, 2 * k0 + 1:2 * k1:2, :],
                                op=mybir.AluOpType.max)
        nc.vector.tensor_tensor(out=pl[64:128, k0:k1, :],
                                in0=plh[64:128, 2 * k0:2 * k1:2, :],
                                in1=plh[64:128, 2 * k0 + 1:2 * k1:2, :],
                                op=mybir.AluOpType.add)
        if k0 == 0:
            nc.vector.tensor_copy(out=vint[:, 0, :], in_=pl[:, 0, :])
        # vertical bilinear rows that only need pl rows < k1
        ke_o = min(k1 - 1, 25)
        if ke_o > kv_o:
            a, b = kv_o, ke_o
            nc.vector.tensor_scalar(out=tscr[:, a:b, :], in0=pl[:, a + 1:b + 1, :],
                                    scalar1=0.25, scalar2=None,
                                    op0=mybir.AluOpType.mult)
            nc.vector.scalar_tensor_tensor(
                out=vint[:, 2 * a + 1:2 * b:2, :], in0=pl[:, a:b, :],
                scalar=0.75, in1=tscr[:, a:b, :],
                op0=mybir.AluOpType.mult, op1=mybir.AluOpType.add)
            kv_o = ke_o
        ke_e = min(k1 - 1, 24)
        if ke_e > kv_e:
            a, b = kv_e, ke_e
            nc.vector.tensor_scalar(out=tscr[:, a:b, :], in0=pl[:, a + 1:b + 1, :],
                                    scalar1=0.75, scalar2=None,
                                    op0=mybir.AluOpType.mult)
            nc.vector.scalar_tensor_tensor(
                out=vint[:, 2 * a + 2:2 * b + 1:2, :], in0=pl[:, a:b, :],
                scalar=0.25, in1=tscr[:, a:b, :],
                op0=mybir.AluOpType.mult, op1=mybir.AluOpType.add)
            kv_e = ke_e
        # horizontal pass over fully-available vint rows
        avail = min(2 * kv_o + 1, 2 * kv_e + 2) if k1 < 26 else TR
        hpass(hv, avail)
        hv = avail

    # brs rows (fp8) -> T slab slots; row-chunked for conv1 pipelining.
    # (ky slot s stores t row rho at tile row rho+1-s.)
    for (a, b) in ((0, 8), (8, 18), (18, 28), (28, 38), (38, 50)):
        nc.sync.dma_start(out=slabv(0, 0, 64, a, b), in_=brv[0:64, a:b, :])
        nc.sync.dma_start(out=slabv(1, 0, 64, a, b), in_=brv[64:128, a:b, :])
        nc.sync.dma_start(out=slabv(1, 64, 128, a, b),
                          in_=brv[0:64, a + 1:b + 1, :])
        nc.sync.dma_start(out=slabv(2, 64, 128, a, b),
                          in_=brv[64:128, a + 1:b + 1, :])
        bb = min(b, 49)
        nc.sync.dma_start(out=slabv(3, 0, 64, a, bb),
                          in_=brv[0:64, a + 2:bb + 2, :])
        nc.sync.dma_start(out=slabv(4, 0, 64, a, bb),
                          in_=brv[64:128, a + 2:bb + 2, :])

    # oa (= xpool reuse) borders for the depthwise reads; ob borders
    oav = xpv[:, 0:50, :]
    nc.vector.memset(oav[:, 0:1, :], 0.0)
    nc.vector.memset(oav[:, :, 0:1], 0.0)
    nc.vector.memset(oav[:, :, 97:98], 0.0)
    nc.vector.memset(obv[:, 0:1, :], 0.0)
    nc.vector.memset(obv[0:64, :, 1:2], 0.0)
    nc.vector.memset(obv[64:128, :, 0:1], 0.0)
    nc.vector.memset(obv[:, :, 97:98], 0.0)

    # ---------------- conv1 (fp8 DoubleRow) + attention prologue ----------
    qk_blocks = _row_blocks(SR)
    v_blocks = _row_blocks(QR)

    def conv1_block(ot, bi, r0, nr, evac):
        mw = 64 if ot == 4 else 128
        m0 = 128 * ot
        ps = pp1.tile([128, 5 * WPD], F32, tag="c1ps")
        for pi, (p0, p1) in enumerate(PAIRS_C1):
            nc.tensor.matmul(
                ps[0:mw, 0:nr * WPD], lhsT=w1s[:, pi, :, m0:m0 + mw],
                rhs=c1_rhs(tall_b, p0, p1, r0, nr),
                start=(pi == 0), stop=False,
                perf_mode=mybir.MatmulPerfMode.DoubleRow)
        for qi, (p0, p1) in enumerate(PAIRS_T4):
            nc.tensor.matmul(
                ps[0:mw, 0:nr * WPD], lhsT=w1s4[:, qi, :, m0:m0 + mw],
                rhs=c1_rhs(t4_b, p0, p1, r0, nr),
                start=False, stop=(qi == 1),
                perf_mode=mybir.MatmulPerfMode.DoubleRow)
        psv = ps.rearrange("p (r x) -> p r x", x=WPD)
        evac(bi, r0, nr, psv, mw)

    def conv1_tile(ot, blocks, evac):
        for bi, (r0, nr) in enumerate(blocks):
            conv1_block(ot, bi, r0, nr, evac)

    def evac_qk(ot):
        def f(bi, r0, nr, psv, mw):
            nc.scalar.activation(
                out=qkv[ot][:, r0:r0 + nr, :], in_=psv[:, 0:nr, 1:97],
                func=mybir.ActivationFunctionType.Relu,
                bias=sc1[ot][:, 1:2], scale=sc1[ot][:, 0:1])
            dump = ev.tile([128, 5 * W], BF16, tag="sqd")
            nc.vector.tensor_tensor(
                out=dump[:, 0:nr * W],
                in0=qk[ot][:, r0 * W:(r0 + nr) * W],
                in1=qk[ot][:, r0 * W:(r0 + nr) * W],
                op=mybir.AluOpType.mult)
            nc.vector.reduce_sum(out=sqac[ot][:, bi:bi + 1],
                                 in_=dump[:, 0:nr * W],
                                 axis=mybir.AxisListType.X)
        return f

    def evac_v(ot):
        off = 0 if ot == 3 else VS

        def f(bi, r0, nr, psv, mw):
            dst = vall[0:mw, off + r0 * W:off + (r0 + nr) * W]
            nc.scalar.activation(
                out=dst, in_=psv[0:mw, 0:nr, 1:97],
                func=mybir.ActivationFunctionType.Relu,
                bias=sc1[ot][:, 1:2], scale=sc1[ot][:, 0:1])
        return f

    with tc.tile_pool(name="ps_c1", bufs=3, space="PSUM") as pp1, \
         tc.tile_pool(name="ps_tr", bufs=2, space="PSUM") as ppt, \
         tc.tile_pool(name="ps_s", bufs=1, space="PSUM") as pps:

        # conv1 qk row-blocks interleaved across the 3 out-tiles, with S'
        # transpose groups issued as soon as their pixel chunks are covered.
        qk0r = qk[0].rearrange("p (c k) -> p c k", k=128)
        qk1r = qk[1].rearrange("p (c k) -> p c k", k=128)
        qk2r = qk[2].rearrange("p (c k) -> p c k", k=128)
        sp = pps.tile([128, 2 * C], F32, tag="sp")

        def s_group(g):
            tq = ppt.tile([128, 3 * C], BF16, tag="tq")
            tk = ppt.tile([128, 3 * C], BF16, tag="tk")
            tqv = tq.rearrange("p (i c) -> p i c", c=C)
            tkv = tk.rearrange("p (i c) -> p i c", c=C)
            for i in range(3):
                ci = 3 * g + i
                nc.tensor.transpose(tqv[:, i, 0:128], qk0r[:, ci, :], ident[:])
                nc.tensor.transpose(tqv[:, i, 128:192], qk1r[0:64, ci, :],
                                    ident[0:64, 0:64])
                nc.tensor.transpose(tkv[:, i, 0:64], qk1r[64:128, ci, :],
                                    ident[64:128, 64:128])
                nc.tensor.transpose(tkv[:, i, 64:192], qk2r[:, ci, :], ident[:])
            qtc = ev.tile([128, 3 * C], BF16, tag="qtc")
            ktc = ev.tile([128, 3 * C], BF16, tag="ktc")
            nc.scalar.copy(out=qtc[:], in_=tq[:])
            nc.scalar.copy(out=ktc[:], in_=tk[:])
            qcv = qtc.rearrange("p (i c) -> p i c", c=C)
            kcv = ktc.rearrange("p (i c) -> p i c", c=C)
            for i in range(3):
                nc.tensor.matmul(sp[:, 0:C], lhsT=kcv[:, i, 0:128],
                                 rhs=qcv[:, i, :],
                                 start=(g == 0 and i == 0),
                                 stop=(g == NCH // 3 - 1 and i == 2))
                nc.tensor.matmul(sp[0:64, C:2 * C], lhsT=kcv[:, i, 128:192],
                                 rhs=qcv[:, i, :],
                                 start=(g == 0 and i == 0),
                                 stop=(g == NCH // 3 - 1 and i == 2))

        next_g = 0
        for bi, (r0, nr) in enumerate(qk_blocks):
            for ot in range(3):
                conv1_block(ot, bi, r0, nr, evac_qk(ot))
            px_done = (r0 + nr) * W
            while next_g < NCH // 3 and (3 * next_g + 3) * 128 <= px_done:
                s_group(next_g)
                next_g += 1
        while next_g < NCH // 3:
            s_group(next_g)
            next_g += 1

        # deferred zero-clears (needed only from P@V onward): ptall plane1
        # upper rows are a zero K-pad for the P@V DR weights; vall plane1
        # upper rows are junk read by that zero plane -- must be non-NaN
        # (0 x NaN = NaN).
        nc.vector.memset(ptall[64:128, C:2 * C], 0.0)
        nc.vector.memset(vall[64:128, VS:2 * VS], 0.0)

        # sumsq totals + AllReduce staging
        for ti in range(3):
            nc.vector.reduce_sum(out=accs[ti][:], in_=sqac[ti][:],
                                 axis=mybir.AxisListType.X)
        nc.vector.tensor_copy(out=ssa[:, 192:193], in_=accs[0][:])
        nc.vector.tensor_copy(out=ssb[:, 192:193], in_=accs[1][0:64, :])
        nc.vector.tensor_copy(out=accb[:, 0:1], in_=accs[1][:])
        nc.vector.tensor_copy(out=accb[:, 1:2], in_=accs[2][:])
        nc.sync.dma_start(out=ssa[0:64, 193:194], in_=accb[64:128, 0:1])
        nc.sync.dma_start(out=ssa[64:128, 193:194], in_=accb[0:64, 1:2])
        nc.sync.dma_start(out=ssb[0:64, 193:194], in_=accb[64:128, 1:2])
        nc.scalar.copy(out=ssa[:, 0:192], in_=sp[:, 0:C])
        nc.scalar.copy(out=ssb[:, 0:192], in_=sp[0:64, C:2 * C])
        nc.sync.dma_start(out=cc2i[0:64, :], in_=ssa[0:64, :])
        nc.scalar.dma_start(out=cc2i[64:128, :], in_=ssa[64:128, :])
        nc.gpsimd.dma_start(out=cc2i[128:192, :], in_=ssb[:])
        nc.gpsimd.collective_compute(
            "AllReduce", mybir.AluOpType.add, replica_groups=GROUPS,
            ins=[cc2i[:]], outs=[cc2o[:]])
        # sumsq columns first: the norm chain needs only these
        nc.gpsimd.dma_start(out=sfa[:, 192:194], in_=cc2o[0:128, 192:194])
        nc.gpsimd.dma_start(out=sfb[:, 192:194], in_=cc2o[128:192, 192:194])
        nc.sync.dma_start(out=sfa[0:64, 0:192], in_=cc2o[0:64, 0:192])
        nc.scalar.dma_start(out=sfa[64:128, 0:192], in_=cc2o[64:128, 0:192])
        nc.sync.dma_start(out=sfb[:, 0:192], in_=cc2o[128:192, 0:192])

        # conv1 v tiles overlap the collective
        conv1_tile(3, v_blocks, evac_v(3))
        conv1_tile(4, v_blocks, evac_v(4))

    # ---------------- transpose-free softmax -> fp8 P^T planes ------------
    with tc.tile_pool(name="ps_sm", bufs=1, space="PSUM") as ppm, \
         tc.tile_pool(name="ps_pv", bufs=2, space="PSUM") as ppv, \
         tc.tile_pool(name="ps_dw", bufs=2, space="PSUM") as ppd:
        psq = ppm.tile([1, C], BF16, tag="psq")
        pm = ppm.tile([128, 392], F32, tag="pm")
        bcast = pm[:, 0:192]
        pcs = pm[0:1, 196:388]

        nc.tensor.transpose(psq[0:1, 0:128], sfa[:, 192:193], ident[:])
        nc.tensor.transpose(psq[0:1, 128:192], sfb[0:64, 192:193],
                            ident[0:64, 0:64])
        nc.vector.tensor_scalar(out=qs[:], in0=psq[0:1, :], scalar1=1e-24,
                                scalar2=None, op0=mybir.AluOpType.max)
        nc.vector.tensor_scalar(out=rska[:], in0=sfa[:, 193:194],
                                scalar1=1e-24, scalar2=None,
                                op0=mybir.AluOpType.max)
        nc.vector.tensor_scalar(out=rskb[:], in0=sfb[0:64, 193:194],
                                scalar1=1e-24, scalar2=None,
                                op0=mybir.AluOpType.max)
        for r in (qs, rska, rskb):
            nc.vector.reciprocal(out=r[:], in_=r[:])
            nc.scalar.activation(out=r[:], in_=r[:],
                                 func=mybir.ActivationFunctionType.Sqrt)
        nc.tensor.matmul(bcast, lhsT=ones1[:, :], rhs=qs[:],
                         start=True, stop=True)
        nc.vector.scalar_tensor_tensor(
            out=ea[:], in0=sfa[:, 0:192], scalar=tmps[:, 0:1], in1=bcast,
            op0=mybir.AluOpType.mult, op1=mybir.AluOpType.mult)
        nc.vector.scalar_tensor_tensor(
            out=eb[:], in0=sfb[0:64, 0:192], scalar=tmps[0:64, 0:1],
            in1=bcast[0:64, :],
            op0=mybir.AluOpType.mult, op1=mybir.AluOpType.mult)
        # P^T planes (unnormalized): plane0 = keys 0:128, plane1 = keys 128:192
        nc.scalar.activation(out=ptall[:, 0:C], in_=ea[:],
                             func=mybir.ActivationFunctionType.Exp,
                             bias=0.0, scale=rska[:, 0:1])
        nc.scalar.activation(out=ptall[0:64, C:2 * C], in_=eb[:],
                             func=mybir.ActivationFunctionType.Exp,
                             bias=0.0, scale=rskb[:, 0:1])
        # softmax denominator: colsum over keys via ones-matmul
        nc.tensor.matmul(pcs, lhsT=ones8[:, 0:1],
                         rhs=ptall[:, 0:C], start=True, stop=False)
        nc.tensor.matmul(pcs, lhsT=ones8[:, 0:1],
                         rhs=ptall[:, C:2 * C], start=False, stop=True)
        nc.vector.reciprocal(out=isr[:], in_=pcs)
        prt = pm[:, 192:194]
        nc.tensor.transpose(prt[0:128, 0:1], isr[0:1, 0:128],
                            identf[0:1, 0:1])
        nc.tensor.transpose(prt[0:64, 1:2], isr[0:1, 128:192],
                            identf[0:1, 0:1])
        nc.vector.tensor_copy(out=rsA[:], in_=prt[0:128, 0:1])
        nc.vector.tensor_copy(out=rsB[:], in_=prt[0:64, 1:2])

        # out = P^T.T @ v (fp8 DoubleRow over the two key planes), interleaved
        # with depthwise-A blocks so dw matmuls hide the P@V evac latency.
        ptv = ptall.rearrange("p (two q) -> p two q", two=2)
        va_b = vall[:]
        yv = yout.rearrange("c (r w) -> c r w", w=W)

        def pv_rhs(r0, nr):
            return bass.AP(tensor=va_b.tensor, offset=va_b.offset + r0 * W,
                           ap=[[va_b.ap[0][0], 128], [VS, 2], [1, nr * W]])

        def pv_block(r0, nr):
            po = ppv.tile([128, 5 * W], F32, tag="po")
            po2 = ppv.tile([64, 5 * W], F32, tag="po2")
            nc.tensor.matmul(po[:, 0:nr * W], lhsT=ptv[:, :, 0:128],
                             rhs=pv_rhs(r0, nr), start=True, stop=True,
                             perf_mode=mybir.MatmulPerfMode.DoubleRow)
            nc.tensor.matmul(po2[0:64, 0:nr * W], lhsT=ptv[:, :, 128:192],
                             rhs=pv_rhs(r0, nr), start=True, stop=True,
                             perf_mode=mybir.MatmulPerfMode.DoubleRow)
            pov = po.rearrange("p (r w) -> p r w", w=W)
            po2v = po2.rearrange("p (r w) -> p r w", w=W)
            nc.scalar.copy(out=oav[:, r0 + 1:r0 + 1 + nr, 1:97],
                           in_=pov[:, 0:nr, :])
            nc.vector.tensor_copy(out=obv[0:64, r0 + 1:r0 + 1 + nr, 2:98],
                                  in_=po2v[0:64, 0:nr, :])

        def dwa_block(r0, nr):
            ps = ppd.tile([128, 5, W], F32, tag="dwps")
            for t in range(9):
                dy, dx = t // 3 - 1, t % 3 - 1
                nc.tensor.matmul(
                    ps[:, 0:nr, :],
                    lhsT=w2av[:, t, :],
                    rhs=oav[:, r0 + 1 + dy:r0 + 1 + dy + nr, 1 + dx:97 + dx],
                    start=(t == 0), stop=(t == 8))
            fo = ev.tile([128, 5, W], F32, tag="fo")
            nc.scalar.activation(out=fo[:, 0:nr, :], in_=ps[:, 0:nr, :],
                                 func=mybir.ActivationFunctionType.Identity,
                                 bias=b2a[:, 0:1], scale=rsA[:, 0:1])
            nc.sync.dma_start(out=yv[0:128, r0:r0 + nr, :],
                              in_=fo[:, 0:nr, :])

        dw_blocks = _row_blocks(OR_)
        pv_block(*v_blocks[0])
        pv_block(*v_blocks[1])
        for bi in range(2, len(v_blocks)):
            dwa_block(*dw_blocks[bi - 2])
            pv_block(*v_blocks[bi])
            if bi == 5:
                # replicate ob rows 1..25 (needs P@V-B evacs through block 4)
                nc.sync.dma_start(out=obv[64:128, 1:26, 1:97],
                                  in_=obv[0:64, 1:26, 2:98])
        nc.sync.dma_start(out=obv[64:128, 26:50, 1:97],
                          in_=obv[0:64, 26:50, 2:98])
        dwa_block(*dw_blocks[8])
        dwa_block(*dw_blocks[9])
        for (r0, nr) in dw_blocks:
            ps = ppd.tile([128, 5, W], F32, tag="dwps")
            for dy in range(3):
                nc.tensor.matmul(
                    ps[0:64, 0:nr, :], lhsT=w2fv[:, dy, :],
                    rhs=obv[:, r0 + dy:r0 + dy + nr, 1:97],
                    start=(dy == 0), stop=False)
                nc.tensor.matmul(
                    ps[0:64, 0:nr, :], lhsT=w2gv[64:128, dy, :],
                    rhs=obv[64:128, r0 + dy:r0 + dy + nr, 2:98],
                    start=False, stop=(dy == 2))
            fo = ev.tile([128, 5, W], F32, tag="fo")
            nc.scalar.activation(out=fo[0:64, 0:nr, :], in_=ps[0:64, 0:nr, :],
                                 func=mybir.ActivationFunctionType.Identity,
                                 bias=b2b[:, 0:1], scale=rsB[:, 0:1])
            nc.sync.dma_start(out=yv[128:192, r0:r0 + nr, :],
                              in_=fo[0:64, 0:nr, :])
    ctx.close()


# ---------------- host side ----------------
_NC_CACHE = None


def _get_nc():
    global _NC_CACHE
    if _NC_CACHE is None:
        _NC_CACHE = build_nc()
    return _NC_CACHE


def _pack_weights(inp, flip):
    bf = ml_dtypes.bfloat16
    w0 = inp["w0"][:, :, ::-1, :] if flip else inp["w0"]
    w1 = inp["w1"][:, :, ::-1, :] if flip else inp["w1"]
    w2 = inp["w2"][:, :, ::-1, :] if flip else inp["w2"]
    w0 = np.asarray(w0, np.float32)
    w1 = np.asarray(w1, np.float32)
    w2 = np.asarray(w2, np.float32)

    # conv0: out-channel order [x1(0:64), x3(128:192), x2(64:128)]
    cho = np.concatenate([np.arange(0, 64), np.arange(128, 192),
                          np.arange(64, 128)])
    wt = w0[cho].transpose(1, 2, 3, 0)        # [64c, 3ky, 3kx, 192m]
    w0p = np.zeros((128, 3, 2, C), np.float32)
    w0p[0:64, 0, 0] = wt[:, 0, 0]
    w0p[0:64, 0, 1] = wt[:, 0, 2]
    w0p[0:64, 1, 0] = wt[:, 0, 1]
    w0p[64:128, 0, 0] = wt[:, 1, 0]
    w0p[64:128, 0, 1] = wt[:, 1, 2]
    w0p[64:128, 1, 0] = wt[:, 1, 1]
    w0p[64:128, 1, 1] = wt[:, 2, 1]
    w0p[64:128, 2, 0] = wt[:, 2, 0]
    w0p[64:128, 2, 1] = wt[:, 2, 2]
    s0 = inp["g0"] / np.sqrt(inp["v0"] + BN_EPS)
    t0 = inp["be0"] + (inp["b0"] - inp["m0"]) * s0
    sb0 = np.stack([s0 / W0S, t0], axis=1).astype(np.float32)[cho]

    # conv1 DoubleRow packs.  Slab k-partition -> (ky, c) maps:
    def slab_map(s):
        k = np.arange(128 if s < 4 else 64)
        if s == 0:
            return np.zeros_like(k), k
        if s == 1:
            return np.where(k < 64, 0, 1), np.where(k < 64, 128 + k, k - 64)
        if s == 2:
            return np.ones_like(k), 64 + k
        if s == 3:
            return np.full_like(k, 2), k
        return np.full_like(k, 2), 128 + k

    wt1 = w1.transpose(1, 2, 3, 0)            # [192c, 3ky, 3kx, 576m]
    w1p = np.zeros((128, 8, 2, C3), np.float32)
    for pi, (p0, p1) in enumerate(PAIRS_C1):
        for pl, spec in enumerate((p0, p1)):
            if spec is None:
                continue
            s, dx = spec
            ky, cc = slab_map(s)
            w1p[:, pi, pl] = wt1[cc, ky, dx]
    w1p4 = np.zeros((64, 2, 2, C3), np.float32)
    ky4, cc4 = slab_map(4)
    w1p4[:, 0, 0] = wt1[cc4, ky4, 0]
    w1p4[:, 0, 1] = wt1[cc4, ky4, 2]
    w1p4[:, 1, 0] = wt1[cc4, ky4, 1]
    s1 = inp["g1"] / np.sqrt(inp["v1"] + BN_EPS)
    t1 = inp["be1"] + (inp["b1"] - inp["m1"]) * s1
    sb1 = np.stack([s1 / W1S, t1], axis=1).astype(np.float32)

    w2da = np.zeros((128, 9, 128), np.float32)
    w2fb = np.zeros((128, 3, 64), np.float32)
    w2gb = np.zeros((128, 3, 64), np.float32)
    r64, r128 = np.arange(64), np.arange(128)
    for t in range(9):
        d = w2[:, 0, t // 3, t % 3]
        w2da[r128, t, r128] = d[0:128]
    for dy in range(3):
        db = w2[128:192, 0, dy, :]
        w2fb[r64, dy, r64] = db[:, 0]
        w2fb[64 + r64, dy, r64] = db[:, 1]
        w2gb[64 + r64, dy, r64] = db[:, 2]

    out = {
        "w0p": np.ascontiguousarray(
            (w0p * W0S).reshape(128, 3 * 2 * C)).astype(f8np),
        "sb0p": sb0,
        "w1p": np.ascontiguousarray(
            (w1p * W1S).reshape(128, 8 * 2 * C3)).astype(f8np),
        "w1p4": np.ascontiguousarray(
            (w1p4 * W1S).reshape(64, 2 * 2 * C3)).astype(f8np),
        "sb1": sb1,
        "w2da": np.ascontiguousarray(w2da.reshape(128, 9 * 128)).astype(bf),
        "w2fb": np.ascontiguousarray(w2fb.reshape(128, 3 * 64)).astype(bf),
        "w2gb": np.ascontiguousarray(w2gb.reshape(128, 3 * 64)).astype(bf),
        "b2v": np.asarray(inp["b2"], np.float32).reshape(C, 1),
    }
    return out


def kernel(**inputs):
    inputs = {k: np.asarray(v) for k, v in inputs.items()}
    x = inputs["x"]
    B = x.shape[0]
    packs = [_pack_weights(inputs, flip) for flip in (False, True)]
    tempv = np.asarray(inputs["temp"], np.float32).reshape(1, 1)

    in_maps = []
    for core in range(8):
        s, h = core // 2, core % 2
        xi = np.asarray(x[s], np.float32)
        if h:
            xi = xi[:, ::-1, :]
        slab = np.zeros((64, XR, WPD), np.float32)
        slab[:, 1:54, 1:97] = xi[:, 0:53, :]
        m = dict(packs[h])
        m["xs"] = np.ascontiguousarray(slab.reshape(64, XR * WPD)).astype(f8np)
        m["tempv"] = tempv
        m["zpad8"] = np.zeros((1, 512), f8np)
        m["zpad16"] = np.zeros((1, 512), ml_dtypes.bfloat16)
        in_maps.append(m)

    nc = _get_nc()
    res = run_bass_kernel_spmd(nc, in_maps, list(range(8)))
    out = np.zeros((B, C, 96, 96), np.float32)
    for core in range(8):
        s, h = core // 2, core % 2
        yc = res.results[core]["yout"].reshape(C, OR_, W)
        if h:
            out[s, :, 48:96] = yc[:, ::-1, :]
        else:
            out[s, :, 0:48] = yc
    return out


# revision 36
# speedup vs baseline: 1.0059x; 1.0059x over previous
"""Bass/Trainium2 kernel for nn_CSEM sparse_attention problem (v3).

Sharding: 8 cores = 4 samples x 2 spatial halves (bottom half vertically
flipped on host so all cores run an identical NEFF).

v3 vs v2: fp8e4 DoubleRow matmuls for conv0 / conv1 / P@V (one DR matmul
accumulates two (weights-plane, ifmap-plane) K-tiles at 0.5 cycles/row).
Activations and weights for those convs are fp8; conv taps are addressed as
column/row offsets into padded flat slabs so tap pairs become stride-`d`
plane pairs of a single AP. Softmax is transpose-free: S'^T stays key-major,
q-norms are broadcast along partitions, exp goes straight to fp8 P^T planes,
and the softmax denominator (from an ones-matmul colsum) is folded into the
depthwise-conv evacuation scale. Depthwise conv + q/k transposes + S' stay
bf16 for accuracy.
"""

import numpy as np
import ml_dtypes

import concourse.bass as bass
import concourse.mybir as mybir
import concourse.tile as tile
from concourse.bass_utils import run_bass_kernel_spmd
from concourse.masks import make_identity

F8 = mybir.dt.float8e4
BF16 = mybir.dt.bfloat16
F32 = mybir.dt.float32
BN_EPS = 1e-5
f8np = ml_dtypes.float8_e4m3

CIN, C, C3 = 64, 192, 576
W, WPD = 96, 98
XR = 54          # x slab rows (1 zero pad + 53 data)
YR = 52          # y rows computed locally (0..51)
TR = 50          # t rows 0..49
QR = 49          # conv1 v rows 0..48
SR = 48          # rows feeding S partial
OR_ = 48         # final output rows per core
NCH = SR * W // 128   # 36 transpose chunks
VS = QR * W      # v plane stride (4704)
SL = 4908        # Tall slab stride (1 lead + 50*98 data + slack)
XLEN = 1 + XR * WPD + 8
W0S, W1S = 32.0, 64.0
GROUPS = [[0, 1], [2, 3], [4, 5], [6, 7]]

# conv1 DoubleRow pairs as ((slab, dx), (slab, dx)); None = zero-weight dummy
# plane (rhs offset +2 -- a dim1 stride equal to the element stride wedges the
# device). Pairs stay within one slab so each block's read footprint is
# row-local and conv1 pipelines with the bilinear-branch writes.
PAIRS_C1 = [((0, 0), (0, 2)), ((0, 1), None), ((1, 0), (1, 2)),
            ((1, 1), None), ((2, 0), (2, 2)), ((2, 1), None),
            ((3, 0), (3, 2)), ((3, 1), None)]
PAIRS_T4 = [((0, 0), (0, 2)), ((0, 1), None)]   # slab idx relative to T4


def _split_waits(nc, limit=1):
    """This walrus build rejects instructions carrying more than one sem-wait
    command. Spread extra waits onto same-engine ENGINE_NOPs inserted just
    before the offending instruction (semantically identical: the engine
    blocks on each wait in program order)."""
    ctr = [0]
    for f in nc.m.functions:
        for blk in f.blocks:
            il = blk.instructions
            new = []
            for inst in il:
                si = inst.sync_info
                waits = list(si.on_wait) if (si and si.on_wait) else []
                if len(waits) > limit:
                    for w in waits[:-limit]:
                        ctr[0] += 1
                        nop = mybir.InstNoOp(name=f"WNOP-{ctr[0]}")
                        nop.engine = inst.engine
                        nop.sync_info = mybir.SyncInfo(on_wait=[w], on_update=[])
                        new.append(nop)
                    si.on_wait = waits[-limit:]
                new.append(inst)
            il[:] = new


def _row_blocks(nrows, per=5):
    out, r = [], 0
    while r < nrows:
        n = min(per, nrows - r)
        out.append((r, n))
        r += n
    return out


def build_nc():
    nc = bass.Bass()

    xs_d = nc.declare_dram_parameter("xs", [64, XR * WPD], F8, isOutput=False)
    w0_d = nc.declare_dram_parameter("w0p", [128, 3 * 2 * C], F8, isOutput=False)
    sb0_d = nc.declare_dram_parameter("sb0p", [C, 2], F32, isOutput=False)
    w1_d = nc.declare_dram_parameter("w1p", [128, 8 * 2 * C3], F8, isOutput=False)
    w14_d = nc.declare_dram_parameter("w1p4", [64, 2 * 2 * C3], F8, isOutput=False)
    sb1_d = nc.declare_dram_parameter("sb1", [C3, 2], F32, isOutput=False)
    w2a_d = nc.declare_dram_parameter("w2da", [128, 9 * 128], BF16, isOutput=False)
    w2f_d = nc.declare_dram_parameter("w2fb", [128, 3 * 64], BF16, isOutput=False)
    w2g_d = nc.declare_dram_parameter("w2gb", [128, 3 * 64], BF16, isOutput=False)
    b2_d = nc.declare_dram_parameter("b2v", [C, 1], F32, isOutput=False)
    tmp_d = nc.declare_dram_parameter("tempv", [1, 1], F32, isOutput=False)
    z8_d = nc.declare_dram_parameter("zpad8", [1, 512], F8, isOutput=False)
    z16_d = nc.declare_dram_parameter("zpad16", [1, 512], BF16, isOutput=False)
    yout = nc.declare_dram_parameter("yout", [C, OR_ * W], F32, isOutput=True)

    cc2i = nc.dram_tensor("cc2i", [C, 194], BF16)
    cc2o = nc.dram_tensor("cc2o", [C, 194], BF16)

    with tile.TileContext(nc) as tc:
        _body(nc, tc, xs_d, w0_d, sb0_d, w1_d, w14_d, sb1_d, w2a_d, w2f_d,
              w2g_d, b2_d, tmp_d, z8_d, z16_d, yout, cc2i, cc2o)
    _split_waits(nc)
    return nc


def _body(nc, tc, xs_d, w0_d, sb0_d, w1_d, w14_d, sb1_d, w2a_d, w2f_d,
          w2g_d, b2_d, tmp_d, z8_d, z16_d, yout, cc2i, cc2o):
    import contextlib
    ctx = contextlib.ExitStack()
    P = ctx.enter_context(tc.tile_pool(name="persist", bufs=1))
    ev = ctx.enter_context(tc.tile_pool(name="evac", bufs=3))

    # ---- persistent SBUF ----
    xfold = P.tile([128, XLEN], F8, tag="xfold")
    w0s = P.tile([128, 3, 2, C], F8, tag="w0s")
    w1s = P.tile([128, 8, 2, C3], F8, tag="w1s")
    w1s4 = P.tile([64, 2, 2, C3], F8, tag="w1s4")
    sc0a = P.tile([128, 2], F32, tag="sc0a")
    sc0b = P.tile([64, 2], F32, tag="sc0b")
    scp = [128, 128, 128, 128, 64]
    sc1 = [P.tile([scp[i], 2], F32, tag=f"sc1_{i}", name=f"sc1_{i}")
           for i in range(5)]
    w2da = P.tile([128, 9 * 128], BF16, tag="w2da")
    w2fb = P.tile([128, 3 * 64], BF16, tag="w2fb")
    w2gb = P.tile([128, 3 * 64], BF16, tag="w2gb")  # data at partitions 64..
    b2a = P.tile([128, 1], F32, tag="b2a")
    b2b = P.tile([64, 1], F32, tag="b2b")
    tmps = P.tile([128, 1], F32, tag="tmps")

    Tall = P.tile([128, 4 * SL], F8, tag="Tall")
    T4 = P.tile([64, SL], F8, tag="T4")
    xpool = P.tile([128, YR * WPD], BF16, tag="xpool")   # later reused as oa
    plh = P.tile([128, YR, 48], BF16, tag="plh")
    pl = P.tile([128, 26, 48], BF16, tag="pl")
    vint = P.tile([128, TR, 48], BF16, tag="vint")
    tscr = P.tile([128, TR, 48], BF16, tag="tscr")
    brs = P.tile([128, 51 * WPD], F8, tag="brs")
    ob = P.tile([128, TR * WPD], BF16, tag="ob")
    qk = [P.tile([128, SR * W], BF16, tag=f"qk{i}", name=f"qk{i}")
          for i in range(3)]
    vall = P.tile([128, 2 * VS], F8, tag="vall")
    sqac = [P.tile([128, 10], F32, tag=f"sqac{i}", name=f"sqac{i}")
            for i in range(3)]
    accs = [P.tile([128, 1], F32, tag=f"accs{i}", name=f"accs{i}")
            for i in range(3)]
    ssa = P.tile([128, 194], BF16, tag="ssa")
    ssb = P.tile([64, 194], BF16, tag="ssb")
    sfa = P.tile([128, 194], BF16, tag="sfa")
    sfb = P.tile([64, 194], BF16, tag="sfb")
    accb = P.tile([128, 2], BF16, tag="accb")
    rska = P.tile([128, 1], F32, tag="rska")
    rskb = P.tile([64, 1], F32, tag="rskb")
    qs = P.tile([1, C], F32, tag="qs")
    rqbc = P.tile([128, C], F32, tag="rqbc")
    ea = P.tile([128, C], F32, tag="ea")
    eb = P.tile([64, C], F32, tag="eb")
    ptall = P.tile([128, 2 * C], F8, tag="ptall")
    isr = P.tile([1, C], F32, tag="isr")
    rsA = P.tile([128, 1], F32, tag="rsA")
    rsB = P.tile([64, 1], F32, tag="rsB")
    ones8 = P.tile([128, 1], F8, tag="ones8")
    ones1 = P.tile([1, 128], F32, tag="ones1")

    ident = P.tile([128, 128], BF16, tag="ident")
    make_identity(nc, ident[:])
    identf = P.tile([128, 128], F32, tag="identf")
    make_identity(nc, identf[:])
    nc.vector.memset(ones8[:], 1.0)
    nc.vector.memset(ones1[:], 1.0)

    # ---- input DMAs (x chunked so conv0 starts early) ----
    nc.sync.dma_start(out=w0s[:], in_=w0_d.rearrange("p (a b m) -> p a b m",
                                                     a=3, b=2))
    nc.gpsimd.dma_start(out=sc0a[:], in_=sb0_d[0:128, :])
    nc.gpsimd.dma_start(out=sc0b[:], in_=sb0_d[128:192, :])
    for (a, b) in ((0, 8), (8, 32), (32, XR)):
        nc.sync.dma_start(out=xfold[0:64, 1 + a * WPD:1 + b * WPD],
                          in_=xs_d[:, a * WPD:b * WPD])
        bb = min(b, XR - 1)
        nc.sync.dma_start(out=xfold[64:128, 1 + a * WPD:1 + bb * WPD],
                          in_=xs_d[:, (a + 1) * WPD:(bb + 1) * WPD])
    nc.gpsimd.dma_start(out=w1s[:], in_=w1_d.rearrange("p (a b m) -> p a b m",
                                                     a=8, b=2))
    nc.gpsimd.dma_start(out=w1s4[:], in_=w14_d.rearrange(
        "p (a b m) -> p a b m", a=2, b=2))
    for i, (lo, hi) in enumerate([(0, 128), (128, 256), (256, 384),
                                  (384, 512), (512, 576)]):
        nc.gpsimd.dma_start(out=sc1[i][:], in_=sb1_d[lo:hi, :])
    nc.gpsimd.dma_start(out=w2da[:], in_=w2a_d[:])
    nc.gpsimd.dma_start(out=w2fb[:], in_=w2f_d[:])
    nc.gpsimd.dma_start(out=w2gb[:], in_=w2g_d[:])
    nc.gpsimd.dma_start(out=b2a[:], in_=b2_d[0:128, :])
    nc.gpsimd.dma_start(out=b2b[:], in_=b2_d[128:192, :])
    nc.gpsimd.dma_start(
        out=tmps[:],
        in_=bass.AP(tensor=tmp_d, offset=0, ap=[[0, 128], [1, 1]]))

    # ---- border memsets (DVE; only the load-bearing zeros) ----
    # xfold lead/tail and Tall slab leads/tails are read only by discarded
    # junk output columns -- no clears needed there. T4 row 49 IS needed:
    # the zero-weight dummy plane reads it (0 x NaN = NaN).
    tall_b = Tall[:]
    t4_b = T4[:]

    def slabv(s, plo, phi, r0, r1, c0=0, c1=WPD):
        """[phi-plo, r1-r0, c1-c0] view of slab s rows r0..r1."""
        base = tall_b if s < 4 else t4_b
        off = (s % 4) * SL if s < 4 else 0
        ap0 = base.ap[0][0]
        return bass.AP(tensor=base.tensor,
                       offset=base.offset + plo * ap0 + off + 1 + r0 * WPD + c0,
                       ap=[[ap0, phi - plo], [WPD, r1 - r0], [1, c1 - c0]])

    for s in range(4):
        nc.vector.memset(slabv(s, 0, 128, 0, 1), 0.0)              # row 0
        nc.vector.memset(slabv(s, 0, 128, 0, TR, 0, 1), 0.0)       # col 0
        nc.vector.memset(slabv(s, 0, 128, 0, TR, 97, 98), 0.0)     # col 97
    nc.vector.memset(T4[:, 1 + 49 * WPD:SL], 0.0)   # row 49 + slack
    nc.vector.memset(slabv(4, 0, 64, 0, 1), 0.0)
    nc.vector.memset(slabv(4, 0, 64, 0, 49, 0, 1), 0.0)
    nc.vector.memset(slabv(4, 0, 64, 0, 49, 97, 98), 0.0)

    brv = brs.rearrange("p (r w) -> p r w", w=WPD)
    nc.vector.memset(brv[:, 0:1, :], 0.0)
    nc.vector.memset(brv[:, :, 0:1], 0.0)
    nc.vector.memset(brv[:, :, 97:98], 0.0)

    xpv = xpool.rearrange("p (r w) -> p r w", w=WPD)
    obv = ob.rearrange("p (r w) -> p r w", w=WPD)
    qkv = [t.rearrange("p (r w) -> p r w", w=W) for t in qk]
    w2av = w2da.rearrange("p (t m) -> p t m", t=9)
    w2fv = w2fb.rearrange("p (t m) -> p t m", t=3)
    w2gv = w2gb.rearrange("p (t m) -> p t m", t=3)

    xf_b = xfold[:]

    def c0_rhs(s0, d0, s1, d1, nr):
        o0 = 1 + s0 * WPD + d0 - 1
        o1 = 1 + s1 * WPD + d1 - 1
        return bass.AP(tensor=xf_b.tensor, offset=xf_b.offset + o0,
                       ap=[[xf_b.ap[0][0], 128], [o1 - o0, 2], [1, nr * WPD]])

    def c1_rhs(pairs_base, p0, p1, r0, nr):
        (s0, d0) = p0
        if p1 is None:
            s1, d1 = s0, d0 + 2
        else:
            s1, d1 = p1
        off = (lambda s, d: s * SL + 1 + r0 * WPD + d - 1)
        o0, o1 = off(s0, d0), off(s1, d1)
        return bass.AP(tensor=pairs_base.tensor, offset=pairs_base.offset + o0,
                       ap=[[pairs_base.ap[0][0], pairs_base.ap[0][1]],
                           [o1 - o0, 2], [1, nr * WPD]])

    # ---------------- conv0 (fp8 DoubleRow) ----------------
    # out-tile A (m 0:128): x1|x3 -> xpool (bf16); out-tile B (m 128:192):
    # x2 -> Tall slab2 lower partitions (fp8)
    blocksA = _row_blocks(YR)
    blocksB = _row_blocks(TR)
    with tc.tile_pool(name="ps_c0", bufs=4, space="PSUM") as pp0:
        def conv0_block(r0, nr, m0, mw):
            ps = pp0.tile([128, 5 * WPD], F32, tag="c0ps")
            specs = ((r0, 0, r0, 2), (r0, 1, r0 + 1, 1),
                     (r0 + 1, 0, r0 + 1, 2))
            for pi, (s0, d0, s1, d1) in enumerate(specs):
                nc.tensor.matmul(
                    ps[0:mw, 0:nr * WPD], lhsT=w0s[:, pi, :, m0:m0 + mw],
                    rhs=c0_rhs(s0, d0, s1, d1, nr),
                    start=(pi == 0), stop=(pi == 2),
                    perf_mode=mybir.MatmulPerfMode.DoubleRow)
            return ps

        for (r0, nr) in blocksA:
            ps = conv0_block(r0, nr, 0, 128)
            psv = ps.rearrange("p (r x) -> p r x", x=WPD)
            nc.scalar.activation(
                out=xpv[:, r0:r0 + nr, 1:97], in_=psv[:, 0:nr, 1:97],
                func=mybir.ActivationFunctionType.Relu,
                bias=sc0a[:, 1:2], scale=sc0a[:, 0:1])
        for (r0, nr) in blocksB:
            ps = conv0_block(r0, nr, 128, 64)
            psv = ps.rearrange("p (r x) -> p r x", x=WPD)
            # x2 -> ky1 slot = slab2 partitions 0..63 (t rows at tile rows)
            nc.scalar.activation(
                out=slabv(2, 0, 64, r0, r0 + nr, 1, 97),
                in_=psv[0:64, 0:nr, 1:97],
                func=mybir.ActivationFunctionType.Relu,
                bias=sc0b[:, 1:2], scale=sc0b[:, 0:1])

    # x2 ky-shifted slot copies (slab2 col pads are zero so full width)
    for (a, b) in ((0, 10), (10, 30), (30, 49)):
        nc.sync.dma_start(out=slabv(0, 64, 128, a + 1, b + 1),
                          in_=slabv(2, 0, 64, a, b))
        nc.sync.dma_start(out=slabv(3, 64, 128, a, b),
                          in_=slabv(2, 0, 64, a + 1, b + 1))

    # ---------------- pools + bilinear (DVE; final writes fp8 brs) --------
    cA = P.tile([128, 2], F32, tag="cA")
    nc.vector.memset(cA[0:64, 0:1], 0.75)
    nc.vector.memset(cA[0:64, 1:2], 0.25)
    nc.vector.memset(cA[64:128, 0:1], 0.1875)
    nc.vector.memset(cA[64:128, 1:2], 0.0625)
    cC = P.tile([128, 1], F32, tag="cC")
    nc.vector.memset(cC[0:64, :], 1.0)
    nc.vector.memset(cC[64:128, :], 0.25)

    pl_chunks = [(0, 5), (5, 10), (10, 15), (15, 20), (20, 25), (25, 26)]
    kv_o = 0   # next odd-row k (vint[2k+1], k<=24)
    kv_e = 0   # next even-row k (vint[2k+2], k<=23)
    hv = 0     # next t-row for the horizontal pass

    def hpass(a, b):
        if b <= a:
            return
        nc.vector.tensor_scalar(out=brv[:, 1 + a:1 + b, 1:2],
                                in0=vint[:, a:b, 0:1], scalar1=cC[:, 0:1],
                                scalar2=None, op0=mybir.AluOpType.mult)
        nc.vector.tensor_scalar(out=brv[:, 1 + a:1 + b, 96:97],
                                in0=vint[:, a:b, 47:48], scalar1=cC[:, 0:1],
                                scalar2=None, op0=mybir.AluOpType.mult)
        nc.vector.tensor_scalar(out=tscr[:, a:b, 0:47], in0=vint[:, a:b, 1:48],
                                scalar1=cA[:, 1:2], scalar2=None,
                                op0=mybir.AluOpType.mult)
        nc.vector.scalar_tensor_tensor(
            out=brv[:, 1 + a:1 + b, 2:96:2], in0=vint[:, a:b, 0:47],
            scalar=cA[:, 0:1], in1=tscr[:, a:b, 0:47],
            op0=mybir.AluOpType.mult, op1=mybir.AluOpType.add)
        nc.vector.tensor_scalar(out=tscr[:, a:b, 0:47], in0=vint[:, a:b, 1:48],
                                scalar1=cA[:, 0:1], scalar2=None,
                                op0=mybir.AluOpType.mult)
        nc.vector.scalar_tensor_tensor(
            out=brv[:, 1 + a:1 + b, 3:96:2], in0=vint[:, a:b, 0:47],
            scalar=cA[:, 1:2], in1=tscr[:, a:b, 0:47],
            op0=mybir.AluOpType.mult, op1=mybir.AluOpType.add)

    for (k0, k1) in pl_chunks:
        # horizontal pool pairs for y rows 2k0..2k1-1
        nc.vector.tensor_tensor(out=plh[0:64, 2 * k0:2 * k1, :],
                                in0=xpv[0:64, 2 * k0:2 * k1, 1:97:2],
                                in1=xpv[0:64, 2 * k0:2 * k1, 2:98:2],
                                op=mybir.AluOpType.max)
        nc.vector.tensor_tensor(out=plh[64:128, 2 * k0:2 * k1, :],
                                in0=xpv[64:128, 2 * k0:2 * k1, 1:97:2],
                                in1=xpv[64:128, 2 * k0:2 * k1, 2:98:2],
                                op=mybir.AluOpType.add)
        # vertical pool pairs -> pl rows k0..k1-1
        nc.vector.tensor_tensor(out=pl[0:64, k0:k1, :],
                                in0=plh[0:64, 2 * k0:2 * k1:2, :],
                                in1=plh[0:64, 2 * k0 + 1:2 * k1:2, :],
                                op=mybir.AluOpType.max)
        nc.vector.tensor_tensor(out=pl[64:128, k0:k1, :],
                                in0=plh[64:128, 2 * k0:2 * k1:2, :],
                                in1=plh[64:128, 2 * k0 + 1:2 * k1:2, :],
                                op=mybir.AluOpType.add)
        if k0 == 0:
            nc.vector.tensor_copy(out=vint[:, 0, :], in_=pl[:, 0, :])
        # vertical bilinear rows that only need pl rows < k1
        ke_o = min(k1 - 1, 25)
        if ke_o > kv_o:
            a, b = kv_o, ke_o
            nc.vector.tensor_scalar(out=tscr[:, a:b, :], in0=pl[:, a + 1:b + 1, :],
                                    scalar1=0.25, scalar2=None,
                                    op0=mybir.AluOpType.mult)
            nc.vector.scalar_tensor_tensor(
                out=vint[:, 2 * a + 1:2 * b:2, :], in0=pl[:, a:b, :],
                scalar=0.75, in1=tscr[:, a:b, :],
                op0=mybir.AluOpType.mult, op1=mybir.AluOpType.add)
            kv_o = ke_o
        ke_e = min(k1 - 1, 24)
        if ke_e > kv_e:
            a, b = kv_e, ke_e
            nc.vector.tensor_scalar(out=tscr[:, a:b, :], in0=pl[:, a + 1:b + 1, :],
                                    scalar1=0.75, scalar2=None,
                                    op0=mybir.AluOpType.mult)
            nc.vector.scalar_tensor_tensor(
                out=vint[:, 2 * a + 2:2 * b + 1:2, :], in0=pl[:, a:b, :],
                scalar=0.25, in1=tscr[:, a:b, :],
                op0=mybir.AluOpType.mult, op1=mybir.AluOpType.add)
            kv_e = ke_e
        # horizontal pass over fully-available vint rows
        avail = min(2 * kv_o + 1, 2 * kv_e + 2) if k1 < 26 else TR
        hpass(hv, avail)
        hv = avail

    # brs rows (fp8) -> T slab slots; row-chunked for conv1 pipelining.
    # (ky slot s stores t row rho at tile row rho+1-s.)
    for (a, b) in ((0, 8), (8, 18), (18, 28), (28, 38), (38, 50)):
        nc.sync.dma_start(out=slabv(0, 0, 64, a, b), in_=brv[0:64, a:b, :])
        nc.sync.dma_start(out=slabv(1, 0, 64, a, b), in_=brv[64:128, a:b, :])
        nc.sync.dma_start(out=slabv(1, 64, 128, a, b),
                          in_=brv[0:64, a + 1:b + 1, :])
        nc.sync.dma_start(out=slabv(2, 64, 128, a, b),
                          in_=brv[64:128, a + 1:b + 1, :])
        bb = min(b, 49)
        nc.sync.dma_start(out=slabv(3, 0, 64, a, bb),
                          in_=brv[0:64, a + 2:bb + 2, :])
        nc.sync.dma_start(out=slabv(4, 0, 64, a, bb),
                          in_=brv[64:128, a + 2:bb + 2, :])

    # oa (= xpool reuse) borders for the depthwise reads; ob borders
    oav = xpv[:, 0:50, :]
    nc.vector.memset(oav[:, 0:1, :], 0.0)
    nc.vector.memset(oav[:, :, 0:1], 0.0)
    nc.vector.memset(oav[:, :, 97:98], 0.0)
    nc.vector.memset(obv[:, 0:1, :], 0.0)
    nc.vector.memset(obv[0:64, :, 1:2], 0.0)
    nc.vector.memset(obv[64:128, :, 0:1], 0.0)
    nc.vector.memset(obv[:, :, 97:98], 0.0)

    # ---------------- conv1 (fp8 DoubleRow) + attention prologue ----------
    qk_blocks = _row_blocks(SR)
    v_blocks = _row_blocks(QR)

    def conv1_block(ot, bi, r0, nr, evac):
        mw = 64 if ot == 4 else 128
        m0 = 128 * ot
        ps = pp1.tile([128, 5 * WPD], F32, tag="c1ps")
        for pi, (p0, p1) in enumerate(PAIRS_C1):
            nc.tensor.matmul(
                ps[0:mw, 0:nr * WPD], lhsT=w1s[:, pi, :, m0:m0 + mw],
                rhs=c1_rhs(tall_b, p0, p1, r0, nr),
                start=(pi == 0), stop=False,
                perf_mode=mybir.MatmulPerfMode.DoubleRow)
        for qi, (p0, p1) in enumerate(PAIRS_T4):
            nc.tensor.matmul(
                ps[0:mw, 0:nr * WPD], lhsT=w1s4[:, qi, :, m0:m0 + mw],
                rhs=c1_rhs(t4_b, p0, p1, r0, nr),
                start=False, stop=(qi == 1),
                perf_mode=mybir.MatmulPerfMode.DoubleRow)
        psv = ps.rearrange("p (r x) -> p r x", x=WPD)
        evac(bi, r0, nr, psv, mw)

    def conv1_tile(ot, blocks, evac):
        for bi, (r0, nr) in enumerate(blocks):
            conv1_block(ot, bi, r0, nr, evac)

    def evac_qk(ot):
        def f(bi, r0, nr, psv, mw):
            nc.scalar.activation(
                out=qkv[ot][:, r0:r0 + nr, :], in_=psv[:, 0:nr, 1:97],
                func=mybir.ActivationFunctionType.Relu,
                bias=sc1[ot][:, 1:2], scale=sc1[ot][:, 0:1])
            dump = ev.tile([128, 5 * W], BF16, tag="sqd")
            nc.vector.tensor_tensor(
                out=dump[:, 0:nr * W],
                in0=qk[ot][:, r0 * W:(r0 + nr) * W],
                in1=qk[ot][:, r0 * W:(r0 + nr) * W],
                op=mybir.AluOpType.mult)
            nc.vector.reduce_sum(out=sqac[ot][:, bi:bi + 1],
                                 in_=dump[:, 0:nr * W],
                                 axis=mybir.AxisListType.X)
        return f

    def evac_v(ot):
        off = 0 if ot == 3 else VS

        def f(bi, r0, nr, psv, mw):
            dst = vall[0:mw, off + r0 * W:off + (r0 + nr) * W]
            nc.scalar.activation(
                out=dst, in_=psv[0:mw, 0:nr, 1:97],
                func=mybir.ActivationFunctionType.Relu,
                bias=sc1[ot][:, 1:2], scale=sc1[ot][:, 0:1])
        return f

    with tc.tile_pool(name="ps_c1", bufs=3, space="PSUM") as pp1, \
         tc.tile_pool(name="ps_tr", bufs=2, space="PSUM") as ppt, \
         tc.tile_pool(name="ps_s", bufs=1, space="PSUM") as pps:

        # conv1 qk row-blocks interleaved across the 3 out-tiles, with S'
        # transpose groups issued as soon as their pixel chunks are covered.
        qk0r = qk[0].rearrange("p (c k) -> p c k", k=128)
        qk1r = qk[1].rearrange("p (c k) -> p c k", k=128)
        qk2r = qk[2].rearrange("p (c k) -> p c k", k=128)
        sp = pps.tile([128, 2 * C], F32, tag="sp")

        def s_group(g):
            tq = ppt.tile([128, 3 * C], BF16, tag="tq")
            tk = ppt.tile([128, 3 * C], BF16, tag="tk")
            tqv = tq.rearrange("p (i c) -> p i c", c=C)
            tkv = tk.rearrange("p (i c) -> p i c", c=C)
            for i in range(3):
                ci = 3 * g + i
                nc.tensor.transpose(tqv[:, i, 0:128], qk0r[:, ci, :], ident[:])
                nc.tensor.transpose(tqv[:, i, 128:192], qk1r[0:64, ci, :],
                                    ident[0:64, 0:64])
                nc.tensor.transpose(tkv[:, i, 0:64], qk1r[64:128, ci, :],
                                    ident[64:128, 64:128])
                nc.tensor.transpose(tkv[:, i, 64:192], qk2r[:, ci, :], ident[:])
            qtc = ev.tile([128, 3 * C], BF16, tag="qtc")
            ktc = ev.tile([128, 3 * C], BF16, tag="ktc")
            nc.scalar.copy(out=qtc[:], in_=tq[:])
            nc.vector.tensor_copy(out=ktc[:], in_=tk[:])
            qcv = qtc.rearrange("p (i c) -> p i c", c=C)
            kcv = ktc.rearrange("p (i c) -> p i c", c=C)
            for i in range(3):
                nc.tensor.matmul(sp[:, 0:C], lhsT=kcv[:, i, 0:128],
                                 rhs=qcv[:, i, :],
                                 start=(g == 0 and i == 0),
                                 stop=(g == NCH // 3 - 1 and i == 2))
                nc.tensor.matmul(sp[0:64, C:2 * C], lhsT=kcv[:, i, 128:192],
                                 rhs=qcv[:, i, :],
                                 start=(g == 0 and i == 0),
                                 stop=(g == NCH // 3 - 1 and i == 2))

        next_g = 0
        for bi, (r0, nr) in enumerate(qk_blocks):
            for ot in range(3):
                conv1_block(ot, bi, r0, nr, evac_qk(ot))
            px_done = (r0 + nr) * W
            while next_g < NCH // 3 and (3 * next_g + 3) * 128 <= px_done:
                s_group(next_g)
                next_g += 1
        while next_g < NCH // 3:
            s_group(next_g)
            next_g += 1

        # deferred zero-clears (needed only from P@V onward): ptall plane1
        # upper rows are a zero K-pad for the P@V DR weights; vall plane1
        # upper rows are junk read by that zero plane -- must be non-NaN
        # (0 x NaN = NaN).
        nc.vector.memset(ptall[64:128, C:2 * C], 0.0)
        nc.vector.memset(vall[64:128, VS:2 * VS], 0.0)

        # sumsq totals + AllReduce staging
        for ti in range(3):
            nc.vector.reduce_sum(out=accs[ti][:], in_=sqac[ti][:],
                                 axis=mybir.AxisListType.X)
        nc.vector.tensor_copy(out=ssa[:, 192:193], in_=accs[0][:])
        nc.vector.tensor_copy(out=ssb[:, 192:193], in_=accs[1][0:64, :])
        nc.vector.tensor_copy(out=accb[:, 0:1], in_=accs[1][:])
        nc.vector.tensor_copy(out=accb[:, 1:2], in_=accs[2][:])
        nc.sync.dma_start(out=ssa[0:64, 193:194], in_=accb[64:128, 0:1])
        nc.sync.dma_start(out=ssa[64:128, 193:194], in_=accb[0:64, 1:2])
        nc.sync.dma_start(out=ssb[0:64, 193:194], in_=accb[64:128, 1:2])
        nc.scalar.copy(out=ssa[:, 0:192], in_=sp[:, 0:C])
        nc.scalar.copy(out=ssb[:, 0:192], in_=sp[0:64, C:2 * C])
        nc.sync.dma_start(out=cc2i[0:64, :], in_=ssa[0:64, :])
        nc.scalar.dma_start(out=cc2i[64:128, :], in_=ssa[64:128, :])
        nc.gpsimd.dma_start(out=cc2i[128:192, :], in_=ssb[:])
        nc.gpsimd.collective_compute(
            "AllReduce", mybir.AluOpType.add, replica_groups=GROUPS,
            ins=[cc2i[:]], outs=[cc2o[:]])
        # sumsq columns first: the norm chain needs only these
        nc.gpsimd.dma_start(out=sfa[:, 192:194], in_=cc2o[0:128, 192:194])
        nc.gpsimd.dma_start(out=sfb[:, 192:194], in_=cc2o[128:192, 192:194])
        nc.sync.dma_start(out=sfa[0:64, 0:192], in_=cc2o[0:64, 0:192])
        nc.scalar.dma_start(out=sfa[64:128, 0:192], in_=cc2o[64:128, 0:192])
        nc.sync.dma_start(out=sfb[:, 0:192], in_=cc2o[128:192, 0:192])

        # conv1 v tiles overlap the collective
        conv1_tile(3, v_blocks, evac_v(3))
        conv1_tile(4, v_blocks, evac_v(4))

    # ---------------- transpose-free softmax -> fp8 P^T planes ------------
    with tc.tile_pool(name="ps_sm", bufs=1, space="PSUM") as ppm, \
         tc.tile_pool(name="ps_pv", bufs=2, space="PSUM") as ppv, \
         tc.tile_pool(name="ps_dw", bufs=2, space="PSUM") as ppd:
        psq = ppm.tile([1, C], BF16, tag="psq")
        pm = ppm.tile([128, 392], F32, tag="pm")
        bcast = pm[:, 0:192]
        pcs = pm[0:1, 196:388]

        nc.tensor.transpose(psq[0:1, 0:128], sfa[:, 192:193], ident[:])
        nc.tensor.transpose(psq[0:1, 128:192], sfb[0:64, 192:193],
                            ident[0:64, 0:64])
        nc.vector.tensor_scalar(out=qs[:], in0=psq[0:1, :], scalar1=1e-24,
                                scalar2=None, op0=mybir.AluOpType.max)
        nc.vector.tensor_scalar(out=rska[:], in0=sfa[:, 193:194],
                                scalar1=1e-24, scalar2=None,
                                op0=mybir.AluOpType.max)
        nc.vector.tensor_scalar(out=rskb[:], in0=sfb[0:64, 193:194],
                                scalar1=1e-24, scalar2=None,
                                op0=mybir.AluOpType.max)
        for r in (qs, rska, rskb):
            nc.vector.reciprocal(out=r[:], in_=r[:])
            nc.scalar.activation(out=r[:], in_=r[:],
                                 func=mybir.ActivationFunctionType.Sqrt)
        nc.tensor.matmul(bcast, lhsT=ones1[:, :], rhs=qs[:],
                         start=True, stop=True)
        nc.vector.scalar_tensor_tensor(
            out=ea[:], in0=sfa[:, 0:192], scalar=tmps[:, 0:1], in1=bcast,
            op0=mybir.AluOpType.mult, op1=mybir.AluOpType.mult)
        nc.vector.scalar_tensor_tensor(
            out=eb[:], in0=sfb[0:64, 0:192], scalar=tmps[0:64, 0:1],
            in1=bcast[0:64, :],
            op0=mybir.AluOpType.mult, op1=mybir.AluOpType.mult)
        # P^T planes (unnormalized): plane0 = keys 0:128, plane1 = keys 128:192
        nc.scalar.activation(out=ptall[:, 0:C], in_=ea[:],
                             func=mybir.ActivationFunctionType.Exp,
                             bias=0.0, scale=rska[:, 0:1])
        nc.scalar.activation(out=ptall[0:64, C:2 * C], in_=eb[:],
                             func=mybir.ActivationFunctionType.Exp,
                             bias=0.0, scale=rskb[:, 0:1])
        # out = P^T.T @ v (fp8 DoubleRow over the two key planes), interleaved
        # with depthwise-A blocks so dw matmuls hide the P@V evac latency.
        ptv = ptall.rearrange("p (two q) -> p two q", two=2)
        va_b = vall[:]
        yv = yout.rearrange("c (r w) -> c r w", w=W)

        def pv_rhs(r0, nr):
            return bass.AP(tensor=va_b.tensor, offset=va_b.offset + r0 * W,
                           ap=[[va_b.ap[0][0], 128], [VS, 2], [1, nr * W]])

        def pv_block(r0, nr):
            po = ppv.tile([128, 5 * W], F32, tag="po")
            po2 = ppv.tile([64, 5 * W], F32, tag="po2")
            nc.tensor.matmul(po[:, 0:nr * W], lhsT=ptv[:, :, 0:128],
                             rhs=pv_rhs(r0, nr), start=True, stop=True,
                             perf_mode=mybir.MatmulPerfMode.DoubleRow)
            nc.tensor.matmul(po2[0:64, 0:nr * W], lhsT=ptv[:, :, 128:192],
                             rhs=pv_rhs(r0, nr), start=True, stop=True,
                             perf_mode=mybir.MatmulPerfMode.DoubleRow)
            pov = po.rearrange("p (r w) -> p r w", w=W)
            po2v = po2.rearrange("p (r w) -> p r w", w=W)
            nc.scalar.copy(out=oav[:, r0 + 1:r0 + 1 + nr, 1:97],
                           in_=pov[:, 0:nr, :])
            nc.vector.tensor_copy(out=obv[0:64, r0 + 1:r0 + 1 + nr, 2:98],
                                  in_=po2v[0:64, 0:nr, :])

        def dwa_block(r0, nr):
            ps = ppd.tile([128, 5, W], F32, tag="dwps")
            for t in range(9):
                dy, dx = t // 3 - 1, t % 3 - 1
                nc.tensor.matmul(
                    ps[:, 0:nr, :],
                    lhsT=w2av[:, t, :],
                    rhs=oav[:, r0 + 1 + dy:r0 + 1 + dy + nr, 1 + dx:97 + dx],
                    start=(t == 0), stop=(t == 8))
            fo = ev.tile([128, 5, W], F32, tag="fo")
            nc.scalar.activation(out=fo[:, 0:nr, :], in_=ps[:, 0:nr, :],
                                 func=mybir.ActivationFunctionType.Identity,
                                 bias=b2a[:, 0:1], scale=rsA[:, 0:1])
            nc.sync.dma_start(out=yv[0:128, r0:r0 + nr, :],
                              in_=fo[:, 0:nr, :])

        dw_blocks = _row_blocks(OR_)
        pv_block(*v_blocks[0])
        pv_block(*v_blocks[1])
        # softmax denominator: colsum over keys via ones-matmul
        nc.tensor.matmul(pcs, lhsT=ones8[:, 0:1],
                         rhs=ptall[:, 0:C], start=True, stop=False)
        nc.tensor.matmul(pcs, lhsT=ones8[:, 0:1],
                         rhs=ptall[:, C:2 * C], start=False, stop=True)
        nc.vector.reciprocal(out=isr[:], in_=pcs)
        prt = pm[:, 192:194]
        nc.tensor.transpose(prt[0:128, 0:1], isr[0:1, 0:128],
                            identf[0:1, 0:1])
        nc.tensor.transpose(prt[0:64, 1:2], isr[0:1, 128:192],
                            identf[0:1, 0:1])
        nc.vector.tensor_copy(out=rsA[:], in_=prt[0:128, 0:1])
        nc.vector.tensor_copy(out=rsB[:], in_=prt[0:64, 1:2])

        for bi in range(2, len(v_blocks)):
            dwa_block(*dw_blocks[bi - 2])
            pv_block(*v_blocks[bi])
            if bi == 5:
                # replicate ob rows 1..25 (needs P@V-B evacs through block 4)
                nc.sync.dma_start(out=obv[64:128, 1:26, 1:97],
                                  in_=obv[0:64, 1:26, 2:98])
        nc.sync.dma_start(out=obv[64:128, 26:50, 1:97],
                          in_=obv[0:64, 26:50, 2:98])
        dwa_block(*dw_blocks[8])
        dwa_block(*dw_blocks[9])
        for (r0, nr) in dw_blocks:
            ps = ppd.tile([128, 5, W], F32, tag="dwps")
            for dy in range(3):
                nc.tensor.matmul(
                    ps[0:64, 0:nr, :], lhsT=w2fv[:, dy, :],
                    rhs=obv[:, r0 + dy:r0 + dy + nr, 1:97],
                    start=(dy == 0), stop=False)
                nc.tensor.matmul(
                    ps[0:64, 0:nr, :], lhsT=w2gv[64:128, dy, :],
                    rhs=obv[64:128, r0 + dy:r0 + dy + nr, 2:98],
                    start=False, stop=(dy == 2))
            fo = ev.tile([128, 5, W], F32, tag="fo")
            nc.scalar.activation(out=fo[0:64, 0:nr, :], in_=ps[0:64, 0:nr, :],
                                 func=mybir.ActivationFunctionType.Identity,
                                 bias=b2b[:, 0:1], scale=rsB[:, 0:1])
            nc.sync.dma_start(out=yv[128:192, r0:r0 + nr, :],
                              in_=fo[0:64, 0:nr, :])
    ctx.close()


# ---------------- host side ----------------
_NC_CACHE = None


def _get_nc():
    global _NC_CACHE
    if _NC_CACHE is None:
        _NC_CACHE = build_nc()
    return _NC_CACHE


def _pack_weights(inp, flip):
    bf = ml_dtypes.bfloat16
    w0 = inp["w0"][:, :, ::-1, :] if flip else inp["w0"]
    w1 = inp["w1"][:, :, ::-1, :] if flip else inp["w1"]
    w2 = inp["w2"][:, :, ::-1, :] if flip else inp["w2"]
    w0 = np.asarray(w0, np.float32)
    w1 = np.asarray(w1, np.float32)
    w2 = np.asarray(w2, np.float32)

    # conv0: out-channel order [x1(0:64), x3(128:192), x2(64:128)]
    cho = np.concatenate([np.arange(0, 64), np.arange(128, 192),
                          np.arange(64, 128)])
    wt = w0[cho].transpose(1, 2, 3, 0)        # [64c, 3ky, 3kx, 192m]
    w0p = np.zeros((128, 3, 2, C), np.float32)
    w0p[0:64, 0, 0] = wt[:, 0, 0]
    w0p[0:64, 0, 1] = wt[:, 0, 2]
    w0p[0:64, 1, 0] = wt[:, 0, 1]
    w0p[64:128, 0, 0] = wt[:, 1, 0]
    w0p[64:128, 0, 1] = wt[:, 1, 2]
    w0p[64:128, 1, 0] = wt[:, 1, 1]
    w0p[64:128, 1, 1] = wt[:, 2, 1]
    w0p[64:128, 2, 0] = wt[:, 2, 0]
    w0p[64:128, 2, 1] = wt[:, 2, 2]
    s0 = inp["g0"] / np.sqrt(inp["v0"] + BN_EPS)
    t0 = inp["be0"] + (inp["b0"] - inp["m0"]) * s0
    sb0 = np.stack([s0 / W0S, t0], axis=1).astype(np.float32)[cho]

    # conv1 DoubleRow packs.  Slab k-partition -> (ky, c) maps:
    def slab_map(s):
        k = np.arange(128 if s < 4 else 64)
        if s == 0:
            return np.zeros_like(k), k
        if s == 1:
            return np.where(k < 64, 0, 1), np.where(k < 64, 128 + k, k - 64)
        if s == 2:
            return np.ones_like(k), 64 + k
        if s == 3:
            return np.full_like(k, 2), k
        return np.full_like(k, 2), 128 + k

    wt1 = w1.transpose(1, 2, 3, 0)            # [192c, 3ky, 3kx, 576m]
    w1p = np.zeros((128, 8, 2, C3), np.float32)
    for pi, (p0, p1) in enumerate(PAIRS_C1):
        for pl, spec in enumerate((p0, p1)):
            if spec is None:
                continue
            s, dx = spec
            ky, cc = slab_map(s)
            w1p[:, pi, pl] = wt1[cc, ky, dx]
    w1p4 = np.zeros((64, 2, 2, C3), np.float32)
    ky4, cc4 = slab_map(4)
    w1p4[:, 0, 0] = wt1[cc4, ky4, 0]
    w1p4[:, 0, 1] = wt1[cc4, ky4, 2]
    w1p4[:, 1, 0] = wt1[cc4, ky4, 1]
    s1 = inp["g1"] / np.sqrt(inp["v1"] + BN_EPS)
    t1 = inp["be1"] + (inp["b1"] - inp["m1"]) * s1
    sb1 = np.stack([s1 / W1S, t1], axis=1).astype(np.float32)

    w2da = np.zeros((128, 9, 128), np.float32)
    w2fb = np.zeros((128, 3, 64), np.float32)
    w2gb = np.zeros((128, 3, 64), np.float32)
    r64, r128 = np.arange(64), np.arange(128)
    for t in range(9):
        d = w2[:, 0, t // 3, t % 3]
        w2da[r128, t, r128] = d[0:128]
    for dy in range(3):
        db = w2[128:192, 0, dy, :]
        w2fb[r64, dy, r64] = db[:, 0]
        w2fb[64 + r64, dy, r64] = db[:, 1]
        w2gb[64 + r64, dy, r64] = db[:, 2]

    out = {
        "w0p": np.ascontiguousarray(
            (w0p * W0S).reshape(128, 3 * 2 * C)).astype(f8np),
        "sb0p": sb0,
        "w1p": np.ascontiguousarray(
            (w1p * W1S).reshape(128, 8 * 2 * C3)).astype(f8np),
        "w1p4": np.ascontiguousarray(
            (w1p4 * W1S).reshape(64, 2 * 2 * C3)).astype(f8np),
        "sb1": sb1,
        "w2da": np.ascontiguousarray(w2da.reshape(128, 9 * 128)).astype(bf),
        "w2fb": np.ascontiguousarray(w2fb.reshape(128, 3 * 64)).astype(bf),
        "w2gb": np.ascontiguousarray(w2gb.reshape(128, 3 * 64)).astype(bf),
        "b2v": np.asarray(inp["b2"], np.float32).reshape(C, 1),
    }
    return out


def kernel(**inputs):
    inputs = {k: np.asarray(v) for k, v in inputs.items()}
    x = inputs["x"]
    B = x.shape[0]
    packs = [_pack_weights(inputs, flip) for flip in (False, True)]
    tempv = np.asarray(inputs["temp"], np.float32).reshape(1, 1)

    in_maps = []
    for core in range(8):
        s, h = core // 2, core % 2
        xi = np.asarray(x[s], np.float32)
        if h:
            xi = xi[:, ::-1, :]
        slab = np.zeros((64, XR, WPD), np.float32)
        slab[:, 1:54, 1:97] = xi[:, 0:53, :]
        m = dict(packs[h])
        m["xs"] = np.ascontiguousarray(slab.reshape(64, XR * WPD)).astype(f8np)
        m["tempv"] = tempv
        m["zpad8"] = np.zeros((1, 512), f8np)
        m["zpad16"] = np.zeros((1, 512), ml_dtypes.bfloat16)
        in_maps.append(m)

    nc = _get_nc()
    res = run_bass_kernel_spmd(nc, in_maps, list(range(8)))
    out = np.zeros((B, C, 96, 96), np.float32)
    for core in range(8):
        s, h = core // 2, core % 2
        yc = res.results[core]["yout"].reshape(C, OR_, W)
        if h:
            out[s, :, 48:96] = yc[:, ::-1, :]
        else:
            out[s, :, 0:48] = yc
    return out


# revision 37
# speedup vs baseline: 1.0264x; 1.0204x over previous
"""Bass/Trainium2 kernel for nn_CSEM sparse_attention problem (v3).

Sharding: 8 cores = 4 samples x 2 spatial halves (bottom half vertically
flipped on host so all cores run an identical NEFF).

v3 vs v2: fp8e4 DoubleRow matmuls for conv0 / conv1 / P@V (one DR matmul
accumulates two (weights-plane, ifmap-plane) K-tiles at 0.5 cycles/row).
Activations and weights for those convs are fp8; conv taps are addressed as
column/row offsets into padded flat slabs so tap pairs become stride-`d`
plane pairs of a single AP. Softmax is transpose-free: S'^T stays key-major,
q-norms are broadcast along partitions, exp goes straight to fp8 P^T planes,
and the softmax denominator (from an ones-matmul colsum) is folded into the
depthwise-conv evacuation scale. Depthwise conv + q/k transposes + S' stay
bf16 for accuracy.
"""

import numpy as np
import ml_dtypes

import concourse.bass as bass
import concourse.mybir as mybir
import concourse.tile as tile
from concourse.bass_utils import run_bass_kernel_spmd
from concourse.masks import make_identity

F8 = mybir.dt.float8e4
BF16 = mybir.dt.bfloat16
F32 = mybir.dt.float32
BN_EPS = 1e-5
f8np = ml_dtypes.float8_e4m3

CIN, C, C3 = 64, 192, 576
W, WPD = 96, 98
XR = 54          # x slab rows (1 zero pad + 53 data)
YR = 52          # y rows computed locally (0..51)
TR = 50          # t rows 0..49
QR = 49          # conv1 v rows 0..48
SR = 48          # rows feeding S partial
OR_ = 48         # final output rows per core
NCH = SR * W // 128   # 36 transpose chunks
VS = QR * W      # v plane stride (4704)
SL = 4908        # Tall slab stride (1 lead + 50*98 data + slack)
XLEN = 1 + XR * WPD + 8
W0S, W1S = 32.0, 64.0
GROUPS = [[0, 1], [2, 3], [4, 5], [6, 7]]

# conv1 DoubleRow pairs as ((slab, dx), (slab, dx)); None = zero-weight dummy
# plane (rhs offset +2 -- a dim1 stride equal to the element stride wedges the
# device). Pairs stay within one slab so each block's read footprint is
# row-local and conv1 pipelines with the bilinear-branch writes.
PAIRS_C1 = [((0, 0), (0, 2)), ((0, 1), None), ((1, 0), (1, 2)),
            ((1, 1), None), ((2, 0), (2, 2)), ((2, 1), None),
            ((3, 0), (3, 2)), ((3, 1), None)]
PAIRS_T4 = [((0, 0), (0, 2)), ((0, 1), None)]   # slab idx relative to T4
# denser cross-slab pairing (no dummy planes): used for row-blocks issued
# after the bilinear branch has fully landed, where the wide byte-interval
# footprint of a cross-slab pair cannot stall the pipeline.
PAIRS_C1X = [((0, 0), (0, 2)), ((1, 0), (1, 2)), ((2, 0), (2, 2)),
             ((3, 0), (3, 2)), ((0, 1), (1, 1)), ((2, 1), (3, 1))]


def _split_waits(nc, limit=1):
    """This walrus build rejects instructions carrying more than one sem-wait
    command. Spread extra waits onto same-engine ENGINE_NOPs inserted just
    before the offending instruction (semantically identical: the engine
    blocks on each wait in program order)."""
    ctr = [0]
    for f in nc.m.functions:
        for blk in f.blocks:
            il = blk.instructions
            new = []
            for inst in il:
                si = inst.sync_info
                waits = list(si.on_wait) if (si and si.on_wait) else []
                if len(waits) > limit:
                    for w in waits[:-limit]:
                        ctr[0] += 1
                        nop = mybir.InstNoOp(name=f"WNOP-{ctr[0]}")
                        nop.engine = inst.engine
                        nop.sync_info = mybir.SyncInfo(on_wait=[w], on_update=[])
                        new.append(nop)
                    si.on_wait = waits[-limit:]
                new.append(inst)
            il[:] = new


def _row_blocks(nrows, per=5):
    out, r = [], 0
    while r < nrows:
        n = min(per, nrows - r)
        out.append((r, n))
        r += n
    return out


def build_nc():
    nc = bass.Bass()

    xs_d = nc.declare_dram_parameter("xs", [64, XR * WPD], F8, isOutput=False)
    w0_d = nc.declare_dram_parameter("w0p", [128, 3 * 2 * C], F8, isOutput=False)
    sb0_d = nc.declare_dram_parameter("sb0p", [C, 2], F32, isOutput=False)
    w1_d = nc.declare_dram_parameter("w1p", [128, 8 * 2 * C3], F8, isOutput=False)
    w1x_d = nc.declare_dram_parameter("w1px", [128, 6 * 2 * C3], F8,
                                      isOutput=False)
    w14_d = nc.declare_dram_parameter("w1p4", [64, 2 * 2 * C3], F8, isOutput=False)
    sb1_d = nc.declare_dram_parameter("sb1", [C3, 2], F32, isOutput=False)
    w2a_d = nc.declare_dram_parameter("w2da", [128, 9 * 128], BF16, isOutput=False)
    w2f_d = nc.declare_dram_parameter("w2fb", [128, 3 * 64], BF16, isOutput=False)
    w2g_d = nc.declare_dram_parameter("w2gb", [128, 3 * 64], BF16, isOutput=False)
    b2_d = nc.declare_dram_parameter("b2v", [C, 1], F32, isOutput=False)
    tmp_d = nc.declare_dram_parameter("tempv", [1, 1], F32, isOutput=False)
    z8_d = nc.declare_dram_parameter("zpad8", [1, 512], F8, isOutput=False)
    z16_d = nc.declare_dram_parameter("zpad16", [1, 512], BF16, isOutput=False)
    yout = nc.declare_dram_parameter("yout", [C, OR_ * W], F32, isOutput=True)

    cc2i = nc.dram_tensor("cc2i", [C, 194], BF16)
    cc2o = nc.dram_tensor("cc2o", [C, 194], BF16)

    with tile.TileContext(nc) as tc:
        _body(nc, tc, xs_d, w0_d, sb0_d, w1_d, w1x_d, w14_d, sb1_d, w2a_d,
              w2f_d, w2g_d, b2_d, tmp_d, z8_d, z16_d, yout, cc2i, cc2o)
    _split_waits(nc)
    return nc


def _body(nc, tc, xs_d, w0_d, sb0_d, w1_d, w1x_d, w14_d, sb1_d, w2a_d,
          w2f_d, w2g_d, b2_d, tmp_d, z8_d, z16_d, yout, cc2i, cc2o):
    import contextlib
    ctx = contextlib.ExitStack()
    P = ctx.enter_context(tc.tile_pool(name="persist", bufs=1))
    ev = ctx.enter_context(tc.tile_pool(name="evac", bufs=3))

    # ---- persistent SBUF ----
    xfold = P.tile([128, XLEN], F8, tag="xfold")
    w0s = P.tile([128, 3, 2, C], F8, tag="w0s")
    w1s = P.tile([128, 8, 2, C3], F8, tag="w1s")
    w1sx = P.tile([128, 6, 2, C3], F8, tag="w1sx")
    w1s4 = P.tile([64, 2, 2, C3], F8, tag="w1s4")
    sc0a = P.tile([128, 2], F32, tag="sc0a")
    sc0b = P.tile([64, 2], F32, tag="sc0b")
    scp = [128, 128, 128, 128, 64]
    sc1 = [P.tile([scp[i], 2], F32, tag=f"sc1_{i}", name=f"sc1_{i}")
           for i in range(5)]
    w2da = P.tile([128, 9 * 128], BF16, tag="w2da")
    w2fb = P.tile([128, 3 * 64], BF16, tag="w2fb")
    w2gb = P.tile([128, 3 * 64], BF16, tag="w2gb")  # data at partitions 64..
    b2a = P.tile([128, 1], F32, tag="b2a")
    b2b = P.tile([64, 1], F32, tag="b2b")
    tmps = P.tile([128, 1], F32, tag="tmps")

    Tall = P.tile([128, 4 * SL], F8, tag="Tall")
    T4 = P.tile([64, SL], F8, tag="T4")
    xpool = P.tile([128, YR * WPD], BF16, tag="xpool")   # later reused as oa
    plh = P.tile([128, YR, 48], BF16, tag="plh")
    pl = P.tile([128, 26, 48], BF16, tag="pl")
    vint = P.tile([128, TR, 48], BF16, tag="vint")
    tscr = P.tile([128, TR, 48], BF16, tag="tscr")
    brs = P.tile([128, 51 * WPD], F8, tag="brs")
    ob = P.tile([128, TR * WPD], BF16, tag="ob")
    qk = [P.tile([128, SR * W], BF16, tag=f"qk{i}", name=f"qk{i}")
          for i in range(3)]
    vall = P.tile([128, 2 * VS], F8, tag="vall")
    sqac = [P.tile([128, 10], F32, tag=f"sqac{i}", name=f"sqac{i}")
            for i in range(3)]
    accs = [P.tile([128, 1], F32, tag=f"accs{i}", name=f"accs{i}")
            for i in range(3)]
    ssa = P.tile([128, 194], BF16, tag="ssa")
    ssb = P.tile([64, 194], BF16, tag="ssb")
    sfa = P.tile([128, 194], BF16, tag="sfa")
    sfb = P.tile([64, 194], BF16, tag="sfb")
    accb = P.tile([128, 2], BF16, tag="accb")
    rska = P.tile([128, 1], F32, tag="rska")
    rskb = P.tile([64, 1], F32, tag="rskb")
    qs = P.tile([1, C], F32, tag="qs")
    rqbc = P.tile([128, C], F32, tag="rqbc")
    ea = P.tile([128, C], F32, tag="ea")
    eb = P.tile([64, C], F32, tag="eb")
    ptall = P.tile([128, 2 * C], F8, tag="ptall")
    isr = P.tile([1, C], F32, tag="isr")
    rsA = P.tile([128, 1], F32, tag="rsA")
    rsB = P.tile([64, 1], F32, tag="rsB")
    ones8 = P.tile([128, 1], F8, tag="ones8")
    ones1 = P.tile([1, 128], F32, tag="ones1")

    ident = P.tile([128, 128], BF16, tag="ident")
    make_identity(nc, ident[:])
    identf = P.tile([128, 128], F32, tag="identf")
    make_identity(nc, identf[:])
    nc.vector.memset(ones8[:], 1.0)
    nc.vector.memset(ones1[:], 1.0)

    # ---- input DMAs (x chunked so conv0 starts early) ----
    nc.sync.dma_start(out=w0s[:], in_=w0_d.rearrange("p (a b m) -> p a b m",
                                                     a=3, b=2))
    nc.gpsimd.dma_start(out=sc0a[:], in_=sb0_d[0:128, :])
    nc.gpsimd.dma_start(out=sc0b[:], in_=sb0_d[128:192, :])
    for (a, b) in ((0, 8), (8, 32), (32, XR)):
        nc.sync.dma_start(out=xfold[0:64, 1 + a * WPD:1 + b * WPD],
                          in_=xs_d[:, a * WPD:b * WPD])
        bb = min(b, XR - 1)
        nc.sync.dma_start(out=xfold[64:128, 1 + a * WPD:1 + bb * WPD],
                          in_=xs_d[:, (a + 1) * WPD:(bb + 1) * WPD])
    nc.gpsimd.dma_start(out=w1s[:], in_=w1_d.rearrange("p (a b m) -> p a b m",
                                                     a=8, b=2))
    nc.gpsimd.dma_start(out=w1sx[:], in_=w1x_d.rearrange(
        "p (a b m) -> p a b m", a=6, b=2))
    nc.gpsimd.dma_start(out=w1s4[:], in_=w14_d.rearrange(
        "p (a b m) -> p a b m", a=2, b=2))
    for i, (lo, hi) in enumerate([(0, 128), (128, 256), (256, 384),
                                  (384, 512), (512, 576)]):
        nc.gpsimd.dma_start(out=sc1[i][:], in_=sb1_d[lo:hi, :])
    nc.gpsimd.dma_start(out=w2da[:], in_=w2a_d[:])
    nc.gpsimd.dma_start(out=w2fb[:], in_=w2f_d[:])
    nc.gpsimd.dma_start(out=w2gb[:], in_=w2g_d[:])
    nc.gpsimd.dma_start(out=b2a[:], in_=b2_d[0:128, :])
    nc.gpsimd.dma_start(out=b2b[:], in_=b2_d[128:192, :])
    nc.gpsimd.dma_start(
        out=tmps[:],
        in_=bass.AP(tensor=tmp_d, offset=0, ap=[[0, 128], [1, 1]]))

    # ---- border memsets (DVE; only the load-bearing zeros) ----
    # xfold lead/tail and Tall slab leads/tails are read only by discarded
    # junk output columns -- no clears needed there. T4 row 49 IS needed:
    # the zero-weight dummy plane reads it (0 x NaN = NaN).
    tall_b = Tall[:]
    t4_b = T4[:]

    def slabv(s, plo, phi, r0, r1, c0=0, c1=WPD):
        """[phi-plo, r1-r0, c1-c0] view of slab s rows r0..r1."""
        base = tall_b if s < 4 else t4_b
        off = (s % 4) * SL if s < 4 else 0
        ap0 = base.ap[0][0]
        return bass.AP(tensor=base.tensor,
                       offset=base.offset + plo * ap0 + off + 1 + r0 * WPD + c0,
                       ap=[[ap0, phi - plo], [WPD, r1 - r0], [1, c1 - c0]])

    for s in range(4):
        nc.vector.memset(slabv(s, 0, 128, 0, 1), 0.0)              # row 0
        nc.vector.memset(slabv(s, 0, 128, 0, TR, 0, 1), 0.0)       # col 0
        nc.vector.memset(slabv(s, 0, 128, 0, TR, 97, 98), 0.0)     # col 97
    nc.vector.memset(T4[:, 1 + 49 * WPD:SL], 0.0)   # row 49 + slack
    nc.vector.memset(slabv(4, 0, 64, 0, 1), 0.0)
    nc.vector.memset(slabv(4, 0, 64, 0, 49, 0, 1), 0.0)
    nc.vector.memset(slabv(4, 0, 64, 0, 49, 97, 98), 0.0)

    brv = brs.rearrange("p (r w) -> p r w", w=WPD)
    nc.vector.memset(brv[:, 0:1, :], 0.0)
    nc.vector.memset(brv[:, :, 0:1], 0.0)
    nc.vector.memset(brv[:, :, 97:98], 0.0)

    xpv = xpool.rearrange("p (r w) -> p r w", w=WPD)
    obv = ob.rearrange("p (r w) -> p r w", w=WPD)
    qkv = [t.rearrange("p (r w) -> p r w", w=W) for t in qk]
    w2av = w2da.rearrange("p (t m) -> p t m", t=9)
    w2fv = w2fb.rearrange("p (t m) -> p t m", t=3)
    w2gv = w2gb.rearrange("p (t m) -> p t m", t=3)

    xf_b = xfold[:]

    def c0_rhs(s0, d0, s1, d1, nr):
        o0 = 1 + s0 * WPD + d0 - 1
        o1 = 1 + s1 * WPD + d1 - 1
        return bass.AP(tensor=xf_b.tensor, offset=xf_b.offset + o0,
                       ap=[[xf_b.ap[0][0], 128], [o1 - o0, 2], [1, nr * WPD]])

    def c1_rhs(pairs_base, p0, p1, r0, nr):
        (s0, d0) = p0
        if p1 is None:
            s1, d1 = s0, d0 + 2
        else:
            s1, d1 = p1
        off = (lambda s, d: s * SL + 1 + r0 * WPD + d - 1)
        o0, o1 = off(s0, d0), off(s1, d1)
        return bass.AP(tensor=pairs_base.tensor, offset=pairs_base.offset + o0,
                       ap=[[pairs_base.ap[0][0], pairs_base.ap[0][1]],
                           [o1 - o0, 2], [1, nr * WPD]])

    # ---------------- conv0 (fp8 DoubleRow) ----------------
    # out-tile A (m 0:128): x1|x3 -> xpool (bf16); out-tile B (m 128:192):
    # x2 -> Tall slab2 lower partitions (fp8)
    blocksA = _row_blocks(YR)
    blocksB = _row_blocks(TR)
    with tc.tile_pool(name="ps_c0", bufs=4, space="PSUM") as pp0:
        def conv0_block(r0, nr, m0, mw):
            ps = pp0.tile([128, 5 * WPD], F32, tag="c0ps")
            specs = ((r0, 0, r0, 2), (r0, 1, r0 + 1, 1),
                     (r0 + 1, 0, r0 + 1, 2))
            for pi, (s0, d0, s1, d1) in enumerate(specs):
                nc.tensor.matmul(
                    ps[0:mw, 0:nr * WPD], lhsT=w0s[:, pi, :, m0:m0 + mw],
                    rhs=c0_rhs(s0, d0, s1, d1, nr),
                    start=(pi == 0), stop=(pi == 2),
                    perf_mode=mybir.MatmulPerfMode.DoubleRow)
            return ps

        for (r0, nr) in blocksA:
            ps = conv0_block(r0, nr, 0, 128)
            psv = ps.rearrange("p (r x) -> p r x", x=WPD)
            nc.scalar.activation(
                out=xpv[:, r0:r0 + nr, 1:97], in_=psv[:, 0:nr, 1:97],
                func=mybir.ActivationFunctionType.Relu,
                bias=sc0a[:, 1:2], scale=sc0a[:, 0:1])
        for (r0, nr) in blocksB:
            ps = conv0_block(r0, nr, 128, 64)
            psv = ps.rearrange("p (r x) -> p r x", x=WPD)
            # x2 -> ky1 slot = slab2 partitions 0..63 (t rows at tile rows)
            nc.scalar.activation(
                out=slabv(2, 0, 64, r0, r0 + nr, 1, 97),
                in_=psv[0:64, 0:nr, 1:97],
                func=mybir.ActivationFunctionType.Relu,
                bias=sc0b[:, 1:2], scale=sc0b[:, 0:1])

    # x2 ky-shifted slot copies (slab2 col pads are zero so full width)
    for (a, b) in ((0, 10), (10, 30), (30, 49)):
        nc.sync.dma_start(out=slabv(0, 64, 128, a + 1, b + 1),
                          in_=slabv(2, 0, 64, a, b))
        nc.sync.dma_start(out=slabv(3, 64, 128, a, b),
                          in_=slabv(2, 0, 64, a + 1, b + 1))

    # ---------------- pools + bilinear (DVE; final writes fp8 brs) --------
    cA = P.tile([128, 2], F32, tag="cA")
    nc.vector.memset(cA[0:64, 0:1], 0.75)
    nc.vector.memset(cA[0:64, 1:2], 0.25)
    nc.vector.memset(cA[64:128, 0:1], 0.1875)
    nc.vector.memset(cA[64:128, 1:2], 0.0625)
    cC = P.tile([128, 1], F32, tag="cC")
    nc.vector.memset(cC[0:64, :], 1.0)
    nc.vector.memset(cC[64:128, :], 0.25)

    pl_chunks = [(0, 5), (5, 10), (10, 15), (15, 20), (20, 25), (25, 26)]
    kv_o = 0   # next odd-row k (vint[2k+1], k<=24)
    kv_e = 0   # next even-row k (vint[2k+2], k<=23)
    hv = 0     # next t-row for the horizontal pass

    def hpass(a, b):
        if b <= a:
            return
        nc.vector.tensor_scalar(out=brv[:, 1 + a:1 + b, 1:2],
                                in0=vint[:, a:b, 0:1], scalar1=cC[:, 0:1],
                                scalar2=None, op0=mybir.AluOpType.mult)
        nc.vector.tensor_scalar(out=brv[:, 1 + a:1 + b, 96:97],
                                in0=vint[:, a:b, 47:48], scalar1=cC[:, 0:1],
                                scalar2=None, op0=mybir.AluOpType.mult)
        nc.vector.tensor_scalar(out=tscr[:, a:b, 0:47], in0=vint[:, a:b, 1:48],
                                scalar1=cA[:, 1:2], scalar2=None,
                                op0=mybir.AluOpType.mult)
        nc.vector.scalar_tensor_tensor(
            out=brv[:, 1 + a:1 + b, 2:96:2], in0=vint[:, a:b, 0:47],
            scalar=cA[:, 0:1], in1=tscr[:, a:b, 0:47],
            op0=mybir.AluOpType.mult, op1=mybir.AluOpType.add)
        nc.vector.tensor_scalar(out=tscr[:, a:b, 0:47], in0=vint[:, a:b, 1:48],
                                scalar1=cA[:, 0:1], scalar2=None,
                                op0=mybir.AluOpType.mult)
        nc.vector.scalar_tensor_tensor(
            out=brv[:, 1 + a:1 + b, 3:96:2], in0=vint[:, a:b, 0:47],
            scalar=cA[:, 1:2], in1=tscr[:, a:b, 0:47],
            op0=mybir.AluOpType.mult, op1=mybir.AluOpType.add)

    for (k0, k1) in pl_chunks:
        # horizontal pool pairs for y rows 2k0..2k1-1
        nc.vector.tensor_tensor(out=plh[0:64, 2 * k0:2 * k1, :],
                                in0=xpv[0:64, 2 * k0:2 * k1, 1:97:2],
                                in1=xpv[0:64, 2 * k0:2 * k1, 2:98:2],
                                op=mybir.AluOpType.max)
        nc.vector.tensor_tensor(out=plh[64:128, 2 * k0:2 * k1, :],
                                in0=xpv[64:128, 2 * k0:2 * k1, 1:97:2],
                                in1=xpv[64:128, 2 * k0:2 * k1, 2:98:2],
                                op=mybir.AluOpType.add)
        # vertical pool pairs -> pl rows k0..k1-1
        nc.vector.tensor_tensor(out=pl[0:64, k0:k1, :],
                                in0=plh[0:64, 2 * k0:2 * k1:2, :],
                                in1=plh[0:64, 2 * k0 + 1:2 * k1:2, :],
                                op=mybir.AluOpType.max)
        nc.vector.tensor_tensor(out=pl[64:128, k0:k1, :],
                                in0=plh[64:128, 2 * k0:2 * k1:2, :],
                                in1=plh[64:128, 2 * k0 + 1:2 * k1:2, :],
                                op=mybir.AluOpType.add)
        if k0 == 0:
            nc.vector.tensor_copy(out=vint[:, 0, :], in_=pl[:, 0, :])
        # vertical bilinear rows that only need pl rows < k1
        ke_o = min(k1 - 1, 25)
        if ke_o > kv_o:
            a, b = kv_o, ke_o
            nc.vector.tensor_scalar(out=tscr[:, a:b, :], in0=pl[:, a + 1:b + 1, :],
                                    scalar1=0.25, scalar2=None,
                                    op0=mybir.AluOpType.mult)
            nc.vector.scalar_tensor_tensor(
                out=vint[:, 2 * a + 1:2 * b:2, :], in0=pl[:, a:b, :],
                scalar=0.75, in1=tscr[:, a:b, :],
                op0=mybir.AluOpType.mult, op1=mybir.AluOpType.add)
            kv_o = ke_o
        ke_e = min(k1 - 1, 24)
        if ke_e > kv_e:
            a, b = kv_e, ke_e
            nc.vector.tensor_scalar(out=tscr[:, a:b, :], in0=pl[:, a + 1:b + 1, :],
                                    scalar1=0.75, scalar2=None,
                                    op0=mybir.AluOpType.mult)
            nc.vector.scalar_tensor_tensor(
                out=vint[:, 2 * a + 2:2 * b + 1:2, :], in0=pl[:, a:b, :],
                scalar=0.25, in1=tscr[:, a:b, :],
                op0=mybir.AluOpType.mult, op1=mybir.AluOpType.add)
            kv_e = ke_e
        # horizontal pass over fully-available vint rows
        avail = min(2 * kv_o + 1, 2 * kv_e + 2) if k1 < 26 else TR
        hpass(hv, avail)
        hv = avail

    # brs rows (fp8) -> T slab slots; row-chunked for conv1 pipelining.
    # (ky slot s stores t row rho at tile row rho+1-s.)
    for (a, b) in ((0, 8), (8, 18), (18, 28), (28, 38), (38, 50)):
        nc.sync.dma_start(out=slabv(0, 0, 64, a, b), in_=brv[0:64, a:b, :])
        nc.sync.dma_start(out=slabv(1, 0, 64, a, b), in_=brv[64:128, a:b, :])
        nc.sync.dma_start(out=slabv(1, 64, 128, a, b),
                          in_=brv[0:64, a + 1:b + 1, :])
        nc.sync.dma_start(out=slabv(2, 64, 128, a, b),
                          in_=brv[64:128, a + 1:b + 1, :])
        bb = min(b, 49)
        nc.sync.dma_start(out=slabv(3, 0, 64, a, bb),
                          in_=brv[0:64, a + 2:bb + 2, :])
        nc.sync.dma_start(out=slabv(4, 0, 64, a, bb),
                          in_=brv[64:128, a + 2:bb + 2, :])

    # oa (= xpool reuse) borders for the depthwise reads; ob borders
    oav = xpv[:, 0:50, :]
    nc.vector.memset(oav[:, 0:1, :], 0.0)
    nc.vector.memset(oav[:, :, 0:1], 0.0)
    nc.vector.memset(oav[:, :, 97:98], 0.0)
    nc.vector.memset(obv[:, 0:1, :], 0.0)
    nc.vector.memset(obv[0:64, :, 1:2], 0.0)
    nc.vector.memset(obv[64:128, :, 0:1], 0.0)
    nc.vector.memset(obv[:, :, 97:98], 0.0)

    # ---------------- conv1 (fp8 DoubleRow) + attention prologue ----------
    qk_blocks = _row_blocks(SR)
    v_blocks = _row_blocks(QR)

    def conv1_block(ot, bi, r0, nr, evac, dense=False):
        mw = 64 if ot == 4 else 128
        m0 = 128 * ot
        ps = pp1.tile([128, 5 * WPD], F32, tag="c1ps")
        pairs, wt = ((PAIRS_C1X, w1sx) if dense else (PAIRS_C1, w1s))
        for pi, (p0, p1) in enumerate(pairs):
            nc.tensor.matmul(
                ps[0:mw, 0:nr * WPD], lhsT=wt[:, pi, :, m0:m0 + mw],
                rhs=c1_rhs(tall_b, p0, p1, r0, nr),
                start=(pi == 0), stop=False,
                perf_mode=mybir.MatmulPerfMode.DoubleRow)
        for qi, (p0, p1) in enumerate(PAIRS_T4):
            nc.tensor.matmul(
                ps[0:mw, 0:nr * WPD], lhsT=w1s4[:, qi, :, m0:m0 + mw],
                rhs=c1_rhs(t4_b, p0, p1, r0, nr),
                start=False, stop=(qi == 1),
                perf_mode=mybir.MatmulPerfMode.DoubleRow)
        psv = ps.rearrange("p (r x) -> p r x", x=WPD)
        evac(bi, r0, nr, psv, mw)

    def conv1_tile(ot, blocks, evac):
        for bi, (r0, nr) in enumerate(blocks):
            conv1_block(ot, bi, r0, nr, evac, dense=True)

    def evac_qk(ot):
        def f(bi, r0, nr, psv, mw):
            nc.scalar.activation(
                out=qkv[ot][:, r0:r0 + nr, :], in_=psv[:, 0:nr, 1:97],
                func=mybir.ActivationFunctionType.Relu,
                bias=sc1[ot][:, 1:2], scale=sc1[ot][:, 0:1])
            dump = ev.tile([128, 5 * W], BF16, tag="sqd")
            nc.vector.tensor_tensor(
                out=dump[:, 0:nr * W],
                in0=qk[ot][:, r0 * W:(r0 + nr) * W],
                in1=qk[ot][:, r0 * W:(r0 + nr) * W],
                op=mybir.AluOpType.mult)
            nc.vector.reduce_sum(out=sqac[ot][:, bi:bi + 1],
                                 in_=dump[:, 0:nr * W],
                                 axis=mybir.AxisListType.X)
        return f

    def evac_v(ot):
        off = 0 if ot == 3 else VS

        def f(bi, r0, nr, psv, mw):
            dst = vall[0:mw, off + r0 * W:off + (r0 + nr) * W]
            nc.scalar.activation(
                out=dst, in_=psv[0:mw, 0:nr, 1:97],
                func=mybir.ActivationFunctionType.Relu,
                bias=sc1[ot][:, 1:2], scale=sc1[ot][:, 0:1])
        return f

    with tc.tile_pool(name="ps_c1", bufs=3, space="PSUM") as pp1, \
         tc.tile_pool(name="ps_tr", bufs=2, space="PSUM") as ppt, \
         tc.tile_pool(name="ps_s", bufs=1, space="PSUM") as pps:

        # conv1 qk row-blocks interleaved across the 3 out-tiles, with S'
        # transpose groups issued as soon as their pixel chunks are covered.
        qk0r = qk[0].rearrange("p (c k) -> p c k", k=128)
        qk1r = qk[1].rearrange("p (c k) -> p c k", k=128)
        qk2r = qk[2].rearrange("p (c k) -> p c k", k=128)
        sp = pps.tile([128, 2 * C], F32, tag="sp")

        def s_group(g):
            tq = ppt.tile([128, 3 * C], BF16, tag="tq")
            tk = ppt.tile([128, 3 * C], BF16, tag="tk")
            tqv = tq.rearrange("p (i c) -> p i c", c=C)
            tkv = tk.rearrange("p (i c) -> p i c", c=C)
            for i in range(3):
                ci = 3 * g + i
                nc.tensor.transpose(tqv[:, i, 0:128], qk0r[:, ci, :], ident[:])
                nc.tensor.transpose(tqv[:, i, 128:192], qk1r[0:64, ci, :],
                                    ident[0:64, 0:64])
                nc.tensor.transpose(tkv[:, i, 0:64], qk1r[64:128, ci, :],
                                    ident[64:128, 64:128])
                nc.tensor.transpose(tkv[:, i, 64:192], qk2r[:, ci, :], ident[:])
            qtc = ev.tile([128, 3 * C], BF16, tag="qtc")
            ktc = ev.tile([128, 3 * C], BF16, tag="ktc")
            nc.scalar.copy(out=qtc[:], in_=tq[:])
            nc.vector.tensor_copy(out=ktc[:], in_=tk[:])
            qcv = qtc.rearrange("p (i c) -> p i c", c=C)
            kcv = ktc.rearrange("p (i c) -> p i c", c=C)
            for i in range(3):
                nc.tensor.matmul(sp[:, 0:C], lhsT=kcv[:, i, 0:128],
                                 rhs=qcv[:, i, :],
                                 start=(g == 0 and i == 0),
                                 stop=(g == NCH // 3 - 1 and i == 2))
                nc.tensor.matmul(sp[0:64, C:2 * C], lhsT=kcv[:, i, 128:192],
                                 rhs=qcv[:, i, :],
                                 start=(g == 0 and i == 0),
                                 stop=(g == NCH // 3 - 1 and i == 2))

        next_g = 0
        for bi, (r0, nr) in enumerate(qk_blocks):
            for ot in range(3):
                conv1_block(ot, bi, r0, nr, evac_qk(ot), dense=(bi >= 4))
            px_done = (r0 + nr) * W
            while next_g < NCH // 3 and (3 * next_g + 3) * 128 <= px_done:
                s_group(next_g)
                next_g += 1
        while next_g < NCH // 3:
            s_group(next_g)
            next_g += 1

        # deferred zero-clears (needed only from P@V onward): ptall plane1
        # upper rows are a zero K-pad for the P@V DR weights; vall plane1
        # upper rows are junk read by that zero plane -- must be non-NaN
        # (0 x NaN = NaN).
        nc.vector.memset(ptall[64:128, C:2 * C], 0.0)
        nc.vector.memset(vall[64:128, VS:2 * VS], 0.0)

        # sumsq totals + AllReduce staging
        for ti in range(3):
            nc.vector.reduce_sum(out=accs[ti][:], in_=sqac[ti][:],
                                 axis=mybir.AxisListType.X)
        nc.vector.tensor_copy(out=ssa[:, 192:193], in_=accs[0][:])
        nc.vector.tensor_copy(out=ssb[:, 192:193], in_=accs[1][0:64, :])
        nc.vector.tensor_copy(out=accb[:, 0:1], in_=accs[1][:])
        nc.vector.tensor_copy(out=accb[:, 1:2], in_=accs[2][:])
        nc.sync.dma_start(out=ssa[0:64, 193:194], in_=accb[64:128, 0:1])
        nc.sync.dma_start(out=ssa[64:128, 193:194], in_=accb[0:64, 1:2])
        nc.sync.dma_start(out=ssb[0:64, 193:194], in_=accb[64:128, 1:2])
        nc.scalar.copy(out=ssa[:, 0:192], in_=sp[:, 0:C])
        nc.scalar.copy(out=ssb[:, 0:192], in_=sp[0:64, C:2 * C])
        nc.sync.dma_start(out=cc2i[0:64, :], in_=ssa[0:64, :])
        nc.scalar.dma_start(out=cc2i[64:128, :], in_=ssa[64:128, :])
        nc.gpsimd.dma_start(out=cc2i[128:192, :], in_=ssb[:])
        nc.gpsimd.collective_compute(
            "AllReduce", mybir.AluOpType.add, replica_groups=GROUPS,
            ins=[cc2i[:]], outs=[cc2o[:]])
        # sumsq columns first: the norm chain needs only these
        nc.gpsimd.dma_start(out=sfa[:, 192:194], in_=cc2o[0:128, 192:194])
        nc.gpsimd.dma_start(out=sfb[:, 192:194], in_=cc2o[128:192, 192:194])
        nc.sync.dma_start(out=sfa[0:64, 0:192], in_=cc2o[0:64, 0:192])
        nc.scalar.dma_start(out=sfa[64:128, 0:192], in_=cc2o[64:128, 0:192])
        nc.sync.dma_start(out=sfb[:, 0:192], in_=cc2o[128:192, 0:192])

        # conv1 v tiles overlap the collective
        conv1_tile(3, v_blocks, evac_v(3))
        conv1_tile(4, v_blocks, evac_v(4))

    # ---------------- transpose-free softmax -> fp8 P^T planes ------------
    with tc.tile_pool(name="ps_sm", bufs=1, space="PSUM") as ppm, \
         tc.tile_pool(name="ps_pv", bufs=2, space="PSUM") as ppv, \
         tc.tile_pool(name="ps_dw", bufs=2, space="PSUM") as ppd:
        psq = ppm.tile([1, C], BF16, tag="psq")
        pm = ppm.tile([128, 392], F32, tag="pm")
        bcast = pm[:, 0:192]
        pcs = pm[0:1, 196:388]

        nc.tensor.transpose(psq[0:1, 0:128], sfa[:, 192:193], ident[:])
        nc.tensor.transpose(psq[0:1, 128:192], sfb[0:64, 192:193],
                            ident[0:64, 0:64])
        nc.vector.tensor_scalar(out=qs[:], in0=psq[0:1, :], scalar1=1e-24,
                                scalar2=None, op0=mybir.AluOpType.max)
        nc.vector.tensor_scalar(out=rska[:], in0=sfa[:, 193:194],
                                scalar1=1e-24, scalar2=None,
                                op0=mybir.AluOpType.max)
        nc.vector.tensor_scalar(out=rskb[:], in0=sfb[0:64, 193:194],
                                scalar1=1e-24, scalar2=None,
                                op0=mybir.AluOpType.max)
        for r in (qs, rska, rskb):
            nc.vector.reciprocal(out=r[:], in_=r[:])
            nc.scalar.activation(out=r[:], in_=r[:],
                                 func=mybir.ActivationFunctionType.Sqrt)
        nc.tensor.matmul(bcast, lhsT=ones1[:, :], rhs=qs[:],
                         start=True, stop=True)
        nc.vector.scalar_tensor_tensor(
            out=ea[:], in0=sfa[:, 0:192], scalar=tmps[:, 0:1], in1=bcast,
            op0=mybir.AluOpType.mult, op1=mybir.AluOpType.mult)
        nc.vector.scalar_tensor_tensor(
            out=eb[:], in0=sfb[0:64, 0:192], scalar=tmps[0:64, 0:1],
            in1=bcast[0:64, :],
            op0=mybir.AluOpType.mult, op1=mybir.AluOpType.mult)
        # P^T planes (unnormalized): plane0 = keys 0:128, plane1 = keys 128:192
        nc.scalar.activation(out=ptall[:, 0:C], in_=ea[:],
                             func=mybir.ActivationFunctionType.Exp,
                             bias=0.0, scale=rska[:, 0:1])
        nc.scalar.activation(out=ptall[0:64, C:2 * C], in_=eb[:],
                             func=mybir.ActivationFunctionType.Exp,
                             bias=0.0, scale=rskb[:, 0:1])
        # out = P^T.T @ v (fp8 DoubleRow over the two key planes), interleaved
        # with depthwise-A blocks so dw matmuls hide the P@V evac latency.
        ptv = ptall.rearrange("p (two q) -> p two q", two=2)
        va_b = vall[:]
        yv = yout.rearrange("c (r w) -> c r w", w=W)

        def pv_rhs(r0, nr):
            return bass.AP(tensor=va_b.tensor, offset=va_b.offset + r0 * W,
                           ap=[[va_b.ap[0][0], 128], [VS, 2], [1, nr * W]])

        def pv_block(r0, nr):
            po = ppv.tile([128, 5 * W], F32, tag="po")
            po2 = ppv.tile([64, 5 * W], F32, tag="po2")
            nc.tensor.matmul(po[:, 0:nr * W], lhsT=ptv[:, :, 0:128],
                             rhs=pv_rhs(r0, nr), start=True, stop=True,
                             perf_mode=mybir.MatmulPerfMode.DoubleRow)
            nc.tensor.matmul(po2[0:64, 0:nr * W], lhsT=ptv[:, :, 128:192],
                             rhs=pv_rhs(r0, nr), start=True, stop=True,
                             perf_mode=mybir.MatmulPerfMode.DoubleRow)
            pov = po.rearrange("p (r w) -> p r w", w=W)
            po2v = po2.rearrange("p (r w) -> p r w", w=W)
            nc.scalar.copy(out=oav[:, r0 + 1:r0 + 1 + nr, 1:97],
                           in_=pov[:, 0:nr, :])
            nc.vector.tensor_copy(out=obv[0:64, r0 + 1:r0 + 1 + nr, 2:98],
                                  in_=po2v[0:64, 0:nr, :])

        def dwa_block(r0, nr):
            ps = ppd.tile([128, 5, W], F32, tag="dwps")
            for t in range(9):
                dy, dx = t // 3 - 1, t % 3 - 1
                nc.tensor.matmul(
                    ps[:, 0:nr, :],
                    lhsT=w2av[:, t, :],
                    rhs=oav[:, r0 + 1 + dy:r0 + 1 + dy + nr, 1 + dx:97 + dx],
                    start=(t == 0), stop=(t == 8))
            fo = ev.tile([128, 5, W], F32, tag="fo")
            nc.scalar.activation(out=fo[:, 0:nr, :], in_=ps[:, 0:nr, :],
                                 func=mybir.ActivationFunctionType.Identity,
                                 bias=b2a[:, 0:1], scale=rsA[:, 0:1])
            nc.sync.dma_start(out=yv[0:128, r0:r0 + nr, :],
                              in_=fo[:, 0:nr, :])

        dw_blocks = _row_blocks(OR_)
        pv_block(*v_blocks[0])
        pv_block(*v_blocks[1])
        # softmax denominator: colsum over keys via ones-matmul
        nc.tensor.matmul(pcs, lhsT=ones8[:, 0:1],
                         rhs=ptall[:, 0:C], start=True, stop=False)
        nc.tensor.matmul(pcs, lhsT=ones8[:, 0:1],
                         rhs=ptall[:, C:2 * C], start=False, stop=True)
        nc.vector.reciprocal(out=isr[:], in_=pcs)
        prt = pm[:, 192:194]
        nc.tensor.transpose(prt[0:128, 0:1], isr[0:1, 0:128],
                            identf[0:1, 0:1])
        nc.tensor.transpose(prt[0:64, 1:2], isr[0:1, 128:192],
                            identf[0:1, 0:1])
        nc.vector.tensor_copy(out=rsA[:], in_=prt[0:128, 0:1])
        nc.vector.tensor_copy(out=rsB[:], in_=prt[0:64, 1:2])

        for bi in range(2, len(v_blocks)):
            dwa_block(*dw_blocks[bi - 2])
            pv_block(*v_blocks[bi])
            if bi == 5:
                # replicate ob rows 1..25 (needs P@V-B evacs through block 4)
                nc.sync.dma_start(out=obv[64:128, 1:26, 1:97],
                                  in_=obv[0:64, 1:26, 2:98])
        nc.sync.dma_start(out=obv[64:128, 26:50, 1:97],
                          in_=obv[0:64, 26:50, 2:98])
        dwa_block(*dw_blocks[8])
        dwa_block(*dw_blocks[9])
        for (r0, nr) in dw_blocks:
            ps = ppd.tile([128, 5, W], F32, tag="dwps")
            for dy in range(3):
                nc.tensor.matmul(
                    ps[0:64, 0:nr, :], lhsT=w2fv[:, dy, :],
                    rhs=obv[:, r0 + dy:r0 + dy + nr, 1:97],
                    start=(dy == 0), stop=False)
                nc.tensor.matmul(
                    ps[0:64, 0:nr, :], lhsT=w2gv[64:128, dy, :],
                    rhs=obv[64:128, r0 + dy:r0 + dy + nr, 2:98],
                    start=False, stop=(dy == 2))
            fo = ev.tile([128, 5, W], F32, tag="fo")
            nc.scalar.activation(out=fo[0:64, 0:nr, :], in_=ps[0:64, 0:nr, :],
                                 func=mybir.ActivationFunctionType.Identity,
                                 bias=b2b[:, 0:1], scale=rsB[:, 0:1])
            nc.sync.dma_start(out=yv[128:192, r0:r0 + nr, :],
                              in_=fo[0:64, 0:nr, :])
    ctx.close()


# ---------------- host side ----------------
_NC_CACHE = None


def _get_nc():
    global _NC_CACHE
    if _NC_CACHE is None:
        _NC_CACHE = build_nc()
    return _NC_CACHE


def _pack_weights(inp, flip):
    bf = ml_dtypes.bfloat16
    w0 = inp["w0"][:, :, ::-1, :] if flip else inp["w0"]
    w1 = inp["w1"][:, :, ::-1, :] if flip else inp["w1"]
    w2 = inp["w2"][:, :, ::-1, :] if flip else inp["w2"]
    w0 = np.asarray(w0, np.float32)
    w1 = np.asarray(w1, np.float32)
    w2 = np.asarray(w2, np.float32)

    # conv0: out-channel order [x1(0:64), x3(128:192), x2(64:128)]
    cho = np.concatenate([np.arange(0, 64), np.arange(128, 192),
                          np.arange(64, 128)])
    wt = w0[cho].transpose(1, 2, 3, 0)        # [64c, 3ky, 3kx, 192m]
    w0p = np.zeros((128, 3, 2, C), np.float32)
    w0p[0:64, 0, 0] = wt[:, 0, 0]
    w0p[0:64, 0, 1] = wt[:, 0, 2]
    w0p[0:64, 1, 0] = wt[:, 0, 1]
    w0p[64:128, 0, 0] = wt[:, 1, 0]
    w0p[64:128, 0, 1] = wt[:, 1, 2]
    w0p[64:128, 1, 0] = wt[:, 1, 1]
    w0p[64:128, 1, 1] = wt[:, 2, 1]
    w0p[64:128, 2, 0] = wt[:, 2, 0]
    w0p[64:128, 2, 1] = wt[:, 2, 2]
    s0 = inp["g0"] / np.sqrt(inp["v0"] + BN_EPS)
    t0 = inp["be0"] + (inp["b0"] - inp["m0"]) * s0
    sb0 = np.stack([s0 / W0S, t0], axis=1).astype(np.float32)[cho]

    # conv1 DoubleRow packs.  Slab k-partition -> (ky, c) maps:
    def slab_map(s):
        k = np.arange(128 if s < 4 else 64)
        if s == 0:
            return np.zeros_like(k), k
        if s == 1:
            return np.where(k < 64, 0, 1), np.where(k < 64, 128 + k, k - 64)
        if s == 2:
            return np.ones_like(k), 64 + k
        if s == 3:
            return np.full_like(k, 2), k
        return np.full_like(k, 2), 128 + k

    wt1 = w1.transpose(1, 2, 3, 0)            # [192c, 3ky, 3kx, 576m]
    w1p = np.zeros((128, 8, 2, C3), np.float32)
    for pi, (p0, p1) in enumerate(PAIRS_C1):
        for pl, spec in enumerate((p0, p1)):
            if spec is None:
                continue
            s, dx = spec
            ky, cc = slab_map(s)
            w1p[:, pi, pl] = wt1[cc, ky, dx]
    w1px = np.zeros((128, 6, 2, C3), np.float32)
    for pi, (p0, p1) in enumerate(PAIRS_C1X):
        for pl, (s, dx) in enumerate((p0, p1)):
            ky, cc = slab_map(s)
            w1px[:, pi, pl] = wt1[cc, ky, dx]
    w1p4 = np.zeros((64, 2, 2, C3), np.float32)
    ky4, cc4 = slab_map(4)
    w1p4[:, 0, 0] = wt1[cc4, ky4, 0]
    w1p4[:, 0, 1] = wt1[cc4, ky4, 2]
    w1p4[:, 1, 0] = wt1[cc4, ky4, 1]
    s1 = inp["g1"] / np.sqrt(inp["v1"] + BN_EPS)
    t1 = inp["be1"] + (inp["b1"] - inp["m1"]) * s1
    sb1 = np.stack([s1 / W1S, t1], axis=1).astype(np.float32)

    w2da = np.zeros((128, 9, 128), np.float32)
    w2fb = np.zeros((128, 3, 64), np.float32)
    w2gb = np.zeros((128, 3, 64), np.float32)
    r64, r128 = np.arange(64), np.arange(128)
    for t in range(9):
        d = w2[:, 0, t // 3, t % 3]
        w2da[r128, t, r128] = d[0:128]
    for dy in range(3):
        db = w2[128:192, 0, dy, :]
        w2fb[r64, dy, r64] = db[:, 0]
        w2fb[64 + r64, dy, r64] = db[:, 1]
        w2gb[64 + r64, dy, r64] = db[:, 2]

    out = {
        "w0p": np.ascontiguousarray(
            (w0p * W0S).reshape(128, 3 * 2 * C)).astype(f8np),
        "sb0p": sb0,
        "w1p": np.ascontiguousarray(
            (w1p * W1S).reshape(128, 8 * 2 * C3)).astype(f8np),
        "w1px": np.ascontiguousarray(
            (w1px * W1S).reshape(128, 6 * 2 * C3)).astype(f8np),
        "w1p4": np.ascontiguousarray(
            (w1p4 * W1S).reshape(64, 2 * 2 * C3)).astype(f8np),
        "sb1": sb1,
        "w2da": np.ascontiguousarray(w2da.reshape(128, 9 * 128)).astype(bf),
        "w2fb": np.ascontiguousarray(w2fb.reshape(128, 3 * 64)).astype(bf),
        "w2gb": np.ascontiguousarray(w2gb.reshape(128, 3 * 64)).astype(bf),
        "b2v": np.asarray(inp["b2"], np.float32).reshape(C, 1),
    }
    return out


def kernel(**inputs):
    inputs = {k: np.asarray(v) for k, v in inputs.items()}
    x = inputs["x"]
    B = x.shape[0]
    packs = [_pack_weights(inputs, flip) for flip in (False, True)]
    tempv = np.asarray(inputs["temp"], np.float32).reshape(1, 1)

    in_maps = []
    for core in range(8):
        s, h = core // 2, core % 2
        xi = np.asarray(x[s], np.float32)
        if h:
            xi = xi[:, ::-1, :]
        slab = np.zeros((64, XR, WPD), np.float32)
        slab[:, 1:54, 1:97] = xi[:, 0:53, :]
        m = dict(packs[h])
        m["xs"] = np.ascontiguousarray(slab.reshape(64, XR * WPD)).astype(f8np)
        m["tempv"] = tempv
        m["zpad8"] = np.zeros((1, 512), f8np)
        m["zpad16"] = np.zeros((1, 512), ml_dtypes.bfloat16)
        in_maps.append(m)

    nc = _get_nc()
    res = run_bass_kernel_spmd(nc, in_maps, list(range(8)))
    out = np.zeros((B, C, 96, 96), np.float32)
    for core in range(8):
        s, h = core // 2, core % 2
        yc = res.results[core]["yout"].reshape(C, OR_, W)
        if h:
            out[s, :, 48:96] = yc[:, ::-1, :]
        else:
            out[s, :, 0:48] = yc
    return out


# revision 44
# speedup vs baseline: 1.0293x; 1.0028x over previous
"""Bass/Trainium2 kernel for nn_CSEM sparse_attention problem (v3).

Sharding: 8 cores = 4 samples x 2 spatial halves (bottom half vertically
flipped on host so all cores run an identical NEFF).

v3 vs v2: fp8e4 DoubleRow matmuls for conv0 / conv1 / P@V (one DR matmul
accumulates two (weights-plane, ifmap-plane) K-tiles at 0.5 cycles/row).
Activations and weights for those convs are fp8; conv taps are addressed as
column/row offsets into padded flat slabs so tap pairs become stride-`d`
plane pairs of a single AP. Softmax is transpose-free: S'^T stays key-major,
q-norms are broadcast along partitions, exp goes straight to fp8 P^T planes,
and the softmax denominator (from an ones-matmul colsum) is folded into the
depthwise-conv evacuation scale. Depthwise conv + q/k transposes + S' stay
bf16 for accuracy.
"""

import numpy as np
import ml_dtypes

import concourse.bass as bass
import concourse.mybir as mybir
import concourse.tile as tile
from concourse.bass_utils import run_bass_kernel_spmd
from concourse.masks import make_identity

F8 = mybir.dt.float8e4
BF16 = mybir.dt.bfloat16
F32 = mybir.dt.float32
BN_EPS = 1e-5
f8np = ml_dtypes.float8_e4m3

CIN, C, C3 = 64, 192, 576
W, WPD = 96, 98
XR = 54          # x slab rows (1 zero pad + 53 data)
YR = 52          # y rows computed locally (0..51)
TR = 50          # t rows 0..49
QR = 49          # conv1 v rows 0..48
SR = 48          # rows feeding S partial
OR_ = 48         # final output rows per core
NCH = SR * W // 128   # 36 transpose chunks
VS = QR * W      # v plane stride (4704)
SL = 4908        # Tall slab stride (1 lead + 50*98 data + slack)
XLEN = 1 + XR * WPD + 8
W0S, W1S = 32.0, 64.0
GROUPS = [[0, 1], [2, 3], [4, 5], [6, 7]]

# conv1 DoubleRow pairs as ((slab, dx), (slab, dx)); None = zero-weight dummy
# plane (rhs offset +2 -- a dim1 stride equal to the element stride wedges the
# device). Pairs stay within one slab so each block's read footprint is
# row-local and conv1 pipelines with the bilinear-branch writes.
PAIRS_C1 = [((0, 0), (0, 2)), ((0, 1), None), ((1, 0), (1, 2)),
            ((1, 1), None), ((2, 0), (2, 2)), ((2, 1), None),
            ((3, 0), (3, 2)), ((3, 1), None)]
PAIRS_T4 = [((0, 0), (0, 2)), ((0, 1), None)]   # slab idx relative to T4
# denser cross-slab pairing (no dummy planes): used for row-blocks issued
# after the bilinear branch has fully landed, where the wide byte-interval
# footprint of a cross-slab pair cannot stall the pipeline.
PAIRS_C1X = [((0, 0), (0, 2)), ((1, 0), (1, 2)), ((2, 0), (2, 2)),
             ((3, 0), (3, 2)), ((0, 1), (1, 1)), ((2, 1), (3, 1))]


def _split_waits(nc, limit=1):
    """This walrus build rejects instructions carrying more than one sem-wait
    command. Spread extra waits onto same-engine ENGINE_NOPs inserted just
    before the offending instruction (semantically identical: the engine
    blocks on each wait in program order)."""
    ctr = [0]
    for f in nc.m.functions:
        for blk in f.blocks:
            il = blk.instructions
            new = []
            for inst in il:
                si = inst.sync_info
                waits = list(si.on_wait) if (si and si.on_wait) else []
                if len(waits) > limit:
                    for w in waits[:-limit]:
                        ctr[0] += 1
                        nop = mybir.InstNoOp(name=f"WNOP-{ctr[0]}")
                        nop.engine = inst.engine
                        nop.sync_info = mybir.SyncInfo(on_wait=[w], on_update=[])
                        new.append(nop)
                    si.on_wait = waits[-limit:]
                new.append(inst)
            il[:] = new


def _row_blocks(nrows, per=5):
    out, r = [], 0
    while r < nrows:
        n = min(per, nrows - r)
        out.append((r, n))
        r += n
    return out


def build_nc():
    nc = bass.Bass()

    xs_d = nc.declare_dram_parameter("xs", [64, XR * WPD], F8, isOutput=False)
    w0_d = nc.declare_dram_parameter("w0p", [128, 3 * 2 * C], F8, isOutput=False)
    sb0_d = nc.declare_dram_parameter("sb0p", [C, 2], F32, isOutput=False)
    w1_d = nc.declare_dram_parameter("w1p", [128, 8 * 2 * C3], F8, isOutput=False)
    w1x_d = nc.declare_dram_parameter("w1px", [128, 6 * 2 * C3], F8,
                                      isOutput=False)
    w14_d = nc.declare_dram_parameter("w1p4", [64, 2 * 2 * C3], F8, isOutput=False)
    sb1_d = nc.declare_dram_parameter("sb1", [C3, 2], F32, isOutput=False)
    w2a_d = nc.declare_dram_parameter("w2da", [128, 9 * 128], BF16, isOutput=False)
    w2f_d = nc.declare_dram_parameter("w2fb", [128, 3 * 64], BF16, isOutput=False)
    w2g_d = nc.declare_dram_parameter("w2gb", [128, 3 * 64], BF16, isOutput=False)
    b2_d = nc.declare_dram_parameter("b2v", [C, 1], F32, isOutput=False)
    tmp_d = nc.declare_dram_parameter("tempv", [1, 1], F32, isOutput=False)
    z8_d = nc.declare_dram_parameter("zpad8", [1, 512], F8, isOutput=False)
    z16_d = nc.declare_dram_parameter("zpad16", [1, 512], BF16, isOutput=False)
    yout = nc.declare_dram_parameter("yout", [C, OR_ * W], F32, isOutput=True)

    cc2i = nc.dram_tensor("cc2i", [C, 194], BF16)
    cc2o = nc.dram_tensor("cc2o", [C, 194], BF16)

    with tile.TileContext(nc) as tc:
        _body(nc, tc, xs_d, w0_d, sb0_d, w1_d, w1x_d, w14_d, sb1_d, w2a_d,
              w2f_d, w2g_d, b2_d, tmp_d, z8_d, z16_d, yout, cc2i, cc2o)
    _split_waits(nc)
    return nc


def _body(nc, tc, xs_d, w0_d, sb0_d, w1_d, w1x_d, w14_d, sb1_d, w2a_d,
          w2f_d, w2g_d, b2_d, tmp_d, z8_d, z16_d, yout, cc2i, cc2o):
    import contextlib
    ctx = contextlib.ExitStack()
    P = ctx.enter_context(tc.tile_pool(name="persist", bufs=1))
    ev = ctx.enter_context(tc.tile_pool(name="evac", bufs=3))

    # ---- persistent SBUF ----
    xfold = P.tile([128, XLEN], F8, tag="xfold")
    w0s = P.tile([128, 3, 2, C], F8, tag="w0s")
    w1s = P.tile([128, 8, 2, C3], F8, tag="w1s")
    w1sx = P.tile([128, 6, 2, C3], F8, tag="w1sx")
    w1s4 = P.tile([64, 2, 2, C3], F8, tag="w1s4")
    sc0a = P.tile([128, 2], F32, tag="sc0a")
    sc0b = P.tile([64, 2], F32, tag="sc0b")
    scp = [128, 128, 128, 128, 64]
    sc1 = [P.tile([scp[i], 2], F32, tag=f"sc1_{i}", name=f"sc1_{i}")
           for i in range(5)]
    w2da = P.tile([128, 9 * 128], BF16, tag="w2da")
    w2fb = P.tile([128, 3 * 64], BF16, tag="w2fb")
    w2gb = P.tile([128, 3 * 64], BF16, tag="w2gb")  # data at partitions 64..
    b2a = P.tile([128, 1], F32, tag="b2a")
    b2b = P.tile([64, 1], F32, tag="b2b")
    tmps = P.tile([128, 1], F32, tag="tmps")

    Tall = P.tile([128, 4 * SL], F8, tag="Tall")
    T4 = P.tile([64, SL], F8, tag="T4")
    xpool = P.tile([128, YR * WPD], BF16, tag="xpool")   # later reused as oa
    plh = P.tile([128, YR, 48], BF16, tag="plh")
    pl = P.tile([128, 26, 48], BF16, tag="pl")
    vint = P.tile([128, TR, 48], BF16, tag="vint")
    tscr = P.tile([128, TR, 48], BF16, tag="tscr")
    brs = P.tile([128, 51 * WPD], F8, tag="brs")
    ob = P.tile([128, TR * WPD], BF16, tag="ob")
    qk = [P.tile([128, SR * W], BF16, tag=f"qk{i}", name=f"qk{i}")
          for i in range(3)]
    vall = P.tile([128, 2 * VS], F8, tag="vall")
    sqac = [P.tile([128, 10], F32, tag=f"sqac{i}", name=f"sqac{i}")
            for i in range(3)]
    accs = [P.tile([128, 1], F32, tag=f"accs{i}", name=f"accs{i}")
            for i in range(3)]
    ssa = P.tile([128, 194], BF16, tag="ssa")
    ssb = P.tile([64, 194], BF16, tag="ssb")
    sfa = P.tile([128, 194], BF16, tag="sfa")
    sfb = P.tile([64, 194], BF16, tag="sfb")
    accb = P.tile([128, 2], BF16, tag="accb")
    rska = P.tile([128, 1], F32, tag="rska")
    rskb = P.tile([64, 1], F32, tag="rskb")
    qs = P.tile([1, C], F32, tag="qs")
    rqbc = P.tile([128, C], F32, tag="rqbc")
    ea = P.tile([128, C], F32, tag="ea")
    eb = P.tile([64, C], F32, tag="eb")
    ptall = P.tile([128, 2 * C], F8, tag="ptall")
    isr = P.tile([1, C], F32, tag="isr")
    rsA = P.tile([128, 1], F32, tag="rsA")
    rsB = P.tile([64, 1], F32, tag="rsB")
    ones8 = P.tile([128, 1], F8, tag="ones8")
    ones1 = P.tile([1, 128], F32, tag="ones1")

    ident = P.tile([128, 128], BF16, tag="ident")
    make_identity(nc, ident[:])
    identf = P.tile([128, 128], F32, tag="identf")
    make_identity(nc, identf[:])
    nc.vector.memset(ones8[:], 1.0)
    nc.vector.memset(ones1[:], 1.0)

    # ---- input DMAs (x chunked so conv0 starts early) ----
    nc.sync.dma_start(out=w0s[:], in_=w0_d.rearrange("p (a b m) -> p a b m",
                                                     a=3, b=2))
    nc.gpsimd.dma_start(out=sc0a[:], in_=sb0_d[0:128, :])
    nc.gpsimd.dma_start(out=sc0b[:], in_=sb0_d[128:192, :])
    for (a, b) in ((0, 8), (8, 32), (32, XR)):
        nc.sync.dma_start(out=xfold[0:64, 1 + a * WPD:1 + b * WPD],
                          in_=xs_d[:, a * WPD:b * WPD])
        bb = min(b, XR - 1)
        nc.sync.dma_start(out=xfold[64:128, 1 + a * WPD:1 + bb * WPD],
                          in_=xs_d[:, (a + 1) * WPD:(bb + 1) * WPD])
    nc.gpsimd.dma_start(out=w1s[:], in_=w1_d.rearrange("p (a b m) -> p a b m",
                                                     a=8, b=2))
    nc.gpsimd.dma_start(out=w1sx[:], in_=w1x_d.rearrange(
        "p (a b m) -> p a b m", a=6, b=2))
    nc.gpsimd.dma_start(out=w1s4[:], in_=w14_d.rearrange(
        "p (a b m) -> p a b m", a=2, b=2))
    for i, (lo, hi) in enumerate([(0, 128), (128, 256), (256, 384),
                                  (384, 512), (512, 576)]):
        nc.gpsimd.dma_start(out=sc1[i][:], in_=sb1_d[lo:hi, :])
    nc.gpsimd.dma_start(out=w2da[:], in_=w2a_d[:])
    nc.gpsimd.dma_start(out=w2fb[:], in_=w2f_d[:])
    nc.gpsimd.dma_start(out=w2gb[:], in_=w2g_d[:])
    nc.gpsimd.dma_start(out=b2a[:], in_=b2_d[0:128, :])
    nc.gpsimd.dma_start(out=b2b[:], in_=b2_d[128:192, :])
    nc.gpsimd.dma_start(
        out=tmps[:],
        in_=bass.AP(tensor=tmp_d, offset=0, ap=[[0, 128], [1, 1]]))

    # ---- border memsets (DVE; only the load-bearing zeros) ----
    # xfold lead/tail and Tall slab leads/tails are read only by discarded
    # junk output columns -- no clears needed there. T4 row 49 IS needed:
    # the zero-weight dummy plane reads it (0 x NaN = NaN).
    tall_b = Tall[:]
    t4_b = T4[:]

    def slabv(s, plo, phi, r0, r1, c0=0, c1=WPD):
        """[phi-plo, r1-r0, c1-c0] view of slab s rows r0..r1."""
        base = tall_b if s < 4 else t4_b
        off = (s % 4) * SL if s < 4 else 0
        ap0 = base.ap[0][0]
        return bass.AP(tensor=base.tensor,
                       offset=base.offset + plo * ap0 + off + 1 + r0 * WPD + c0,
                       ap=[[ap0, phi - plo], [WPD, r1 - r0], [1, c1 - c0]])

    for s in range(4):
        nc.vector.memset(slabv(s, 0, 128, 0, 1), 0.0)              # row 0
        nc.vector.memset(slabv(s, 0, 128, 0, TR, 0, 1), 0.0)       # col 0
        nc.vector.memset(slabv(s, 0, 128, 0, TR, 97, 98), 0.0)     # col 97
    nc.vector.memset(T4[:, 1 + 49 * WPD:SL], 0.0)   # row 49 + slack
    nc.vector.memset(slabv(4, 0, 64, 0, 1), 0.0)
    nc.vector.memset(slabv(4, 0, 64, 0, 49, 0, 1), 0.0)
    nc.vector.memset(slabv(4, 0, 64, 0, 49, 97, 98), 0.0)

    brv = brs.rearrange("p (r w) -> p r w", w=WPD)
    nc.vector.memset(brv[:, 0:1, :], 0.0)
    nc.vector.memset(brv[:, :, 0:1], 0.0)
    nc.vector.memset(brv[:, :, 97:98], 0.0)

    xpv = xpool.rearrange("p (r w) -> p r w", w=WPD)
    obv = ob.rearrange("p (r w) -> p r w", w=WPD)
    qkv = [t.rearrange("p (r w) -> p r w", w=W) for t in qk]
    w2av = w2da.rearrange("p (t m) -> p t m", t=9)
    w2fv = w2fb.rearrange("p (t m) -> p t m", t=3)
    w2gv = w2gb.rearrange("p (t m) -> p t m", t=3)

    xf_b = xfold[:]

    def c0_rhs(s0, d0, s1, d1, nr):
        o0 = 1 + s0 * WPD + d0 - 1
        o1 = 1 + s1 * WPD + d1 - 1
        return bass.AP(tensor=xf_b.tensor, offset=xf_b.offset + o0,
                       ap=[[xf_b.ap[0][0], 128], [o1 - o0, 2], [1, nr * WPD]])

    def c1_rhs(pairs_base, p0, p1, r0, nr):
        (s0, d0) = p0
        if p1 is None:
            s1, d1 = s0, d0 + 2
        else:
            s1, d1 = p1
        off = (lambda s, d: s * SL + 1 + r0 * WPD + d - 1)
        o0, o1 = off(s0, d0), off(s1, d1)
        return bass.AP(tensor=pairs_base.tensor, offset=pairs_base.offset + o0,
                       ap=[[pairs_base.ap[0][0], pairs_base.ap[0][1]],
                           [o1 - o0, 2], [1, nr * WPD]])

    # ---------------- conv0 (fp8 DoubleRow) ----------------
    # out-tile A (m 0:128): x1|x3 -> xpool (bf16); out-tile B (m 128:192):
    # x2 -> Tall slab2 lower partitions (fp8)
    blocksA = _row_blocks(YR)
    blocksB = _row_blocks(TR)
    with tc.tile_pool(name="ps_c0", bufs=8, space="PSUM") as pp0:
        def conv0_block(r0, nr, m0, mw):
            ps = pp0.tile([128, 5 * WPD], F32, tag="c0ps")
            specs = ((r0, 0, r0, 2), (r0, 1, r0 + 1, 1),
                     (r0 + 1, 0, r0 + 1, 2))
            for pi, (s0, d0, s1, d1) in enumerate(specs):
                nc.tensor.matmul(
                    ps[0:mw, 0:nr * WPD], lhsT=w0s[:, pi, :, m0:m0 + mw],
                    rhs=c0_rhs(s0, d0, s1, d1, nr),
                    start=(pi == 0), stop=(pi == 2),
                    perf_mode=mybir.MatmulPerfMode.DoubleRow)
            return ps

        for (r0, nr) in blocksA:
            ps = conv0_block(r0, nr, 0, 128)
            psv = ps.rearrange("p (r x) -> p r x", x=WPD)
            nc.scalar.activation(
                out=xpv[:, r0:r0 + nr, 1:97], in_=psv[:, 0:nr, 1:97],
                func=mybir.ActivationFunctionType.Relu,
                bias=sc0a[:, 1:2], scale=sc0a[:, 0:1])
        for (r0, nr) in blocksB:
            ps = conv0_block(r0, nr, 128, 64)
            psv = ps.rearrange("p (r x) -> p r x", x=WPD)
            # x2 -> ky1 slot = slab2 partitions 0..63 (t rows at tile rows)
            nc.scalar.activation(
                out=slabv(2, 0, 64, r0, r0 + nr, 1, 97),
                in_=psv[0:64, 0:nr, 1:97],
                func=mybir.ActivationFunctionType.Relu,
                bias=sc0b[:, 1:2], scale=sc0b[:, 0:1])

    # x2 ky-shifted slot copies (slab2 col pads are zero so full width)
    def x2_shift(a, b):
        nc.sync.dma_start(out=slabv(0, 64, 128, a + 1, b + 1),
                          in_=slabv(2, 0, 64, a, b))
        nc.sync.dma_start(out=slabv(3, 64, 128, a, b),
                          in_=slabv(2, 0, 64, a + 1, b + 1))

    # ---------------- pools + bilinear (DVE; final writes fp8 brs) --------
    cA = P.tile([128, 2], F32, tag="cA")
    nc.vector.memset(cA[0:64, 0:1], 0.75)
    nc.vector.memset(cA[0:64, 1:2], 0.25)
    nc.vector.memset(cA[64:128, 0:1], 0.1875)
    nc.vector.memset(cA[64:128, 1:2], 0.0625)
    cC = P.tile([128, 1], F32, tag="cC")
    nc.vector.memset(cC[0:64, :], 1.0)
    nc.vector.memset(cC[64:128, :], 0.25)

    pl_chunks = [(0, 5), (5, 10), (10, 15), (15, 20), (20, 25), (25, 26)]
    kv_o = 0   # next odd-row k (vint[2k+1], k<=24)
    kv_e = 0   # next even-row k (vint[2k+2], k<=23)
    hv = 0     # next t-row for the horizontal pass

    def hpass(a, b):
        if b <= a:
            return
        nc.vector.tensor_scalar(out=brv[:, 1 + a:1 + b, 1:2],
                                in0=vint[:, a:b, 0:1], scalar1=cC[:, 0:1],
                                scalar2=None, op0=mybir.AluOpType.mult)
        nc.vector.tensor_scalar(out=brv[:, 1 + a:1 + b, 96:97],
                                in0=vint[:, a:b, 47:48], scalar1=cC[:, 0:1],
                                scalar2=None, op0=mybir.AluOpType.mult)
        nc.vector.tensor_scalar(out=tscr[:, a:b, 0:47], in0=vint[:, a:b, 1:48],
                                scalar1=cA[:, 1:2], scalar2=None,
                                op0=mybir.AluOpType.mult)
        nc.vector.scalar_tensor_tensor(
            out=brv[:, 1 + a:1 + b, 2:96:2], in0=vint[:, a:b, 0:47],
            scalar=cA[:, 0:1], in1=tscr[:, a:b, 0:47],
            op0=mybir.AluOpType.mult, op1=mybir.AluOpType.add)
        nc.vector.tensor_scalar(out=tscr[:, a:b, 0:47], in0=vint[:, a:b, 1:48],
                                scalar1=cA[:, 0:1], scalar2=None,
                                op0=mybir.AluOpType.mult)
        nc.vector.scalar_tensor_tensor(
            out=brv[:, 1 + a:1 + b, 3:96:2], in0=vint[:, a:b, 0:47],
            scalar=cA[:, 1:2], in1=tscr[:, a:b, 0:47],
            op0=mybir.AluOpType.mult, op1=mybir.AluOpType.add)

    for (k0, k1) in pl_chunks:
        # horizontal pool pairs for y rows 2k0..2k1-1
        nc.vector.tensor_tensor(out=plh[0:64, 2 * k0:2 * k1, :],
                                in0=xpv[0:64, 2 * k0:2 * k1, 1:97:2],
                                in1=xpv[0:64, 2 * k0:2 * k1, 2:98:2],
                                op=mybir.AluOpType.max)
        nc.vector.tensor_tensor(out=plh[64:128, 2 * k0:2 * k1, :],
                                in0=xpv[64:128, 2 * k0:2 * k1, 1:97:2],
                                in1=xpv[64:128, 2 * k0:2 * k1, 2:98:2],
                                op=mybir.AluOpType.add)
        # vertical pool pairs -> pl rows k0..k1-1
        nc.vector.tensor_tensor(out=pl[0:64, k0:k1, :],
                                in0=plh[0:64, 2 * k0:2 * k1:2, :],
                                in1=plh[0:64, 2 * k0 + 1:2 * k1:2, :],
                                op=mybir.AluOpType.max)
        nc.vector.tensor_tensor(out=pl[64:128, k0:k1, :],
                                in0=plh[64:128, 2 * k0:2 * k1:2, :],
                                in1=plh[64:128, 2 * k0 + 1:2 * k1:2, :],
                                op=mybir.AluOpType.add)
        if k0 == 0:
            nc.vector.tensor_copy(out=vint[:, 0, :], in_=pl[:, 0, :])
        # vertical bilinear rows that only need pl rows < k1
        ke_o = min(k1 - 1, 25)
        if ke_o > kv_o:
            a, b = kv_o, ke_o
            nc.vector.tensor_scalar(out=tscr[:, a:b, :], in0=pl[:, a + 1:b + 1, :],
                                    scalar1=0.25, scalar2=None,
                                    op0=mybir.AluOpType.mult)
            nc.vector.scalar_tensor_tensor(
                out=vint[:, 2 * a + 1:2 * b:2, :], in0=pl[:, a:b, :],
                scalar=0.75, in1=tscr[:, a:b, :],
                op0=mybir.AluOpType.mult, op1=mybir.AluOpType.add)
            kv_o = ke_o
        ke_e = min(k1 - 1, 24)
        if ke_e > kv_e:
            a, b = kv_e, ke_e
            nc.vector.tensor_scalar(out=tscr[:, a:b, :], in0=pl[:, a + 1:b + 1, :],
                                    scalar1=0.75, scalar2=None,
                                    op0=mybir.AluOpType.mult)
            nc.vector.scalar_tensor_tensor(
                out=vint[:, 2 * a + 2:2 * b + 1:2, :], in0=pl[:, a:b, :],
                scalar=0.25, in1=tscr[:, a:b, :],
                op0=mybir.AluOpType.mult, op1=mybir.AluOpType.add)
            kv_e = ke_e
        # horizontal pass over fully-available vint rows
        avail = min(2 * kv_o + 1, 2 * kv_e + 2) if k1 < 26 else TR
        hpass(hv, avail)
        hv = avail

    # brs rows (fp8) -> T slab slots; row-chunked for conv1 pipelining.
    # (ky slot s stores t row rho at tile row rho+1-s.)
    x2s = iter(((0, 10), (10, 30), (30, 49)))
    for ci, (a, b) in enumerate(((0, 8), (8, 18), (18, 28), (28, 38), (38, 50))):
        nc.sync.dma_start(out=slabv(0, 0, 64, a, b), in_=brv[0:64, a:b, :])
        nc.sync.dma_start(out=slabv(1, 0, 64, a, b), in_=brv[64:128, a:b, :])
        nc.sync.dma_start(out=slabv(1, 64, 128, a, b),
                          in_=brv[0:64, a + 1:b + 1, :])
        nc.sync.dma_start(out=slabv(2, 64, 128, a, b),
                          in_=brv[64:128, a + 1:b + 1, :])
        bb = min(b, 49)
        nc.sync.dma_start(out=slabv(3, 0, 64, a, bb),
                          in_=brv[0:64, a + 2:bb + 2, :])
        nc.sync.dma_start(out=slabv(4, 0, 64, a, bb),
                          in_=brv[64:128, a + 2:bb + 2, :])
        if ci < 3:
            x2_shift(*next(x2s))

    # oa (= xpool reuse) borders for the depthwise reads; ob borders
    oav = xpv[:, 0:50, :]
    nc.vector.memset(oav[:, 0:1, :], 0.0)
    nc.vector.memset(oav[:, :, 0:1], 0.0)
    nc.vector.memset(oav[:, :, 97:98], 0.0)
    nc.vector.memset(obv[:, 0:1, :], 0.0)
    nc.vector.memset(obv[0:64, :, 1:2], 0.0)
    nc.vector.memset(obv[64:128, :, 0:1], 0.0)
    nc.vector.memset(obv[:, :, 97:98], 0.0)

    # ---------------- conv1 (fp8 DoubleRow) + attention prologue ----------
    qk_blocks = _row_blocks(SR)
    v_blocks = _row_blocks(QR)

    def conv1_block(ot, bi, r0, nr, evac, dense=False):
        mw = 64 if ot == 4 else 128
        m0 = 128 * ot
        ps = pp1.tile([128, 5 * WPD], F32, tag="c1ps")
        pairs, wt = ((PAIRS_C1X, w1sx) if dense else (PAIRS_C1, w1s))
        for pi, (p0, p1) in enumerate(pairs):
            nc.tensor.matmul(
                ps[0:mw, 0:nr * WPD], lhsT=wt[:, pi, :, m0:m0 + mw],
                rhs=c1_rhs(tall_b, p0, p1, r0, nr),
                start=(pi == 0), stop=False,
                perf_mode=mybir.MatmulPerfMode.DoubleRow)
        for qi, (p0, p1) in enumerate(PAIRS_T4):
            nc.tensor.matmul(
                ps[0:mw, 0:nr * WPD], lhsT=w1s4[:, qi, :, m0:m0 + mw],
                rhs=c1_rhs(t4_b, p0, p1, r0, nr),
                start=False, stop=(qi == 1),
                perf_mode=mybir.MatmulPerfMode.DoubleRow)
        psv = ps.rearrange("p (r x) -> p r x", x=WPD)
        evac(bi, r0, nr, psv, mw)

    def conv1_tile(ot, blocks, evac):
        for bi, (r0, nr) in enumerate(blocks):
            conv1_block(ot, bi, r0, nr, evac, dense=True)

    def evac_qk(ot):
        def f(bi, r0, nr, psv, mw):
            nc.scalar.activation(
                out=qkv[ot][:, r0:r0 + nr, :], in_=psv[:, 0:nr, 1:97],
                func=mybir.ActivationFunctionType.Relu,
                bias=sc1[ot][:, 1:2], scale=sc1[ot][:, 0:1])
            dump = ev.tile([128, 5 * W], BF16, tag="sqd")
            nc.vector.tensor_tensor(
                out=dump[:, 0:nr * W],
                in0=qk[ot][:, r0 * W:(r0 + nr) * W],
                in1=qk[ot][:, r0 * W:(r0 + nr) * W],
                op=mybir.AluOpType.mult)
            nc.vector.reduce_sum(out=sqac[ot][:, bi:bi + 1],
                                 in_=dump[:, 0:nr * W],
                                 axis=mybir.AxisListType.X)
        return f

    def evac_v(ot):
        off = 0 if ot == 3 else VS

        def f(bi, r0, nr, psv, mw):
            dst = vall[0:mw, off + r0 * W:off + (r0 + nr) * W]
            nc.scalar.activation(
                out=dst, in_=psv[0:mw, 0:nr, 1:97],
                func=mybir.ActivationFunctionType.Relu,
                bias=sc1[ot][:, 1:2], scale=sc1[ot][:, 0:1])
        return f

    with tc.tile_pool(name="ps_c1", bufs=3, space="PSUM") as pp1, \
         tc.tile_pool(name="ps_tr", bufs=2, space="PSUM") as ppt, \
         tc.tile_pool(name="ps_s", bufs=1, space="PSUM") as pps:

        # conv1 qk row-blocks interleaved across the 3 out-tiles, with S'
        # transpose groups issued as soon as their pixel chunks are covered.
        qk0r = qk[0].rearrange("p (c k) -> p c k", k=128)
        qk1r = qk[1].rearrange("p (c k) -> p c k", k=128)
        qk2r = qk[2].rearrange("p (c k) -> p c k", k=128)
        sp = pps.tile([128, 2 * C], F32, tag="sp")

        def s_group(g):
            tq = ppt.tile([128, 3 * C], BF16, tag="tq")
            tk = ppt.tile([128, 3 * C], BF16, tag="tk")
            tqv = tq.rearrange("p (i c) -> p i c", c=C)
            tkv = tk.rearrange("p (i c) -> p i c", c=C)
            for i in range(3):
                ci = 3 * g + i
                nc.tensor.transpose(tqv[:, i, 0:128], qk0r[:, ci, :], ident[:])
                nc.tensor.transpose(tqv[:, i, 128:192], qk1r[0:64, ci, :],
                                    ident[0:64, 0:64])
                nc.tensor.transpose(tkv[:, i, 0:64], qk1r[64:128, ci, :],
                                    ident[64:128, 64:128])
                nc.tensor.transpose(tkv[:, i, 64:192], qk2r[:, ci, :], ident[:])
            qtc = ev.tile([128, 3 * C], BF16, tag="qtc")
            ktc = ev.tile([128, 3 * C], BF16, tag="ktc")
            nc.scalar.copy(out=qtc[:], in_=tq[:])
            nc.vector.tensor_copy(out=ktc[:], in_=tk[:])
            qcv = qtc.rearrange("p (i c) -> p i c", c=C)
            kcv = ktc.rearrange("p (i c) -> p i c", c=C)
            for i in range(3):
                nc.tensor.matmul(sp[:, 0:C], lhsT=kcv[:, i, 0:128],
                                 rhs=qcv[:, i, :],
                                 start=(g == 0 and i == 0),
                                 stop=(g == NCH // 3 - 1 and i == 2))
                nc.tensor.matmul(sp[0:64, C:2 * C], lhsT=kcv[:, i, 128:192],
                                 rhs=qcv[:, i, :],
                                 start=(g == 0 and i == 0),
                                 stop=(g == NCH // 3 - 1 and i == 2))

        next_g = 0
        for bi, (r0, nr) in enumerate(qk_blocks):
            for ot in range(3):
                conv1_block(ot, bi, r0, nr, evac_qk(ot), dense=(bi >= 4))
            px_done = (r0 + nr) * W
            while next_g < NCH // 3 and (3 * next_g + 3) * 128 <= px_done:
                s_group(next_g)
                next_g += 1
        while next_g < NCH // 3:
            s_group(next_g)
            next_g += 1

        # deferred zero-clears (needed only from P@V onward): ptall plane1
        # upper rows are a zero K-pad for the P@V DR weights; vall plane1
        # upper rows are junk read by that zero plane -- must be non-NaN
        # (0 x NaN = NaN).
        nc.vector.memset(ptall[64:128, C:2 * C], 0.0)
        nc.vector.memset(vall[64:128, VS:2 * VS], 0.0)

        # sumsq totals + AllReduce staging
        for ti in range(3):
            nc.vector.reduce_sum(out=accs[ti][:], in_=sqac[ti][:],
                                 axis=mybir.AxisListType.X)
        nc.vector.tensor_copy(out=ssa[:, 192:193], in_=accs[0][:])
        nc.vector.tensor_copy(out=ssb[:, 192:193], in_=accs[1][0:64, :])
        nc.vector.tensor_copy(out=accb[:, 0:1], in_=accs[1][:])
        nc.vector.tensor_copy(out=accb[:, 1:2], in_=accs[2][:])
        nc.sync.dma_start(out=ssa[0:64, 193:194], in_=accb[64:128, 0:1])
        nc.sync.dma_start(out=ssa[64:128, 193:194], in_=accb[0:64, 1:2])
        nc.sync.dma_start(out=ssb[0:64, 193:194], in_=accb[64:128, 1:2])
        nc.scalar.copy(out=ssa[:, 0:192], in_=sp[:, 0:C])
        nc.scalar.copy(out=ssb[:, 0:192], in_=sp[0:64, C:2 * C])
        nc.sync.dma_start(out=cc2i[0:64, :], in_=ssa[0:64, :])
        nc.scalar.dma_start(out=cc2i[64:128, :], in_=ssa[64:128, :])
        nc.gpsimd.dma_start(out=cc2i[128:192, :], in_=ssb[:])
        nc.gpsimd.collective_compute(
            "AllReduce", mybir.AluOpType.add, replica_groups=GROUPS,
            ins=[cc2i[:]], outs=[cc2o[:]])
        # sumsq columns first: the norm chain needs only these
        nc.gpsimd.dma_start(out=sfa[:, 192:194], in_=cc2o[0:128, 192:194])
        nc.gpsimd.dma_start(out=sfb[:, 192:194], in_=cc2o[128:192, 192:194])
        nc.sync.dma_start(out=sfa[0:64, 0:192], in_=cc2o[0:64, 0:192])
        nc.scalar.dma_start(out=sfa[64:128, 0:192], in_=cc2o[64:128, 0:192])
        nc.sync.dma_start(out=sfb[:, 0:192], in_=cc2o[128:192, 0:192])

        # conv1 v tiles overlap the collective
        conv1_tile(3, v_blocks, evac_v(3))
        conv1_tile(4, v_blocks, evac_v(4))

    # ---------------- transpose-free softmax -> fp8 P^T planes ------------
    with tc.tile_pool(name="ps_sm", bufs=1, space="PSUM") as ppm, \
         tc.tile_pool(name="ps_pv", bufs=2, space="PSUM") as ppv, \
         tc.tile_pool(name="ps_dw", bufs=2, space="PSUM") as ppd:
        psq = ppm.tile([1, C], BF16, tag="psq")
        pm = ppm.tile([128, 392], F32, tag="pm")
        bcast = pm[:, 0:192]
        pcs = pm[0:1, 196:388]

        nc.tensor.transpose(psq[0:1, 0:128], sfa[:, 192:193], ident[:])
        nc.tensor.transpose(psq[0:1, 128:192], sfb[0:64, 192:193],
                            ident[0:64, 0:64])
        nc.vector.tensor_scalar(out=qs[:], in0=psq[0:1, :], scalar1=1e-24,
                                scalar2=None, op0=mybir.AluOpType.max)
        nc.vector.tensor_scalar(out=rska[:], in0=sfa[:, 193:194],
                                scalar1=1e-24, scalar2=None,
                                op0=mybir.AluOpType.max)
        nc.vector.tensor_scalar(out=rskb[:], in0=sfb[0:64, 193:194],
                                scalar1=1e-24, scalar2=None,
                                op0=mybir.AluOpType.max)
        for r in (qs, rska, rskb):
            nc.vector.reciprocal(out=r[:], in_=r[:])
            nc.scalar.activation(out=r[:], in_=r[:],
                                 func=mybir.ActivationFunctionType.Sqrt)
        nc.tensor.matmul(bcast, lhsT=ones1[:, :], rhs=qs[:],
                         start=True, stop=True)
        nc.vector.scalar_tensor_tensor(
            out=ea[:], in0=sfa[:, 0:192], scalar=tmps[:, 0:1], in1=bcast,
            op0=mybir.AluOpType.mult, op1=mybir.AluOpType.mult)
        nc.vector.scalar_tensor_tensor(
            out=eb[:], in0=sfb[0:64, 0:192], scalar=tmps[0:64, 0:1],
            in1=bcast[0:64, :],
            op0=mybir.AluOpType.mult, op1=mybir.AluOpType.mult)
        # P^T planes (unnormalized): plane0 = keys 0:128, plane1 = keys 128:192
        nc.scalar.activation(out=ptall[:, 0:C], in_=ea[:],
                             func=mybir.ActivationFunctionType.Exp,
                             bias=0.0, scale=rska[:, 0:1])
        nc.scalar.activation(out=ptall[0:64, C:2 * C], in_=eb[:],
                             func=mybir.ActivationFunctionType.Exp,
                             bias=0.0, scale=rskb[:, 0:1])
        # out = P^T.T @ v (fp8 DoubleRow over the two key planes), interleaved
        # with depthwise-A blocks so dw matmuls hide the P@V evac latency.
        ptv = ptall.rearrange("p (two q) -> p two q", two=2)
        va_b = vall[:]
        yv = yout.rearrange("c (r w) -> c r w", w=W)

        def pv_rhs(r0, nr):
            return bass.AP(tensor=va_b.tensor, offset=va_b.offset + r0 * W,
                           ap=[[va_b.ap[0][0], 128], [VS, 2], [1, nr * W]])

        def pv_block(r0, nr):
            po = ppv.tile([128, 5 * W], F32, tag="po")
            po2 = ppv.tile([64, 5 * W], F32, tag="po2")
            nc.tensor.matmul(po[:, 0:nr * W], lhsT=ptv[:, :, 0:128],
                             rhs=pv_rhs(r0, nr), start=True, stop=True,
                             perf_mode=mybir.MatmulPerfMode.DoubleRow)
            nc.tensor.matmul(po2[0:64, 0:nr * W], lhsT=ptv[:, :, 128:192],
                             rhs=pv_rhs(r0, nr), start=True, stop=True,
                             perf_mode=mybir.MatmulPerfMode.DoubleRow)
            pov = po.rearrange("p (r w) -> p r w", w=W)
            po2v = po2.rearrange("p (r w) -> p r w", w=W)
            nc.scalar.copy(out=oav[:, r0 + 1:r0 + 1 + nr, 1:97],
                           in_=pov[:, 0:nr, :])
            nc.vector.tensor_copy(out=obv[0:64, r0 + 1:r0 + 1 + nr, 2:98],
                                  in_=po2v[0:64, 0:nr, :])

        def dwa_block(r0, nr):
            ps = ppd.tile([128, 5, W], F32, tag="dwps")
            for t in range(9):
                dy, dx = t // 3 - 1, t % 3 - 1
                nc.tensor.matmul(
                    ps[:, 0:nr, :],
                    lhsT=w2av[:, t, :],
                    rhs=oav[:, r0 + 1 + dy:r0 + 1 + dy + nr, 1 + dx:97 + dx],
                    start=(t == 0), stop=(t == 8))
            fo = ev.tile([128, 5, W], F32, tag="fo")
            nc.scalar.activation(out=fo[:, 0:nr, :], in_=ps[:, 0:nr, :],
                                 func=mybir.ActivationFunctionType.Identity,
                                 bias=b2a[:, 0:1], scale=rsA[:, 0:1])
            nc.sync.dma_start(out=yv[0:128, r0:r0 + nr, :],
                              in_=fo[:, 0:nr, :])

        dw_blocks = _row_blocks(OR_)
        pv_block(*v_blocks[0])
        pv_block(*v_blocks[1])
        # softmax denominator: colsum over keys via ones-matmul
        nc.tensor.matmul(pcs, lhsT=ones8[:, 0:1],
                         rhs=ptall[:, 0:C], start=True, stop=False)
        nc.tensor.matmul(pcs, lhsT=ones8[:, 0:1],
                         rhs=ptall[:, C:2 * C], start=False, stop=True)
        nc.vector.reciprocal(out=isr[:], in_=pcs)
        prt = pm[:, 192:194]
        nc.tensor.transpose(prt[0:128, 0:1], isr[0:1, 0:128],
                            identf[0:1, 0:1])
        nc.tensor.transpose(prt[0:64, 1:2], isr[0:1, 128:192],
                            identf[0:1, 0:1])
        nc.vector.tensor_copy(out=rsA[:], in_=prt[0:128, 0:1])
        nc.vector.tensor_copy(out=rsB[:], in_=prt[0:64, 1:2])

        for bi in range(2, len(v_blocks)):
            dwa_block(*dw_blocks[bi - 2])
            pv_block(*v_blocks[bi])
            if bi == 5:
                # replicate ob rows 1..25 (needs P@V-B evacs through block 4)
                nc.sync.dma_start(out=obv[64:128, 1:26, 1:97],
                                  in_=obv[0:64, 1:26, 2:98])
        nc.sync.dma_start(out=obv[64:128, 26:50, 1:97],
                          in_=obv[0:64, 26:50, 2:98])
        dwa_block(*dw_blocks[8])
        dwa_block(*dw_blocks[9])
        for (r0, nr) in dw_blocks:
            ps = ppd.tile([128, 5, W], F32, tag="dwps")
            for dy in range(3):
                nc.tensor.matmul(
                    ps[0:64, 0:nr, :], lhsT=w2fv[:, dy, :],
                    rhs=obv[:, r0 + dy:r0 + dy + nr, 1:97],
                    start=(dy == 0), stop=False)
                nc.tensor.matmul(
                    ps[0:64, 0:nr, :], lhsT=w2gv[64:128, dy, :],
                    rhs=obv[64:128, r0 + dy:r0 + dy + nr, 2:98],
                    start=False, stop=(dy == 2))
            fo = ev.tile([128, 5, W], F32, tag="fo")
            nc.scalar.activation(out=fo[0:64, 0:nr, :], in_=ps[0:64, 0:nr, :],
                                 func=mybir.ActivationFunctionType.Identity,
                                 bias=b2b[:, 0:1], scale=rsB[:, 0:1])
            nc.sync.dma_start(out=yv[128:192, r0:r0 + nr, :],
                              in_=fo[0:64, 0:nr, :])
    ctx.close()


# ---------------- host side ----------------
_NC_CACHE = None


def _get_nc():
    global _NC_CACHE
    if _NC_CACHE is None:
        _NC_CACHE = build_nc()
    return _NC_CACHE


def _pack_weights(inp, flip):
    bf = ml_dtypes.bfloat16
    w0 = inp["w0"][:, :, ::-1, :] if flip else inp["w0"]
    w1 = inp["w1"][:, :, ::-1, :] if flip else inp["w1"]
    w2 = inp["w2"][:, :, ::-1, :] if flip else inp["w2"]
    w0 = np.asarray(w0, np.float32)
    w1 = np.asarray(w1, np.float32)
    w2 = np.asarray(w2, np.float32)

    # conv0: out-channel order [x1(0:64), x3(128:192), x2(64:128)]
    cho = np.concatenate([np.arange(0, 64), np.arange(128, 192),
                          np.arange(64, 128)])
    wt = w0[cho].transpose(1, 2, 3, 0)        # [64c, 3ky, 3kx, 192m]
    w0p = np.zeros((128, 3, 2, C), np.float32)
    w0p[0:64, 0, 0] = wt[:, 0, 0]
    w0p[0:64, 0, 1] = wt[:, 0, 2]
    w0p[0:64, 1, 0] = wt[:, 0, 1]
    w0p[64:128, 0, 0] = wt[:, 1, 0]
    w0p[64:128, 0, 1] = wt[:, 1, 2]
    w0p[64:128, 1, 0] = wt[:, 1, 1]
    w0p[64:128, 1, 1] = wt[:, 2, 1]
    w0p[64:128, 2, 0] = wt[:, 2, 0]
    w0p[64:128, 2, 1] = wt[:, 2, 2]
    s0 = inp["g0"] / np.sqrt(inp["v0"] + BN_EPS)
    t0 = inp["be0"] + (inp["b0"] - inp["m0"]) * s0
    sb0 = np.stack([s0 / W0S, t0], axis=1).astype(np.float32)[cho]

    # conv1 DoubleRow packs.  Slab k-partition -> (ky, c) maps:
    def slab_map(s):
        k = np.arange(128 if s < 4 else 64)
        if s == 0:
            return np.zeros_like(k), k
        if s == 1:
            return np.where(k < 64, 0, 1), np.where(k < 64, 128 + k, k - 64)
        if s == 2:
            return np.ones_like(k), 64 + k
        if s == 3:
            return np.full_like(k, 2), k
        return np.full_like(k, 2), 128 + k

    wt1 = w1.transpose(1, 2, 3, 0)            # [192c, 3ky, 3kx, 576m]
    w1p = np.zeros((128, 8, 2, C3), np.float32)
    for pi, (p0, p1) in enumerate(PAIRS_C1):
        for pl, spec in enumerate((p0, p1)):
            if spec is None:
                continue
            s, dx = spec
            ky, cc = slab_map(s)
            w1p[:, pi, pl] = wt1[cc, ky, dx]
    w1px = np.zeros((128, 6, 2, C3), np.float32)
    for pi, (p0, p1) in enumerate(PAIRS_C1X):
        for pl, (s, dx) in enumerate((p0, p1)):
            ky, cc = slab_map(s)
            w1px[:, pi, pl] = wt1[cc, ky, dx]
    w1p4 = np.zeros((64, 2, 2, C3), np.float32)
    ky4, cc4 = slab_map(4)
    w1p4[:, 0, 0] = wt1[cc4, ky4, 0]
    w1p4[:, 0, 1] = wt1[cc4, ky4, 2]
    w1p4[:, 1, 0] = wt1[cc4, ky4, 1]
    s1 = inp["g1"] / np.sqrt(inp["v1"] + BN_EPS)
    t1 = inp["be1"] + (inp["b1"] - inp["m1"]) * s1
    sb1 = np.stack([s1 / W1S, t1], axis=1).astype(np.float32)

    w2da = np.zeros((128, 9, 128), np.float32)
    w2fb = np.zeros((128, 3, 64), np.float32)
    w2gb = np.zeros((128, 3, 64), np.float32)
    r64, r128 = np.arange(64), np.arange(128)
    for t in range(9):
        d = w2[:, 0, t // 3, t % 3]
        w2da[r128, t, r128] = d[0:128]
    for dy in range(3):
        db = w2[128:192, 0, dy, :]
        w2fb[r64, dy, r64] = db[:, 0]
        w2fb[64 + r64, dy, r64] = db[:, 1]
        w2gb[64 + r64, dy, r64] = db[:, 2]

    out = {
        "w0p": np.ascontiguousarray(
            (w0p * W0S).reshape(128, 3 * 2 * C)).astype(f8np),
        "sb0p": sb0,
        "w1p": np.ascontiguousarray(
            (w1p * W1S).reshape(128, 8 * 2 * C3)).astype(f8np),
        "w1px": np.ascontiguousarray(
            (w1px * W1S).reshape(128, 6 * 2 * C3)).astype(f8np),
        "w1p4": np.ascontiguousarray(
            (w1p4 * W1S).reshape(64, 2 * 2 * C3)).astype(f8np),
        "sb1": sb1,
        "w2da": np.ascontiguousarray(w2da.reshape(128, 9 * 128)).astype(bf),
        "w2fb": np.ascontiguousarray(w2fb.reshape(128, 3 * 64)).astype(bf),
        "w2gb": np.ascontiguousarray(w2gb.reshape(128, 3 * 64)).astype(bf),
        "b2v": np.asarray(inp["b2"], np.float32).reshape(C, 1),
    }
    return out


def kernel(**inputs):
    inputs = {k: np.asarray(v) for k, v in inputs.items()}
    x = inputs["x"]
    B = x.shape[0]
    packs = [_pack_weights(inputs, flip) for flip in (False, True)]
    tempv = np.asarray(inputs["temp"], np.float32).reshape(1, 1)

    in_maps = []
    for core in range(8):
        s, h = core // 2, core % 2
        xi = np.asarray(x[s], np.float32)
        if h:
            xi = xi[:, ::-1, :]
        slab = np.zeros((64, XR, WPD), np.float32)
        slab[:, 1:54, 1:97] = xi[:, 0:53, :]
        m = dict(packs[h])
        m["xs"] = np.ascontiguousarray(slab.reshape(64, XR * WPD)).astype(f8np)
        m["tempv"] = tempv
        m["zpad8"] = np.zeros((1, 512), f8np)
        m["zpad16"] = np.zeros((1, 512), ml_dtypes.bfloat16)
        in_maps.append(m)

    nc = _get_nc()
    res = run_bass_kernel_spmd(nc, in_maps, list(range(8)))
    out = np.zeros((B, C, 96, 96), np.float32)
    for core in range(8):
        s, h = core // 2, core % 2
        yc = res.results[core]["yout"].reshape(C, OR_, W)
        if h:
            out[s, :, 48:96] = yc[:, ::-1, :]
        else:
            out[s, :, 0:48] = yc
    return out


# revision 47
# speedup vs baseline: 1.0339x; 1.0044x over previous
"""Bass/Trainium2 kernel for nn_CSEM sparse_attention problem (v3).

Sharding: 8 cores = 4 samples x 2 spatial halves (bottom half vertically
flipped on host so all cores run an identical NEFF).

v3 vs v2: fp8e4 DoubleRow matmuls for conv0 / conv1 / P@V (one DR matmul
accumulates two (weights-plane, ifmap-plane) K-tiles at 0.5 cycles/row).
Activations and weights for those convs are fp8; conv taps are addressed as
column/row offsets into padded flat slabs so tap pairs become stride-`d`
plane pairs of a single AP. Softmax is transpose-free: S'^T stays key-major,
q-norms are broadcast along partitions, exp goes straight to fp8 P^T planes,
and the softmax denominator (from an ones-matmul colsum) is folded into the
depthwise-conv evacuation scale. Depthwise conv + q/k transposes + S' stay
bf16 for accuracy.
"""

import numpy as np
import ml_dtypes

import concourse.bass as bass
import concourse.mybir as mybir
import concourse.tile as tile
from concourse.bass_utils import run_bass_kernel_spmd
from concourse.masks import make_identity

F8 = mybir.dt.float8e4
BF16 = mybir.dt.bfloat16
F32 = mybir.dt.float32
BN_EPS = 1e-5
f8np = ml_dtypes.float8_e4m3

CIN, C, C3 = 64, 192, 576
W, WPD = 96, 98
XR = 54          # x slab rows (1 zero pad + 53 data)
YR = 52          # y rows computed locally (0..51)
TR = 50          # t rows 0..49
QR = 49          # conv1 v rows 0..48
SR = 48          # rows feeding S partial
OR_ = 48         # final output rows per core
NCH = SR * W // 128   # 36 transpose chunks
VS = QR * W      # v plane stride (4704)
SL = 4908        # Tall slab stride (1 lead + 50*98 data + slack)
XLEN = 1 + XR * WPD + 8
W0S, W1S = 32.0, 64.0
GROUPS = [[0, 1], [2, 3], [4, 5], [6, 7]]

# conv1 DoubleRow pairs as ((slab, dx), (slab, dx)); None = zero-weight dummy
# plane (rhs offset +2 -- a dim1 stride equal to the element stride wedges the
# device). Pairs stay within one slab so each block's read footprint is
# row-local and conv1 pipelines with the bilinear-branch writes.
PAIRS_C1 = [((0, 0), (0, 2)), ((0, 1), None), ((1, 0), (1, 2)),
            ((1, 1), None), ((2, 0), (2, 2)), ((2, 1), None),
            ((3, 0), (3, 2)), ((3, 1), None)]
PAIRS_T4 = [((0, 0), (0, 2)), ((0, 1), None)]   # slab idx relative to T4
# denser cross-slab pairing (no dummy planes): used for row-blocks issued
# after the bilinear branch has fully landed, where the wide byte-interval
# footprint of a cross-slab pair cannot stall the pipeline.
PAIRS_C1X = [((0, 0), (0, 2)), ((1, 0), (1, 2)), ((2, 0), (2, 2)),
             ((3, 0), (3, 2)), ((0, 1), (1, 1)), ((2, 1), (3, 1))]


def _split_waits(nc, limit=1):
    """This walrus build rejects instructions carrying more than one sem-wait
    command. Spread extra waits onto same-engine ENGINE_NOPs inserted just
    before the offending instruction (semantically identical: the engine
    blocks on each wait in program order)."""
    ctr = [0]
    for f in nc.m.functions:
        for blk in f.blocks:
            il = blk.instructions
            new = []
            for inst in il:
                si = inst.sync_info
                waits = list(si.on_wait) if (si and si.on_wait) else []
                if len(waits) > limit:
                    for w in waits[:-limit]:
                        ctr[0] += 1
                        nop = mybir.InstNoOp(name=f"WNOP-{ctr[0]}")
                        nop.engine = inst.engine
                        nop.sync_info = mybir.SyncInfo(on_wait=[w], on_update=[])
                        new.append(nop)
                    si.on_wait = waits[-limit:]
                new.append(inst)
            il[:] = new


def _row_blocks(nrows, per=5):
    out, r = [], 0
    while r < nrows:
        n = min(per, nrows - r)
        out.append((r, n))
        r += n
    return out


def build_nc():
    nc = bass.Bass()

    xs_d = nc.declare_dram_parameter("xs", [64, XR * WPD], F8, isOutput=False)
    w0_d = nc.declare_dram_parameter("w0p", [128, 3 * 2 * C], F8, isOutput=False)
    sb0_d = nc.declare_dram_parameter("sb0p", [C, 2], F32, isOutput=False)
    w1_d = nc.declare_dram_parameter("w1p", [128, 8 * 2 * C3], F8, isOutput=False)
    w1x_d = nc.declare_dram_parameter("w1px", [128, 6 * 2 * C3], F8,
                                      isOutput=False)
    w14_d = nc.declare_dram_parameter("w1p4", [64, 2 * 2 * C3], F8, isOutput=False)
    sb1_d = nc.declare_dram_parameter("sb1", [C3, 2], F32, isOutput=False)
    w2a_d = nc.declare_dram_parameter("w2da", [128, 9 * 128], BF16, isOutput=False)
    w2f_d = nc.declare_dram_parameter("w2fb", [128, 3 * 64], BF16, isOutput=False)
    w2g_d = nc.declare_dram_parameter("w2gb", [128, 3 * 64], BF16, isOutput=False)
    b2_d = nc.declare_dram_parameter("b2v", [C, 1], F32, isOutput=False)
    tmp_d = nc.declare_dram_parameter("tempv", [1, 1], F32, isOutput=False)
    z8_d = nc.declare_dram_parameter("zpad8", [1, 512], F8, isOutput=False)
    z16_d = nc.declare_dram_parameter("zpad16", [1, 512], BF16, isOutput=False)
    yout = nc.declare_dram_parameter("yout", [C, OR_ * W], F32, isOutput=True)

    cc2i = nc.dram_tensor("cc2i", [C, 194], BF16)
    cc2o = nc.dram_tensor("cc2o", [C, 194], BF16)

    with tile.TileContext(nc) as tc:
        _body(nc, tc, xs_d, w0_d, sb0_d, w1_d, w1x_d, w14_d, sb1_d, w2a_d,
              w2f_d, w2g_d, b2_d, tmp_d, z8_d, z16_d, yout, cc2i, cc2o)
    _split_waits(nc)
    return nc


def _body(nc, tc, xs_d, w0_d, sb0_d, w1_d, w1x_d, w14_d, sb1_d, w2a_d,
          w2f_d, w2g_d, b2_d, tmp_d, z8_d, z16_d, yout, cc2i, cc2o):
    import contextlib
    ctx = contextlib.ExitStack()
    P = ctx.enter_context(tc.tile_pool(name="persist", bufs=1))
    ev = ctx.enter_context(tc.tile_pool(name="evac", bufs=3))

    # ---- persistent SBUF ----
    xfold = P.tile([128, XLEN], F8, tag="xfold")
    w0s = P.tile([128, 3, 2, C], F8, tag="w0s")
    w1s = P.tile([128, 8, 2, C3], F8, tag="w1s")
    w1sx = P.tile([128, 6, 2, C3], F8, tag="w1sx")
    w1s4 = P.tile([64, 2, 2, C3], F8, tag="w1s4")
    sc0a = P.tile([128, 2], F32, tag="sc0a")
    sc0b = P.tile([64, 2], F32, tag="sc0b")
    scp = [128, 128, 128, 128, 64]
    sc1 = [P.tile([scp[i], 2], F32, tag=f"sc1_{i}", name=f"sc1_{i}")
           for i in range(5)]
    w2da = P.tile([128, 9 * 128], BF16, tag="w2da")
    w2fb = P.tile([128, 3 * 64], BF16, tag="w2fb")
    w2gb = P.tile([128, 3 * 64], BF16, tag="w2gb")  # data at partitions 64..
    b2a = P.tile([128, 1], F32, tag="b2a")
    b2b = P.tile([64, 1], F32, tag="b2b")
    tmps = P.tile([128, 1], F32, tag="tmps")

    Tall = P.tile([128, 4 * SL], F8, tag="Tall")
    T4 = P.tile([64, SL], F8, tag="T4")
    xpool = P.tile([128, YR * WPD], BF16, tag="xpool")   # later reused as oa
    plh = P.tile([128, YR, 48], BF16, tag="plh")
    pl = P.tile([128, 26, 48], BF16, tag="pl")
    vint = P.tile([128, TR, 48], BF16, tag="vint")
    tscr = P.tile([128, TR, 48], BF16, tag="tscr")
    brs = P.tile([128, 51 * WPD], F8, tag="brs")
    ob = P.tile([128, TR * WPD], BF16, tag="ob")
    qk = [P.tile([128, SR * W], BF16, tag=f"qk{i}", name=f"qk{i}")
          for i in range(3)]
    vall = P.tile([128, 2 * VS], F8, tag="vall")
    sqac = [P.tile([128, 10], F32, tag=f"sqac{i}", name=f"sqac{i}")
            for i in range(3)]
    accs = [P.tile([128, 1], F32, tag=f"accs{i}", name=f"accs{i}")
            for i in range(3)]
    ssa = P.tile([128, 194], BF16, tag="ssa")
    ssb = P.tile([64, 194], BF16, tag="ssb")
    sfa = P.tile([128, 194], BF16, tag="sfa")
    sfb = P.tile([64, 194], BF16, tag="sfb")
    accb = P.tile([128, 2], BF16, tag="accb")
    rska = P.tile([128, 1], F32, tag="rska")
    rskb = P.tile([64, 1], F32, tag="rskb")
    qs = P.tile([1, C], F32, tag="qs")
    rqbc = P.tile([128, C], F32, tag="rqbc")
    ea = P.tile([128, C], F32, tag="ea")
    eb = P.tile([64, C], F32, tag="eb")
    ptall = P.tile([128, 2 * C], F8, tag="ptall")
    isr = P.tile([1, C], F32, tag="isr")
    rsA = P.tile([128, 1], F32, tag="rsA")
    rsB = P.tile([64, 1], F32, tag="rsB")
    ones8 = P.tile([128, 1], F8, tag="ones8")
    ones1 = P.tile([1, 128], F32, tag="ones1")

    ident = P.tile([128, 128], BF16, tag="ident")
    make_identity(nc, ident[:])
    identf = P.tile([128, 128], F32, tag="identf")
    make_identity(nc, identf[:])
    nc.vector.memset(ones8[:], 1.0)
    nc.vector.memset(ones1[:], 1.0)

    # ---- input DMAs (x chunked so conv0 starts early) ----
    nc.gpsimd.dma_start(out=w0s[:], in_=w0_d.rearrange("p (a b m) -> p a b m",
                                                       a=3, b=2))
    nc.gpsimd.dma_start(out=sc0a[:], in_=sb0_d[0:128, :])
    nc.gpsimd.dma_start(out=sc0b[:], in_=sb0_d[128:192, :])
    for (a, b) in ((0, 8), (8, 32), (32, XR)):
        nc.sync.dma_start(out=xfold[0:64, 1 + a * WPD:1 + b * WPD],
                          in_=xs_d[:, a * WPD:b * WPD])
        bb = min(b, XR - 1)
        nc.sync.dma_start(out=xfold[64:128, 1 + a * WPD:1 + bb * WPD],
                          in_=xs_d[:, (a + 1) * WPD:(bb + 1) * WPD])
    nc.gpsimd.dma_start(out=w1s[:], in_=w1_d.rearrange("p (a b m) -> p a b m",
                                                     a=8, b=2))
    nc.gpsimd.dma_start(out=w1sx[:], in_=w1x_d.rearrange(
        "p (a b m) -> p a b m", a=6, b=2))
    nc.gpsimd.dma_start(out=w1s4[:], in_=w14_d.rearrange(
        "p (a b m) -> p a b m", a=2, b=2))
    for i, (lo, hi) in enumerate([(0, 128), (128, 256), (256, 384),
                                  (384, 512), (512, 576)]):
        nc.gpsimd.dma_start(out=sc1[i][:], in_=sb1_d[lo:hi, :])
    nc.gpsimd.dma_start(out=w2da[:], in_=w2a_d[:])
    nc.gpsimd.dma_start(out=w2fb[:], in_=w2f_d[:])
    nc.gpsimd.dma_start(out=w2gb[:], in_=w2g_d[:])
    nc.gpsimd.dma_start(out=b2a[:], in_=b2_d[0:128, :])
    nc.gpsimd.dma_start(out=b2b[:], in_=b2_d[128:192, :])
    nc.gpsimd.dma_start(
        out=tmps[:],
        in_=bass.AP(tensor=tmp_d, offset=0, ap=[[0, 128], [1, 1]]))

    # ---- border memsets (DVE; only the load-bearing zeros) ----
    # xfold lead/tail and Tall slab leads/tails are read only by discarded
    # junk output columns -- no clears needed there. T4 row 49 IS needed:
    # the zero-weight dummy plane reads it (0 x NaN = NaN).
    tall_b = Tall[:]
    t4_b = T4[:]

    def slabv(s, plo, phi, r0, r1, c0=0, c1=WPD):
        """[phi-plo, r1-r0, c1-c0] view of slab s rows r0..r1."""
        base = tall_b if s < 4 else t4_b
        off = (s % 4) * SL if s < 4 else 0
        ap0 = base.ap[0][0]
        return bass.AP(tensor=base.tensor,
                       offset=base.offset + plo * ap0 + off + 1 + r0 * WPD + c0,
                       ap=[[ap0, phi - plo], [WPD, r1 - r0], [1, c1 - c0]])

    for s in range(4):
        nc.vector.memset(slabv(s, 0, 128, 0, 1), 0.0)              # row 0
        nc.vector.memset(slabv(s, 0, 128, 0, TR, 0, 1), 0.0)       # col 0
        nc.vector.memset(slabv(s, 0, 128, 0, TR, 97, 98), 0.0)     # col 97
    nc.vector.memset(T4[:, 1 + 49 * WPD:SL], 0.0)   # row 49 + slack
    nc.vector.memset(slabv(4, 0, 64, 0, 1), 0.0)
    nc.vector.memset(slabv(4, 0, 64, 0, 49, 0, 1), 0.0)
    nc.vector.memset(slabv(4, 0, 64, 0, 49, 97, 98), 0.0)

    brv = brs.rearrange("p (r w) -> p r w", w=WPD)
    nc.vector.memset(brv[:, 0:1, :], 0.0)
    nc.vector.memset(brv[:, :, 0:1], 0.0)
    nc.vector.memset(brv[:, :, 97:98], 0.0)

    xpv = xpool.rearrange("p (r w) -> p r w", w=WPD)
    obv = ob.rearrange("p (r w) -> p r w", w=WPD)
    qkv = [t.rearrange("p (r w) -> p r w", w=W) for t in qk]
    w2av = w2da.rearrange("p (t m) -> p t m", t=9)
    w2fv = w2fb.rearrange("p (t m) -> p t m", t=3)
    w2gv = w2gb.rearrange("p (t m) -> p t m", t=3)

    xf_b = xfold[:]

    def c0_rhs(s0, d0, s1, d1, nr):
        o0 = 1 + s0 * WPD + d0 - 1
        o1 = 1 + s1 * WPD + d1 - 1
        return bass.AP(tensor=xf_b.tensor, offset=xf_b.offset + o0,
                       ap=[[xf_b.ap[0][0], 128], [o1 - o0, 2], [1, nr * WPD]])

    def c1_rhs(pairs_base, p0, p1, r0, nr):
        (s0, d0) = p0
        if p1 is None:
            s1, d1 = s0, d0 + 2
        else:
            s1, d1 = p1
        off = (lambda s, d: s * SL + 1 + r0 * WPD + d - 1)
        o0, o1 = off(s0, d0), off(s1, d1)
        return bass.AP(tensor=pairs_base.tensor, offset=pairs_base.offset + o0,
                       ap=[[pairs_base.ap[0][0], pairs_base.ap[0][1]],
                           [o1 - o0, 2], [1, nr * WPD]])

    # ---------------- conv0 (fp8 DoubleRow) ----------------
    # out-tile A (m 0:128): x1|x3 -> xpool (bf16); out-tile B (m 128:192):
    # x2 -> Tall slab2 lower partitions (fp8)
    blocksA = _row_blocks(YR)
    blocksB = _row_blocks(TR)
    with tc.tile_pool(name="ps_c0", bufs=8, space="PSUM") as pp0:
        def conv0_block(r0, nr, m0, mw):
            ps = pp0.tile([128, 5 * WPD], F32, tag="c0ps")
            specs = ((r0, 0, r0, 2), (r0, 1, r0 + 1, 1),
                     (r0 + 1, 0, r0 + 1, 2))
            for pi, (s0, d0, s1, d1) in enumerate(specs):
                nc.tensor.matmul(
                    ps[0:mw, 0:nr * WPD], lhsT=w0s[:, pi, :, m0:m0 + mw],
                    rhs=c0_rhs(s0, d0, s1, d1, nr),
                    start=(pi == 0), stop=(pi == 2),
                    perf_mode=mybir.MatmulPerfMode.DoubleRow)
            return ps

        for (r0, nr) in blocksA:
            ps = conv0_block(r0, nr, 0, 128)
            psv = ps.rearrange("p (r x) -> p r x", x=WPD)
            nc.scalar.activation(
                out=xpv[:, r0:r0 + nr, 1:97], in_=psv[:, 0:nr, 1:97],
                func=mybir.ActivationFunctionType.Relu,
                bias=sc0a[:, 1:2], scale=sc0a[:, 0:1])
        for (r0, nr) in blocksB:
            ps = conv0_block(r0, nr, 128, 64)
            psv = ps.rearrange("p (r x) -> p r x", x=WPD)
            # x2 -> ky1 slot = slab2 partitions 0..63 (t rows at tile rows)
            nc.scalar.activation(
                out=slabv(2, 0, 64, r0, r0 + nr, 1, 97),
                in_=psv[0:64, 0:nr, 1:97],
                func=mybir.ActivationFunctionType.Relu,
                bias=sc0b[:, 1:2], scale=sc0b[:, 0:1])

    # x2 ky-shifted slot copies (slab2 col pads are zero so full width)
    def x2_shift(a, b):
        nc.sync.dma_start(out=slabv(0, 64, 128, a + 1, b + 1),
                          in_=slabv(2, 0, 64, a, b))
        nc.sync.dma_start(out=slabv(3, 64, 128, a, b),
                          in_=slabv(2, 0, 64, a + 1, b + 1))

    # ---------------- pools + bilinear (DVE; final writes fp8 brs) --------
    cA = P.tile([128, 2], F32, tag="cA")
    nc.vector.memset(cA[0:64, 0:1], 0.75)
    nc.vector.memset(cA[0:64, 1:2], 0.25)
    nc.vector.memset(cA[64:128, 0:1], 0.1875)
    nc.vector.memset(cA[64:128, 1:2], 0.0625)
    cC = P.tile([128, 1], F32, tag="cC")
    nc.vector.memset(cC[0:64, :], 1.0)
    nc.vector.memset(cC[64:128, :], 0.25)

    pl_chunks = [(0, 5), (5, 10), (10, 15), (15, 20), (20, 25), (25, 26)]
    kv_o = 0   # next odd-row k (vint[2k+1], k<=24)
    kv_e = 0   # next even-row k (vint[2k+2], k<=23)
    hv = 0     # next t-row for the horizontal pass

    def hpass(a, b):
        if b <= a:
            return
        nc.vector.tensor_scalar(out=brv[:, 1 + a:1 + b, 1:2],
                                in0=vint[:, a:b, 0:1], scalar1=cC[:, 0:1],
                                scalar2=None, op0=mybir.AluOpType.mult)
        nc.vector.tensor_scalar(out=brv[:, 1 + a:1 + b, 96:97],
                                in0=vint[:, a:b, 47:48], scalar1=cC[:, 0:1],
                                scalar2=None, op0=mybir.AluOpType.mult)
        nc.vector.tensor_scalar(out=tscr[:, a:b, 0:47], in0=vint[:, a:b, 1:48],
                                scalar1=cA[:, 1:2], scalar2=None,
                                op0=mybir.AluOpType.mult)
        nc.vector.scalar_tensor_tensor(
            out=brv[:, 1 + a:1 + b, 2:96:2], in0=vint[:, a:b, 0:47],
            scalar=cA[:, 0:1], in1=tscr[:, a:b, 0:47],
            op0=mybir.AluOpType.mult, op1=mybir.AluOpType.add)
        nc.vector.tensor_scalar(out=tscr[:, a:b, 0:47], in0=vint[:, a:b, 1:48],
                                scalar1=cA[:, 0:1], scalar2=None,
                                op0=mybir.AluOpType.mult)
        nc.vector.scalar_tensor_tensor(
            out=brv[:, 1 + a:1 + b, 3:96:2], in0=vint[:, a:b, 0:47],
            scalar=cA[:, 1:2], in1=tscr[:, a:b, 0:47],
            op0=mybir.AluOpType.mult, op1=mybir.AluOpType.add)

    for (k0, k1) in pl_chunks:
        # horizontal pool pairs for y rows 2k0..2k1-1
        nc.vector.tensor_tensor(out=plh[0:64, 2 * k0:2 * k1, :],
                                in0=xpv[0:64, 2 * k0:2 * k1, 1:97:2],
                                in1=xpv[0:64, 2 * k0:2 * k1, 2:98:2],
                                op=mybir.AluOpType.max)
        nc.vector.tensor_tensor(out=plh[64:128, 2 * k0:2 * k1, :],
                                in0=xpv[64:128, 2 * k0:2 * k1, 1:97:2],
                                in1=xpv[64:128, 2 * k0:2 * k1, 2:98:2],
                                op=mybir.AluOpType.add)
        # vertical pool pairs -> pl rows k0..k1-1
        nc.vector.tensor_tensor(out=pl[0:64, k0:k1, :],
                                in0=plh[0:64, 2 * k0:2 * k1:2, :],
                                in1=plh[0:64, 2 * k0 + 1:2 * k1:2, :],
                                op=mybir.AluOpType.max)
        nc.vector.tensor_tensor(out=pl[64:128, k0:k1, :],
                                in0=plh[64:128, 2 * k0:2 * k1:2, :],
                                in1=plh[64:128, 2 * k0 + 1:2 * k1:2, :],
                                op=mybir.AluOpType.add)
        if k0 == 0:
            nc.vector.tensor_copy(out=vint[:, 0, :], in_=pl[:, 0, :])
        # vertical bilinear rows that only need pl rows < k1
        ke_o = min(k1 - 1, 25)
        if ke_o > kv_o:
            a, b = kv_o, ke_o
            nc.vector.tensor_scalar(out=tscr[:, a:b, :], in0=pl[:, a + 1:b + 1, :],
                                    scalar1=0.25, scalar2=None,
                                    op0=mybir.AluOpType.mult)
            nc.vector.scalar_tensor_tensor(
                out=vint[:, 2 * a + 1:2 * b:2, :], in0=pl[:, a:b, :],
                scalar=0.75, in1=tscr[:, a:b, :],
                op0=mybir.AluOpType.mult, op1=mybir.AluOpType.add)
            kv_o = ke_o
        ke_e = min(k1 - 1, 24)
        if ke_e > kv_e:
            a, b = kv_e, ke_e
            nc.vector.tensor_scalar(out=tscr[:, a:b, :], in0=pl[:, a + 1:b + 1, :],
                                    scalar1=0.75, scalar2=None,
                                    op0=mybir.AluOpType.mult)
            nc.vector.scalar_tensor_tensor(
                out=vint[:, 2 * a + 2:2 * b + 1:2, :], in0=pl[:, a:b, :],
                scalar=0.25, in1=tscr[:, a:b, :],
                op0=mybir.AluOpType.mult, op1=mybir.AluOpType.add)
            kv_e = ke_e
        # horizontal pass over fully-available vint rows
        avail = min(2 * kv_o + 1, 2 * kv_e + 2) if k1 < 26 else TR
        hpass(hv, avail)
        hv = avail

    # brs rows (fp8) -> T slab slots; row-chunked for conv1 pipelining.
    # (ky slot s stores t row rho at tile row rho+1-s.)
    x2s = iter(((0, 10), (10, 30), (30, 49)))
    for ci, (a, b) in enumerate(((0, 8), (8, 18), (18, 28), (28, 38), (38, 50))):
        nc.sync.dma_start(out=slabv(0, 0, 64, a, b), in_=brv[0:64, a:b, :])
        nc.sync.dma_start(out=slabv(1, 0, 64, a, b), in_=brv[64:128, a:b, :])
        nc.sync.dma_start(out=slabv(1, 64, 128, a, b),
                          in_=brv[0:64, a + 1:b + 1, :])
        nc.sync.dma_start(out=slabv(2, 64, 128, a, b),
                          in_=brv[64:128, a + 1:b + 1, :])
        bb = min(b, 49)
        nc.sync.dma_start(out=slabv(3, 0, 64, a, bb),
                          in_=brv[0:64, a + 2:bb + 2, :])
        nc.sync.dma_start(out=slabv(4, 0, 64, a, bb),
                          in_=brv[64:128, a + 2:bb + 2, :])
        if ci < 3:
            x2_shift(*next(x2s))

    # oa (= xpool reuse) borders for the depthwise reads; ob borders
    oav = xpv[:, 0:50, :]
    nc.vector.memset(oav[:, 0:1, :], 0.0)
    nc.vector.memset(oav[:, :, 0:1], 0.0)
    nc.vector.memset(oav[:, :, 97:98], 0.0)
    nc.vector.memset(obv[:, 0:1, :], 0.0)
    nc.vector.memset(obv[0:64, :, 1:2], 0.0)
    nc.vector.memset(obv[64:128, :, 0:1], 0.0)
    nc.vector.memset(obv[:, :, 97:98], 0.0)

    # ---------------- conv1 (fp8 DoubleRow) + attention prologue ----------
    qk_blocks = _row_blocks(SR)
    v_blocks = _row_blocks(QR)

    def conv1_block(ot, bi, r0, nr, evac, dense=False):
        mw = 64 if ot == 4 else 128
        m0 = 128 * ot
        ps = pp1.tile([128, 5 * WPD], F32, tag="c1ps")
        pairs, wt = ((PAIRS_C1X, w1sx) if dense else (PAIRS_C1, w1s))
        for pi, (p0, p1) in enumerate(pairs):
            nc.tensor.matmul(
                ps[0:mw, 0:nr * WPD], lhsT=wt[:, pi, :, m0:m0 + mw],
                rhs=c1_rhs(tall_b, p0, p1, r0, nr),
                start=(pi == 0), stop=False,
                perf_mode=mybir.MatmulPerfMode.DoubleRow)
        for qi, (p0, p1) in enumerate(PAIRS_T4):
            nc.tensor.matmul(
                ps[0:mw, 0:nr * WPD], lhsT=w1s4[:, qi, :, m0:m0 + mw],
                rhs=c1_rhs(t4_b, p0, p1, r0, nr),
                start=False, stop=(qi == 1),
                perf_mode=mybir.MatmulPerfMode.DoubleRow)
        psv = ps.rearrange("p (r x) -> p r x", x=WPD)
        evac(bi, r0, nr, psv, mw)

    def conv1_tile(ot, blocks, evac):
        for bi, (r0, nr) in enumerate(blocks):
            conv1_block(ot, bi, r0, nr, evac, dense=True)

    def evac_qk(ot):
        def f(bi, r0, nr, psv, mw):
            nc.scalar.activation(
                out=qkv[ot][:, r0:r0 + nr, :], in_=psv[:, 0:nr, 1:97],
                func=mybir.ActivationFunctionType.Relu,
                bias=sc1[ot][:, 1:2], scale=sc1[ot][:, 0:1])
            dump = ev.tile([128, 5 * W], BF16, tag="sqd")
            nc.vector.tensor_tensor(
                out=dump[:, 0:nr * W],
                in0=qk[ot][:, r0 * W:(r0 + nr) * W],
                in1=qk[ot][:, r0 * W:(r0 + nr) * W],
                op=mybir.AluOpType.mult)
            nc.vector.reduce_sum(out=sqac[ot][:, bi:bi + 1],
                                 in_=dump[:, 0:nr * W],
                                 axis=mybir.AxisListType.X)
        return f

    def evac_v(ot):
        off = 0 if ot == 3 else VS

        def f(bi, r0, nr, psv, mw):
            dst = vall[0:mw, off + r0 * W:off + (r0 + nr) * W]
            nc.scalar.activation(
                out=dst, in_=psv[0:mw, 0:nr, 1:97],
                func=mybir.ActivationFunctionType.Relu,
                bias=sc1[ot][:, 1:2], scale=sc1[ot][:, 0:1])
        return f

    with tc.tile_pool(name="ps_c1", bufs=3, space="PSUM") as pp1, \
         tc.tile_pool(name="ps_tr", bufs=2, space="PSUM") as ppt, \
         tc.tile_pool(name="ps_s", bufs=1, space="PSUM") as pps:

        # conv1 qk row-blocks interleaved across the 3 out-tiles, with S'
        # transpose groups issued as soon as their pixel chunks are covered.
        qk0r = qk[0].rearrange("p (c k) -> p c k", k=128)
        qk1r = qk[1].rearrange("p (c k) -> p c k", k=128)
        qk2r = qk[2].rearrange("p (c k) -> p c k", k=128)
        sp = pps.tile([128, 2 * C], F32, tag="sp")

        def s_group(g):
            tq = ppt.tile([128, 3 * C], BF16, tag="tq")
            tk = ppt.tile([128, 3 * C], BF16, tag="tk")
            tqv = tq.rearrange("p (i c) -> p i c", c=C)
            tkv = tk.rearrange("p (i c) -> p i c", c=C)
            for i in range(3):
                ci = 3 * g + i
                nc.tensor.transpose(tqv[:, i, 0:128], qk0r[:, ci, :], ident[:])
                nc.tensor.transpose(tqv[:, i, 128:192], qk1r[0:64, ci, :],
                                    ident[0:64, 0:64])
                nc.tensor.transpose(tkv[:, i, 0:64], qk1r[64:128, ci, :],
                                    ident[64:128, 64:128])
                nc.tensor.transpose(tkv[:, i, 64:192], qk2r[:, ci, :], ident[:])
            qtc = ev.tile([128, 3 * C], BF16, tag="qtc")
            ktc = ev.tile([128, 3 * C], BF16, tag="ktc")
            nc.scalar.copy(out=qtc[:], in_=tq[:])
            nc.vector.tensor_copy(out=ktc[:], in_=tk[:])
            qcv = qtc.rearrange("p (i c) -> p i c", c=C)
            kcv = ktc.rearrange("p (i c) -> p i c", c=C)
            for i in range(3):
                nc.tensor.matmul(sp[:, 0:C], lhsT=kcv[:, i, 0:128],
                                 rhs=qcv[:, i, :],
                                 start=(g == 0 and i == 0),
                                 stop=(g == NCH // 3 - 1 and i == 2))
                nc.tensor.matmul(sp[0:64, C:2 * C], lhsT=kcv[:, i, 128:192],
                                 rhs=qcv[:, i, :],
                                 start=(g == 0 and i == 0),
                                 stop=(g == NCH // 3 - 1 and i == 2))

        next_g = 0
        for bi, (r0, nr) in enumerate(qk_blocks):
            for ot in range(3):
                conv1_block(ot, bi, r0, nr, evac_qk(ot), dense=(bi >= 4))
            px_done = (r0 + nr) * W
            while next_g < NCH // 3 and (3 * next_g + 3) * 128 <= px_done:
                s_group(next_g)
                next_g += 1
        while next_g < NCH // 3:
            s_group(next_g)
            next_g += 1

        # deferred zero-clears (needed only from P@V onward): ptall plane1
        # upper rows are a zero K-pad for the P@V DR weights; vall plane1
        # upper rows are junk read by that zero plane -- must be non-NaN
        # (0 x NaN = NaN).
        nc.vector.memset(ptall[64:128, C:2 * C], 0.0)
        nc.vector.memset(vall[64:128, VS:2 * VS], 0.0)

        # sumsq totals + AllReduce staging
        for ti in range(3):
            nc.vector.reduce_sum(out=accs[ti][:], in_=sqac[ti][:],
                                 axis=mybir.AxisListType.X)
        nc.vector.tensor_copy(out=ssa[:, 192:193], in_=accs[0][:])
        nc.vector.tensor_copy(out=ssb[:, 192:193], in_=accs[1][0:64, :])
        nc.vector.tensor_copy(out=accb[:, 0:1], in_=accs[1][:])
        nc.vector.tensor_copy(out=accb[:, 1:2], in_=accs[2][:])
        nc.sync.dma_start(out=ssa[0:64, 193:194], in_=accb[64:128, 0:1])
        nc.sync.dma_start(out=ssa[64:128, 193:194], in_=accb[0:64, 1:2])
        nc.sync.dma_start(out=ssb[0:64, 193:194], in_=accb[64:128, 1:2])
        nc.scalar.copy(out=ssa[:, 0:192], in_=sp[:, 0:C])
        nc.scalar.copy(out=ssb[:, 0:192], in_=sp[0:64, C:2 * C])
        nc.sync.dma_start(out=cc2i[0:64, :], in_=ssa[0:64, :])
        nc.scalar.dma_start(out=cc2i[64:128, :], in_=ssa[64:128, :])
        nc.gpsimd.dma_start(out=cc2i[128:192, :], in_=ssb[:])
        nc.gpsimd.collective_compute(
            "AllReduce", mybir.AluOpType.add, replica_groups=GROUPS,
            ins=[cc2i[:]], outs=[cc2o[:]])
        # sumsq columns first: the norm chain needs only these
        nc.gpsimd.dma_start(out=sfa[:, 192:194], in_=cc2o[0:128, 192:194])
        nc.gpsimd.dma_start(out=sfb[:, 192:194], in_=cc2o[128:192, 192:194])
        nc.sync.dma_start(out=sfa[0:64, 0:192], in_=cc2o[0:64, 0:192])
        nc.scalar.dma_start(out=sfa[64:128, 0:192], in_=cc2o[64:128, 0:192])
        nc.sync.dma_start(out=sfb[:, 0:192], in_=cc2o[128:192, 0:192])

        # conv1 v tiles overlap the collective
        conv1_tile(3, v_blocks, evac_v(3))
        conv1_tile(4, v_blocks, evac_v(4))

    # ---------------- transpose-free softmax -> fp8 P^T planes ------------
    with tc.tile_pool(name="ps_sm", bufs=1, space="PSUM") as ppm, \
         tc.tile_pool(name="ps_pv", bufs=2, space="PSUM") as ppv, \
         tc.tile_pool(name="ps_dw", bufs=2, space="PSUM") as ppd:
        psq = ppm.tile([1, C], BF16, tag="psq")
        pm = ppm.tile([128, 392], F32, tag="pm")
        bcast = pm[:, 0:192]
        pcs = pm[0:1, 196:388]

        nc.tensor.transpose(psq[0:1, 0:128], sfa[:, 192:193], ident[:])
        nc.tensor.transpose(psq[0:1, 128:192], sfb[0:64, 192:193],
                            ident[0:64, 0:64])
        nc.vector.tensor_scalar(out=qs[:], in0=psq[0:1, :], scalar1=1e-24,
                                scalar2=None, op0=mybir.AluOpType.max)
        nc.vector.tensor_scalar(out=rska[:], in0=sfa[:, 193:194],
                                scalar1=1e-24, scalar2=None,
                                op0=mybir.AluOpType.max)
        nc.vector.tensor_scalar(out=rskb[:], in0=sfb[0:64, 193:194],
                                scalar1=1e-24, scalar2=None,
                                op0=mybir.AluOpType.max)
        for r in (qs, rska, rskb):
            nc.vector.reciprocal(out=r[:], in_=r[:])
            nc.scalar.activation(out=r[:], in_=r[:],
                                 func=mybir.ActivationFunctionType.Sqrt)
        nc.tensor.matmul(bcast, lhsT=ones1[:, :], rhs=qs[:],
                         start=True, stop=True)
        nc.vector.scalar_tensor_tensor(
            out=ea[:], in0=sfa[:, 0:192], scalar=tmps[:, 0:1], in1=bcast,
            op0=mybir.AluOpType.mult, op1=mybir.AluOpType.mult)
        nc.vector.scalar_tensor_tensor(
            out=eb[:], in0=sfb[0:64, 0:192], scalar=tmps[0:64, 0:1],
            in1=bcast[0:64, :],
            op0=mybir.AluOpType.mult, op1=mybir.AluOpType.mult)
        # P^T planes (unnormalized): plane0 = keys 0:128, plane1 = keys 128:192
        nc.scalar.activation(out=ptall[:, 0:C], in_=ea[:],
                             func=mybir.ActivationFunctionType.Exp,
                             bias=0.0, scale=rska[:, 0:1])
        nc.scalar.activation(out=ptall[0:64, C:2 * C], in_=eb[:],
                             func=mybir.ActivationFunctionType.Exp,
                             bias=0.0, scale=rskb[:, 0:1])
        # out = P^T.T @ v (fp8 DoubleRow over the two key planes), interleaved
        # with depthwise-A blocks so dw matmuls hide the P@V evac latency.
        ptv = ptall.rearrange("p (two q) -> p two q", two=2)
        va_b = vall[:]
        yv = yout.rearrange("c (r w) -> c r w", w=W)

        def pv_rhs(r0, nr):
            return bass.AP(tensor=va_b.tensor, offset=va_b.offset + r0 * W,
                           ap=[[va_b.ap[0][0], 128], [VS, 2], [1, nr * W]])

        def pv_block(r0, nr):
            po = ppv.tile([128, 5 * W], F32, tag="po")
            po2 = ppv.tile([64, 5 * W], F32, tag="po2")
            nc.tensor.matmul(po[:, 0:nr * W], lhsT=ptv[:, :, 0:128],
                             rhs=pv_rhs(r0, nr), start=True, stop=True,
                             perf_mode=mybir.MatmulPerfMode.DoubleRow)
            nc.tensor.matmul(po2[0:64, 0:nr * W], lhsT=ptv[:, :, 128:192],
                             rhs=pv_rhs(r0, nr), start=True, stop=True,
                             perf_mode=mybir.MatmulPerfMode.DoubleRow)
            pov = po.rearrange("p (r w) -> p r w", w=W)
            po2v = po2.rearrange("p (r w) -> p r w", w=W)
            nc.scalar.copy(out=oav[:, r0 + 1:r0 + 1 + nr, 1:97],
                           in_=pov[:, 0:nr, :])
            nc.vector.tensor_copy(out=obv[0:64, r0 + 1:r0 + 1 + nr, 2:98],
                                  in_=po2v[0:64, 0:nr, :])

        def dwa_block(r0, nr):
            ps = ppd.tile([128, 5, W], F32, tag="dwps")
            for t in range(9):
                dy, dx = t // 3 - 1, t % 3 - 1
                nc.tensor.matmul(
                    ps[:, 0:nr, :],
                    lhsT=w2av[:, t, :],
                    rhs=oav[:, r0 + 1 + dy:r0 + 1 + dy + nr, 1 + dx:97 + dx],
                    start=(t == 0), stop=(t == 8))
            fo = ev.tile([128, 5, W], F32, tag="fo")
            nc.scalar.activation(out=fo[:, 0:nr, :], in_=ps[:, 0:nr, :],
                                 func=mybir.ActivationFunctionType.Identity,
                                 bias=b2a[:, 0:1], scale=rsA[:, 0:1])
            nc.sync.dma_start(out=yv[0:128, r0:r0 + nr, :],
                              in_=fo[:, 0:nr, :])

        dw_blocks = _row_blocks(OR_)
        pv_block(*v_blocks[0])
        pv_block(*v_blocks[1])
        # softmax denominator: colsum over keys via ones-matmul
        nc.tensor.matmul(pcs, lhsT=ones8[:, 0:1],
                         rhs=ptall[:, 0:C], start=True, stop=False)
        nc.tensor.matmul(pcs, lhsT=ones8[:, 0:1],
                         rhs=ptall[:, C:2 * C], start=False, stop=True)
        nc.vector.reciprocal(out=isr[:], in_=pcs)
        prt = pm[:, 192:194]
        nc.tensor.transpose(prt[0:128, 0:1], isr[0:1, 0:128],
                            identf[0:1, 0:1])
        nc.tensor.transpose(prt[0:64, 1:2], isr[0:1, 128:192],
                            identf[0:1, 0:1])
        nc.vector.tensor_copy(out=rsA[:], in_=prt[0:128, 0:1])
        nc.vector.tensor_copy(out=rsB[:], in_=prt[0:64, 1:2])

        for bi in range(2, len(v_blocks)):
            dwa_block(*dw_blocks[bi - 2])
            pv_block(*v_blocks[bi])
            if bi == 5:
                # replicate ob rows 1..25 (needs P@V-B evacs through block 4)
                nc.sync.dma_start(out=obv[64:128, 1:26, 1:97],
                                  in_=obv[0:64, 1:26, 2:98])
        nc.sync.dma_start(out=obv[64:128, 26:50, 1:97],
                          in_=obv[0:64, 26:50, 2:98])
        dwa_block(*dw_blocks[8])
        dwa_block(*dw_blocks[9])
        for (r0, nr) in dw_blocks:
            ps = ppd.tile([128, 5, W], F32, tag="dwps")
            for dy in range(3):
                nc.tensor.matmul(
                    ps[0:64, 0:nr, :], lhsT=w2fv[:, dy, :],
                    rhs=obv[:, r0 + dy:r0 + dy + nr, 1:97],
                    start=(dy == 0), stop=False)
                nc.tensor.matmul(
                    ps[0:64, 0:nr, :], lhsT=w2gv[64:128, dy, :],
                    rhs=obv[64:128, r0 + dy:r0 + dy + nr, 2:98],
                    start=False, stop=(dy == 2))
            fo = ev.tile([128, 5, W], F32, tag="fo")
            nc.scalar.activation(out=fo[0:64, 0:nr, :], in_=ps[0:64, 0:nr, :],
                                 func=mybir.ActivationFunctionType.Identity,
                                 bias=b2b[:, 0:1], scale=rsB[:, 0:1])
            nc.sync.dma_start(out=yv[128:192, r0:r0 + nr, :],
                              in_=fo[0:64, 0:nr, :])
    ctx.close()


# ---------------- host side ----------------
_NC_CACHE = None


def _get_nc():
    global _NC_CACHE
    if _NC_CACHE is None:
        _NC_CACHE = build_nc()
    return _NC_CACHE


def _pack_weights(inp, flip):
    bf = ml_dtypes.bfloat16
    w0 = inp["w0"][:, :, ::-1, :] if flip else inp["w0"]
    w1 = inp["w1"][:, :, ::-1, :] if flip else inp["w1"]
    w2 = inp["w2"][:, :, ::-1, :] if flip else inp["w2"]
    w0 = np.asarray(w0, np.float32)
    w1 = np.asarray(w1, np.float32)
    w2 = np.asarray(w2, np.float32)

    # conv0: out-channel order [x1(0:64), x3(128:192), x2(64:128)]
    cho = np.concatenate([np.arange(0, 64), np.arange(128, 192),
                          np.arange(64, 128)])
    wt = w0[cho].transpose(1, 2, 3, 0)        # [64c, 3ky, 3kx, 192m]
    w0p = np.zeros((128, 3, 2, C), np.float32)
    w0p[0:64, 0, 0] = wt[:, 0, 0]
    w0p[0:64, 0, 1] = wt[:, 0, 2]
    w0p[0:64, 1, 0] = wt[:, 0, 1]
    w0p[64:128, 0, 0] = wt[:, 1, 0]
    w0p[64:128, 0, 1] = wt[:, 1, 2]
    w0p[64:128, 1, 0] = wt[:, 1, 1]
    w0p[64:128, 1, 1] = wt[:, 2, 1]
    w0p[64:128, 2, 0] = wt[:, 2, 0]
    w0p[64:128, 2, 1] = wt[:, 2, 2]
    s0 = inp["g0"] / np.sqrt(inp["v0"] + BN_EPS)
    t0 = inp["be0"] + (inp["b0"] - inp["m0"]) * s0
    sb0 = np.stack([s0 / W0S, t0], axis=1).astype(np.float32)[cho]

    # conv1 DoubleRow packs.  Slab k-partition -> (ky, c) maps:
    def slab_map(s):
        k = np.arange(128 if s < 4 else 64)
        if s == 0:
            return np.zeros_like(k), k
        if s == 1:
            return np.where(k < 64, 0, 1), np.where(k < 64, 128 + k, k - 64)
        if s == 2:
            return np.ones_like(k), 64 + k
        if s == 3:
            return np.full_like(k, 2), k
        return np.full_like(k, 2), 128 + k

    wt1 = w1.transpose(1, 2, 3, 0)            # [192c, 3ky, 3kx, 576m]
    w1p = np.zeros((128, 8, 2, C3), np.float32)
    for pi, (p0, p1) in enumerate(PAIRS_C1):
        for pl, spec in enumerate((p0, p1)):
            if spec is None:
                continue
            s, dx = spec
            ky, cc = slab_map(s)
            w1p[:, pi, pl] = wt1[cc, ky, dx]
    w1px = np.zeros((128, 6, 2, C3), np.float32)
    for pi, (p0, p1) in enumerate(PAIRS_C1X):
        for pl, (s, dx) in enumerate((p0, p1)):
            ky, cc = slab_map(s)
            w1px[:, pi, pl] = wt1[cc, ky, dx]
    w1p4 = np.zeros((64, 2, 2, C3), np.float32)
    ky4, cc4 = slab_map(4)
    w1p4[:, 0, 0] = wt1[cc4, ky4, 0]
    w1p4[:, 0, 1] = wt1[cc4, ky4, 2]
    w1p4[:, 1, 0] = wt1[cc4, ky4, 1]
    s1 = inp["g1"] / np.sqrt(inp["v1"] + BN_EPS)
    t1 = inp["be1"] + (inp["b1"] - inp["m1"]) * s1
    sb1 = np.stack([s1 / W1S, t1], axis=1).astype(np.float32)

    w2da = np.zeros((128, 9, 128), np.float32)
    w2fb = np.zeros((128, 3, 64), np.float32)
    w2gb = np.zeros((128, 3, 64), np.float32)
    r64, r128 = np.arange(64), np.arange(128)
    for t in range(9):
        d = w2[:, 0, t // 3, t % 3]
        w2da[r128, t, r128] = d[0:128]
    for dy in range(3):
        db = w2[128:192, 0, dy, :]
        w2fb[r64, dy, r64] = db[:, 0]
        w2fb[64 + r64, dy, r64] = db[:, 1]
        w2gb[64 + r64, dy, r64] = db[:, 2]

    out = {
        "w0p": np.ascontiguousarray(
            (w0p * W0S).reshape(128, 3 * 2 * C)).astype(f8np),
        "sb0p": sb0,
        "w1p": np.ascontiguousarray(
            (w1p * W1S).reshape(128, 8 * 2 * C3)).astype(f8np),
        "w1px": np.ascontiguousarray(
            (w1px * W1S).reshape(128, 6 * 2 * C3)).astype(f8np),
        "w1p4": np.ascontiguousarray(
            (w1p4 * W1S).reshape(64, 2 * 2 * C3)).astype(f8np),
        "sb1": sb1,
        "w2da": np.ascontiguousarray(w2da.reshape(128, 9 * 128)).astype(bf),
        "w2fb": np.ascontiguousarray(w2fb.reshape(128, 3 * 64)).astype(bf),
        "w2gb": np.ascontiguousarray(w2gb.reshape(128, 3 * 64)).astype(bf),
        "b2v": np.asarray(inp["b2"], np.float32).reshape(C, 1),
    }
    return out


def kernel(**inputs):
    inputs = {k: np.asarray(v) for k, v in inputs.items()}
    x = inputs["x"]
    B = x.shape[0]
    packs = [_pack_weights(inputs, flip) for flip in (False, True)]
    tempv = np.asarray(inputs["temp"], np.float32).reshape(1, 1)

    in_maps = []
    for core in range(8):
        s, h = core // 2, core % 2
        xi = np.asarray(x[s], np.float32)
        if h:
            xi = xi[:, ::-1, :]
        slab = np.zeros((64, XR, WPD), np.float32)
        slab[:, 1:54, 1:97] = xi[:, 0:53, :]
        m = dict(packs[h])
        m["xs"] = np.ascontiguousarray(slab.reshape(64, XR * WPD)).astype(f8np)
        m["tempv"] = tempv
        m["zpad8"] = np.zeros((1, 512), f8np)
        m["zpad16"] = np.zeros((1, 512), ml_dtypes.bfloat16)
        in_maps.append(m)

    nc = _get_nc()
    res = run_bass_kernel_spmd(nc, in_maps, list(range(8)))
    out = np.zeros((B, C, 96, 96), np.float32)
    for core in range(8):
        s, h = core // 2, core % 2
        yc = res.results[core]["yout"].reshape(C, OR_, W)
        if h:
            out[s, :, 48:96] = yc[:, ::-1, :]
        else:
            out[s, :, 0:48] = yc
    return out


# revision 49
# speedup vs baseline: 1.0348x; 1.0009x over previous
"""Bass/Trainium2 kernel for nn_CSEM sparse_attention problem (v3).

Sharding: 8 cores = 4 samples x 2 spatial halves (bottom half vertically
flipped on host so all cores run an identical NEFF).

v3 vs v2: fp8e4 DoubleRow matmuls for conv0 / conv1 / P@V (one DR matmul
accumulates two (weights-plane, ifmap-plane) K-tiles at 0.5 cycles/row).
Activations and weights for those convs are fp8; conv taps are addressed as
column/row offsets into padded flat slabs so tap pairs become stride-`d`
plane pairs of a single AP. Softmax is transpose-free: S'^T stays key-major,
q-norms are broadcast along partitions, exp goes straight to fp8 P^T planes,
and the softmax denominator (from an ones-matmul colsum) is folded into the
depthwise-conv evacuation scale. Depthwise conv + q/k transposes + S' stay
bf16 for accuracy.
"""

import numpy as np
import ml_dtypes

import concourse.bass as bass
import concourse.mybir as mybir
import concourse.tile as tile
from concourse.bass_utils import run_bass_kernel_spmd
from concourse.masks import make_identity

F8 = mybir.dt.float8e4
BF16 = mybir.dt.bfloat16
F32 = mybir.dt.float32
BN_EPS = 1e-5
f8np = ml_dtypes.float8_e4m3

CIN, C, C3 = 64, 192, 576
W, WPD = 96, 98
XR = 54          # x slab rows (1 zero pad + 53 data)
YR = 52          # y rows computed locally (0..51)
TR = 50          # t rows 0..49
QR = 49          # conv1 v rows 0..48
SR = 48          # rows feeding S partial
OR_ = 48         # final output rows per core
NCH = SR * W // 128   # 36 transpose chunks
VS = QR * W      # v plane stride (4704)
SL = 4908        # Tall slab stride (1 lead + 50*98 data + slack)
XLEN = 1 + XR * WPD + 8
W0S, W1S = 32.0, 64.0
GROUPS = [[0, 1], [2, 3], [4, 5], [6, 7]]

# conv1 DoubleRow pairs as ((slab, dx), (slab, dx)); None = zero-weight dummy
# plane (rhs offset +2 -- a dim1 stride equal to the element stride wedges the
# device). Pairs stay within one slab so each block's read footprint is
# row-local and conv1 pipelines with the bilinear-branch writes.
PAIRS_C1 = [((0, 0), (0, 2)), ((0, 1), None), ((1, 0), (1, 2)),
            ((1, 1), None), ((2, 0), (2, 2)), ((2, 1), None),
            ((3, 0), (3, 2)), ((3, 1), None)]
PAIRS_T4 = [((0, 0), (0, 2)), ((0, 1), None)]   # slab idx relative to T4
# denser cross-slab pairing (no dummy planes): used for row-blocks issued
# after the bilinear branch has fully landed, where the wide byte-interval
# footprint of a cross-slab pair cannot stall the pipeline.
PAIRS_C1X = [((0, 0), (0, 2)), ((1, 0), (1, 2)), ((2, 0), (2, 2)),
             ((3, 0), (3, 2)), ((0, 1), (1, 1)), ((2, 1), (3, 1))]


def _split_waits(nc, limit=1):
    """This walrus build rejects instructions carrying more than one sem-wait
    command. Spread extra waits onto same-engine ENGINE_NOPs inserted just
    before the offending instruction (semantically identical: the engine
    blocks on each wait in program order)."""
    ctr = [0]
    for f in nc.m.functions:
        for blk in f.blocks:
            il = blk.instructions
            new = []
            for inst in il:
                si = inst.sync_info
                waits = list(si.on_wait) if (si and si.on_wait) else []
                if len(waits) > limit:
                    for w in waits[:-limit]:
                        ctr[0] += 1
                        nop = mybir.InstNoOp(name=f"WNOP-{ctr[0]}")
                        nop.engine = inst.engine
                        nop.sync_info = mybir.SyncInfo(on_wait=[w], on_update=[])
                        new.append(nop)
                    si.on_wait = waits[-limit:]
                new.append(inst)
            il[:] = new


def _row_blocks(nrows, per=5):
    out, r = [], 0
    while r < nrows:
        n = min(per, nrows - r)
        out.append((r, n))
        r += n
    return out


def build_nc():
    nc = bass.Bass()

    xs_d = nc.declare_dram_parameter("xs", [64, XR * WPD], F8, isOutput=False)
    w0_d = nc.declare_dram_parameter("w0p", [128, 3 * 2 * C], F8, isOutput=False)
    sb0_d = nc.declare_dram_parameter("sb0p", [C, 2], F32, isOutput=False)
    w1_d = nc.declare_dram_parameter("w1p", [128, 8 * 2 * C3], F8, isOutput=False)
    w1x_d = nc.declare_dram_parameter("w1px", [128, 6 * 2 * C3], F8,
                                      isOutput=False)
    w14_d = nc.declare_dram_parameter("w1p4", [64, 2 * 2 * C3], F8, isOutput=False)
    sb1_d = nc.declare_dram_parameter("sb1", [C3, 2], F32, isOutput=False)
    w2a_d = nc.declare_dram_parameter("w2da", [128, 9 * 128], BF16, isOutput=False)
    w2t_d = nc.declare_dram_parameter("w2ta", [128, 9], F32, isOutput=False)
    w2f_d = nc.declare_dram_parameter("w2fb", [128, 3 * 64], BF16, isOutput=False)
    w2g_d = nc.declare_dram_parameter("w2gb", [128, 3 * 64], BF16, isOutput=False)
    b2_d = nc.declare_dram_parameter("b2v", [C, 1], F32, isOutput=False)
    tmp_d = nc.declare_dram_parameter("tempv", [1, 1], F32, isOutput=False)
    z8_d = nc.declare_dram_parameter("zpad8", [1, 512], F8, isOutput=False)
    z16_d = nc.declare_dram_parameter("zpad16", [1, 512], BF16, isOutput=False)
    yout = nc.declare_dram_parameter("yout", [C, OR_ * W], F32, isOutput=True)

    cc2i = nc.dram_tensor("cc2i", [C, 194], BF16)
    cc2o = nc.dram_tensor("cc2o", [C, 194], BF16)

    with tile.TileContext(nc) as tc:
        _body(nc, tc, xs_d, w0_d, sb0_d, w1_d, w1x_d, w14_d, sb1_d, w2a_d,
              w2t_d, w2f_d, w2g_d, b2_d, tmp_d, z8_d, z16_d, yout, cc2i, cc2o)
    _split_waits(nc)
    return nc


def _body(nc, tc, xs_d, w0_d, sb0_d, w1_d, w1x_d, w14_d, sb1_d, w2a_d,
          w2t_d, w2f_d, w2g_d, b2_d, tmp_d, z8_d, z16_d, yout, cc2i, cc2o):
    import contextlib
    ctx = contextlib.ExitStack()
    P = ctx.enter_context(tc.tile_pool(name="persist", bufs=1))
    ev = ctx.enter_context(tc.tile_pool(name="evac", bufs=3))

    # ---- persistent SBUF ----
    xfold = P.tile([128, XLEN], F8, tag="xfold")
    w0s = P.tile([128, 3, 2, C], F8, tag="w0s")
    w1s = P.tile([128, 8, 2, C3], F8, tag="w1s")
    w1sx = P.tile([128, 6, 2, C3], F8, tag="w1sx")
    w1s4 = P.tile([64, 2, 2, C3], F8, tag="w1s4")
    sc0a = P.tile([128, 2], F32, tag="sc0a")
    sc0b = P.tile([64, 2], F32, tag="sc0b")
    scp = [128, 128, 128, 128, 64]
    sc1 = [P.tile([scp[i], 2], F32, tag=f"sc1_{i}", name=f"sc1_{i}")
           for i in range(5)]
    w2da = P.tile([128, 9 * 128], BF16, tag="w2da")
    w2ta = P.tile([128, 9], F32, tag="w2ta")
    dwacc = [P.tile([128, 5, W], F32, tag=f"dwacc{i}", name=f"dwacc{i}")
             for i in range(2)]
    w2fb = P.tile([128, 3 * 64], BF16, tag="w2fb")
    w2gb = P.tile([128, 3 * 64], BF16, tag="w2gb")  # data at partitions 64..
    b2a = P.tile([128, 1], F32, tag="b2a")
    b2b = P.tile([64, 1], F32, tag="b2b")
    tmps = P.tile([128, 1], F32, tag="tmps")

    Tall = P.tile([128, 4 * SL], F8, tag="Tall")
    T4 = P.tile([64, SL], F8, tag="T4")
    xpool = P.tile([128, YR * WPD], BF16, tag="xpool")   # later reused as oa
    plh = P.tile([128, YR, 48], BF16, tag="plh")
    pl = P.tile([128, 26, 48], BF16, tag="pl")
    vint = P.tile([128, TR, 48], BF16, tag="vint")
    tscr = P.tile([128, TR, 48], BF16, tag="tscr")
    brs = P.tile([128, 51 * WPD], F8, tag="brs")
    ob = P.tile([128, TR * WPD], BF16, tag="ob")
    qk = [P.tile([128, SR * W], BF16, tag=f"qk{i}", name=f"qk{i}")
          for i in range(3)]
    vall = P.tile([128, 2 * VS], F8, tag="vall")
    sqac = [P.tile([128, 10], F32, tag=f"sqac{i}", name=f"sqac{i}")
            for i in range(3)]
    accs = [P.tile([128, 1], F32, tag=f"accs{i}", name=f"accs{i}")
            for i in range(3)]
    ssa = P.tile([128, 194], BF16, tag="ssa")
    ssb = P.tile([64, 194], BF16, tag="ssb")
    sfa = P.tile([128, 194], BF16, tag="sfa")
    sfb = P.tile([64, 194], BF16, tag="sfb")
    accb = P.tile([128, 2], BF16, tag="accb")
    rska = P.tile([128, 1], F32, tag="rska")
    rskb = P.tile([64, 1], F32, tag="rskb")
    qs = P.tile([1, C], F32, tag="qs")
    rqbc = P.tile([128, C], F32, tag="rqbc")
    ea = P.tile([128, C], F32, tag="ea")
    eb = P.tile([64, C], F32, tag="eb")
    ptall = P.tile([128, 2 * C], F8, tag="ptall")
    isr = P.tile([1, C], F32, tag="isr")
    rsA = P.tile([128, 1], F32, tag="rsA")
    rsB = P.tile([64, 1], F32, tag="rsB")
    ones8 = P.tile([128, 1], F8, tag="ones8")
    ones1 = P.tile([1, 128], F32, tag="ones1")

    ident = P.tile([128, 128], BF16, tag="ident")
    make_identity(nc, ident[:])
    identf = P.tile([128, 128], F32, tag="identf")
    make_identity(nc, identf[:])
    nc.vector.memset(ones8[:], 1.0)
    nc.vector.memset(ones1[:], 1.0)

    # ---- input DMAs (x chunked so conv0 starts early) ----
    nc.gpsimd.dma_start(out=w0s[:], in_=w0_d.rearrange("p (a b m) -> p a b m",
                                                       a=3, b=2))
    nc.gpsimd.dma_start(out=sc0a[:], in_=sb0_d[0:128, :])
    nc.gpsimd.dma_start(out=sc0b[:], in_=sb0_d[128:192, :])
    for (a, b) in ((0, 8), (8, 32), (32, XR)):
        nc.sync.dma_start(out=xfold[0:64, 1 + a * WPD:1 + b * WPD],
                          in_=xs_d[:, a * WPD:b * WPD])
        bb = min(b, XR - 1)
        nc.sync.dma_start(out=xfold[64:128, 1 + a * WPD:1 + bb * WPD],
                          in_=xs_d[:, (a + 1) * WPD:(bb + 1) * WPD])
    nc.gpsimd.dma_start(out=w1s[:], in_=w1_d.rearrange("p (a b m) -> p a b m",
                                                     a=8, b=2))
    nc.gpsimd.dma_start(out=w1sx[:], in_=w1x_d.rearrange(
        "p (a b m) -> p a b m", a=6, b=2))
    nc.gpsimd.dma_start(out=w1s4[:], in_=w14_d.rearrange(
        "p (a b m) -> p a b m", a=2, b=2))
    for i, (lo, hi) in enumerate([(0, 128), (128, 256), (256, 384),
                                  (384, 512), (512, 576)]):
        nc.gpsimd.dma_start(out=sc1[i][:], in_=sb1_d[lo:hi, :])
    nc.gpsimd.dma_start(out=w2da[:], in_=w2a_d[:])
    nc.gpsimd.dma_start(out=w2ta[:], in_=w2t_d[:])
    nc.gpsimd.dma_start(out=w2fb[:], in_=w2f_d[:])
    nc.gpsimd.dma_start(out=w2gb[:], in_=w2g_d[:])
    nc.gpsimd.dma_start(out=b2a[:], in_=b2_d[0:128, :])
    nc.gpsimd.dma_start(out=b2b[:], in_=b2_d[128:192, :])
    nc.gpsimd.dma_start(
        out=tmps[:],
        in_=bass.AP(tensor=tmp_d, offset=0, ap=[[0, 128], [1, 1]]))

    # ---- border memsets (DVE; only the load-bearing zeros) ----
    # xfold lead/tail and Tall slab leads/tails are read only by discarded
    # junk output columns -- no clears needed there. T4 row 49 IS needed:
    # the zero-weight dummy plane reads it (0 x NaN = NaN).
    tall_b = Tall[:]
    t4_b = T4[:]

    def slabv(s, plo, phi, r0, r1, c0=0, c1=WPD):
        """[phi-plo, r1-r0, c1-c0] view of slab s rows r0..r1."""
        base = tall_b if s < 4 else t4_b
        off = (s % 4) * SL if s < 4 else 0
        ap0 = base.ap[0][0]
        return bass.AP(tensor=base.tensor,
                       offset=base.offset + plo * ap0 + off + 1 + r0 * WPD + c0,
                       ap=[[ap0, phi - plo], [WPD, r1 - r0], [1, c1 - c0]])

    for s in range(4):
        nc.vector.memset(slabv(s, 0, 128, 0, 1), 0.0)              # row 0
        nc.vector.memset(slabv(s, 0, 128, 0, TR, 0, 1), 0.0)       # col 0
        nc.vector.memset(slabv(s, 0, 128, 0, TR, 97, 98), 0.0)     # col 97
    nc.vector.memset(T4[:, 1 + 49 * WPD:SL], 0.0)   # row 49 + slack
    nc.vector.memset(slabv(4, 0, 64, 0, 1), 0.0)
    nc.vector.memset(slabv(4, 0, 64, 0, 49, 0, 1), 0.0)
    nc.vector.memset(slabv(4, 0, 64, 0, 49, 97, 98), 0.0)

    brv = brs.rearrange("p (r w) -> p r w", w=WPD)
    nc.vector.memset(brv[:, 0:1, :], 0.0)
    nc.vector.memset(brv[:, :, 0:1], 0.0)
    nc.vector.memset(brv[:, :, 97:98], 0.0)

    xpv = xpool.rearrange("p (r w) -> p r w", w=WPD)
    obv = ob.rearrange("p (r w) -> p r w", w=WPD)
    qkv = [t.rearrange("p (r w) -> p r w", w=W) for t in qk]
    w2av = w2da.rearrange("p (t m) -> p t m", t=9)
    w2fv = w2fb.rearrange("p (t m) -> p t m", t=3)
    w2gv = w2gb.rearrange("p (t m) -> p t m", t=3)

    xf_b = xfold[:]

    def c0_rhs(s0, d0, s1, d1, nr):
        o0 = 1 + s0 * WPD + d0 - 1
        o1 = 1 + s1 * WPD + d1 - 1
        return bass.AP(tensor=xf_b.tensor, offset=xf_b.offset + o0,
                       ap=[[xf_b.ap[0][0], 128], [o1 - o0, 2], [1, nr * WPD]])

    def c1_rhs(pairs_base, p0, p1, r0, nr):
        (s0, d0) = p0
        if p1 is None:
            s1, d1 = s0, d0 + 2
        else:
            s1, d1 = p1
        off = (lambda s, d: s * SL + 1 + r0 * WPD + d - 1)
        o0, o1 = off(s0, d0), off(s1, d1)
        return bass.AP(tensor=pairs_base.tensor, offset=pairs_base.offset + o0,
                       ap=[[pairs_base.ap[0][0], pairs_base.ap[0][1]],
                           [o1 - o0, 2], [1, nr * WPD]])

    # ---------------- conv0 (fp8 DoubleRow) ----------------
    # out-tile A (m 0:128): x1|x3 -> xpool (bf16); out-tile B (m 128:192):
    # x2 -> Tall slab2 lower partitions (fp8)
    blocksA = _row_blocks(YR)
    blocksB = _row_blocks(TR)
    with tc.tile_pool(name="ps_c0", bufs=8, space="PSUM") as pp0:
        def conv0_block(r0, nr, m0, mw):
            ps = pp0.tile([128, 5 * WPD], F32, tag="c0ps")
            specs = ((r0, 0, r0, 2), (r0, 1, r0 + 1, 1),
                     (r0 + 1, 0, r0 + 1, 2))
            for pi, (s0, d0, s1, d1) in enumerate(specs):
                nc.tensor.matmul(
                    ps[0:mw, 0:nr * WPD], lhsT=w0s[:, pi, :, m0:m0 + mw],
                    rhs=c0_rhs(s0, d0, s1, d1, nr),
                    start=(pi == 0), stop=(pi == 2),
                    perf_mode=mybir.MatmulPerfMode.DoubleRow)
            return ps

        for (r0, nr) in blocksA:
            ps = conv0_block(r0, nr, 0, 128)
            psv = ps.rearrange("p (r x) -> p r x", x=WPD)
            nc.scalar.activation(
                out=xpv[:, r0:r0 + nr, 1:97], in_=psv[:, 0:nr, 1:97],
                func=mybir.ActivationFunctionType.Relu,
                bias=sc0a[:, 1:2], scale=sc0a[:, 0:1])
        for (r0, nr) in blocksB:
            ps = conv0_block(r0, nr, 128, 64)
            psv = ps.rearrange("p (r x) -> p r x", x=WPD)
            # x2 -> ky1 slot = slab2 partitions 0..63 (t rows at tile rows)
            nc.scalar.activation(
                out=slabv(2, 0, 64, r0, r0 + nr, 1, 97),
                in_=psv[0:64, 0:nr, 1:97],
                func=mybir.ActivationFunctionType.Relu,
                bias=sc0b[:, 1:2], scale=sc0b[:, 0:1])

    # x2 ky-shifted slot copies (slab2 col pads are zero so full width)
    def x2_shift(a, b):
        nc.sync.dma_start(out=slabv(0, 64, 128, a + 1, b + 1),
                          in_=slabv(2, 0, 64, a, b))
        nc.sync.dma_start(out=slabv(3, 64, 128, a, b),
                          in_=slabv(2, 0, 64, a + 1, b + 1))

    # ---------------- pools + bilinear (DVE; final writes fp8 brs) --------
    cA = P.tile([128, 2], F32, tag="cA")
    nc.vector.memset(cA[0:64, 0:1], 0.75)
    nc.vector.memset(cA[0:64, 1:2], 0.25)
    nc.vector.memset(cA[64:128, 0:1], 0.1875)
    nc.vector.memset(cA[64:128, 1:2], 0.0625)
    cC = P.tile([128, 1], F32, tag="cC")
    nc.vector.memset(cC[0:64, :], 1.0)
    nc.vector.memset(cC[64:128, :], 0.25)

    pl_chunks = [(0, 5), (5, 10), (10, 15), (15, 20), (20, 25), (25, 26)]
    kv_o = 0   # next odd-row k (vint[2k+1], k<=24)
    kv_e = 0   # next even-row k (vint[2k+2], k<=23)
    hv = 0     # next t-row for the horizontal pass

    def hpass(a, b):
        if b <= a:
            return
        nc.vector.tensor_scalar(out=brv[:, 1 + a:1 + b, 1:2],
                                in0=vint[:, a:b, 0:1], scalar1=cC[:, 0:1],
                                scalar2=None, op0=mybir.AluOpType.mult)
        nc.vector.tensor_scalar(out=brv[:, 1 + a:1 + b, 96:97],
                                in0=vint[:, a:b, 47:48], scalar1=cC[:, 0:1],
                                scalar2=None, op0=mybir.AluOpType.mult)
        nc.vector.tensor_scalar(out=tscr[:, a:b, 0:47], in0=vint[:, a:b, 1:48],
                                scalar1=cA[:, 1:2], scalar2=None,
                                op0=mybir.AluOpType.mult)
        nc.vector.scalar_tensor_tensor(
            out=brv[:, 1 + a:1 + b, 2:96:2], in0=vint[:, a:b, 0:47],
            scalar=cA[:, 0:1], in1=tscr[:, a:b, 0:47],
            op0=mybir.AluOpType.mult, op1=mybir.AluOpType.add)
        nc.vector.tensor_scalar(out=tscr[:, a:b, 0:47], in0=vint[:, a:b, 1:48],
                                scalar1=cA[:, 0:1], scalar2=None,
                                op0=mybir.AluOpType.mult)
        nc.vector.scalar_tensor_tensor(
            out=brv[:, 1 + a:1 + b, 3:96:2], in0=vint[:, a:b, 0:47],
            scalar=cA[:, 1:2], in1=tscr[:, a:b, 0:47],
            op0=mybir.AluOpType.mult, op1=mybir.AluOpType.add)

    for (k0, k1) in pl_chunks:
        # horizontal pool pairs for y rows 2k0..2k1-1
        nc.vector.tensor_tensor(out=plh[0:64, 2 * k0:2 * k1, :],
                                in0=xpv[0:64, 2 * k0:2 * k1, 1:97:2],
                                in1=xpv[0:64, 2 * k0:2 * k1, 2:98:2],
                                op=mybir.AluOpType.max)
        nc.vector.tensor_tensor(out=plh[64:128, 2 * k0:2 * k1, :],
                                in0=xpv[64:128, 2 * k0:2 * k1, 1:97:2],
                                in1=xpv[64:128, 2 * k0:2 * k1, 2:98:2],
                                op=mybir.AluOpType.add)
        # vertical pool pairs -> pl rows k0..k1-1
        nc.vector.tensor_tensor(out=pl[0:64, k0:k1, :],
                                in0=plh[0:64, 2 * k0:2 * k1:2, :],
                                in1=plh[0:64, 2 * k0 + 1:2 * k1:2, :],
                                op=mybir.AluOpType.max)
        nc.vector.tensor_tensor(out=pl[64:128, k0:k1, :],
                                in0=plh[64:128, 2 * k0:2 * k1:2, :],
                                in1=plh[64:128, 2 * k0 + 1:2 * k1:2, :],
                                op=mybir.AluOpType.add)
        if k0 == 0:
            nc.vector.tensor_copy(out=vint[:, 0, :], in_=pl[:, 0, :])
        # vertical bilinear rows that only need pl rows < k1
        ke_o = min(k1 - 1, 25)
        if ke_o > kv_o:
            a, b = kv_o, ke_o
            nc.vector.tensor_scalar(out=tscr[:, a:b, :], in0=pl[:, a + 1:b + 1, :],
                                    scalar1=0.25, scalar2=None,
                                    op0=mybir.AluOpType.mult)
            nc.vector.scalar_tensor_tensor(
                out=vint[:, 2 * a + 1:2 * b:2, :], in0=pl[:, a:b, :],
                scalar=0.75, in1=tscr[:, a:b, :],
                op0=mybir.AluOpType.mult, op1=mybir.AluOpType.add)
            kv_o = ke_o
        ke_e = min(k1 - 1, 24)
        if ke_e > kv_e:
            a, b = kv_e, ke_e
            nc.vector.tensor_scalar(out=tscr[:, a:b, :], in0=pl[:, a + 1:b + 1, :],
                                    scalar1=0.75, scalar2=None,
                                    op0=mybir.AluOpType.mult)
            nc.vector.scalar_tensor_tensor(
                out=vint[:, 2 * a + 2:2 * b + 1:2, :], in0=pl[:, a:b, :],
                scalar=0.25, in1=tscr[:, a:b, :],
                op0=mybir.AluOpType.mult, op1=mybir.AluOpType.add)
            kv_e = ke_e
        # horizontal pass over fully-available vint rows
        avail = min(2 * kv_o + 1, 2 * kv_e + 2) if k1 < 26 else TR
        hpass(hv, avail)
        hv = avail

    # brs rows (fp8) -> T slab slots; row-chunked for conv1 pipelining.
    # (ky slot s stores t row rho at tile row rho+1-s.)
    x2s = iter(((0, 10), (10, 30), (30, 49)))
    for ci, (a, b) in enumerate(((0, 8), (8, 18), (18, 28), (28, 38), (38, 50))):
        nc.sync.dma_start(out=slabv(0, 0, 64, a, b), in_=brv[0:64, a:b, :])
        nc.sync.dma_start(out=slabv(1, 0, 64, a, b), in_=brv[64:128, a:b, :])
        nc.sync.dma_start(out=slabv(1, 64, 128, a, b),
                          in_=brv[0:64, a + 1:b + 1, :])
        nc.sync.dma_start(out=slabv(2, 64, 128, a, b),
                          in_=brv[64:128, a + 1:b + 1, :])
        bb = min(b, 49)
        nc.sync.dma_start(out=slabv(3, 0, 64, a, bb),
                          in_=brv[0:64, a + 2:bb + 2, :])
        nc.sync.dma_start(out=slabv(4, 0, 64, a, bb),
                          in_=brv[64:128, a + 2:bb + 2, :])
        if ci < 3:
            x2_shift(*next(x2s))

    # oa (= xpool reuse) borders for the depthwise reads; ob borders
    oav = xpv[:, 0:50, :]
    nc.vector.memset(oav[:, 0:1, :], 0.0)
    nc.vector.memset(oav[:, :, 0:1], 0.0)
    nc.vector.memset(oav[:, :, 97:98], 0.0)
    nc.vector.memset(obv[:, 0:1, :], 0.0)
    nc.vector.memset(obv[0:64, :, 1:2], 0.0)
    nc.vector.memset(obv[64:128, :, 0:1], 0.0)
    nc.vector.memset(obv[:, :, 97:98], 0.0)

    # ---------------- conv1 (fp8 DoubleRow) + attention prologue ----------
    qk_blocks = _row_blocks(SR)
    v_blocks = _row_blocks(QR)

    def conv1_block(ot, bi, r0, nr, evac, dense=False):
        mw = 64 if ot == 4 else 128
        m0 = 128 * ot
        ps = pp1.tile([128, 5 * WPD], F32, tag="c1ps")
        pairs, wt = ((PAIRS_C1X, w1sx) if dense else (PAIRS_C1, w1s))
        for pi, (p0, p1) in enumerate(pairs):
            nc.tensor.matmul(
                ps[0:mw, 0:nr * WPD], lhsT=wt[:, pi, :, m0:m0 + mw],
                rhs=c1_rhs(tall_b, p0, p1, r0, nr),
                start=(pi == 0), stop=False,
                perf_mode=mybir.MatmulPerfMode.DoubleRow)
        for qi, (p0, p1) in enumerate(PAIRS_T4):
            nc.tensor.matmul(
                ps[0:mw, 0:nr * WPD], lhsT=w1s4[:, qi, :, m0:m0 + mw],
                rhs=c1_rhs(t4_b, p0, p1, r0, nr),
                start=False, stop=(qi == 1),
                perf_mode=mybir.MatmulPerfMode.DoubleRow)
        psv = ps.rearrange("p (r x) -> p r x", x=WPD)
        evac(bi, r0, nr, psv, mw)

    def conv1_tile(ot, blocks, evac):
        for bi, (r0, nr) in enumerate(blocks):
            conv1_block(ot, bi, r0, nr, evac, dense=True)

    def evac_qk(ot):
        def f(bi, r0, nr, psv, mw):
            nc.scalar.activation(
                out=qkv[ot][:, r0:r0 + nr, :], in_=psv[:, 0:nr, 1:97],
                func=mybir.ActivationFunctionType.Relu,
                bias=sc1[ot][:, 1:2], scale=sc1[ot][:, 0:1])
            dump = ev.tile([128, 5 * W], BF16, tag="sqd")
            nc.vector.tensor_tensor(
                out=dump[:, 0:nr * W],
                in0=qk[ot][:, r0 * W:(r0 + nr) * W],
                in1=qk[ot][:, r0 * W:(r0 + nr) * W],
                op=mybir.AluOpType.mult)
            nc.vector.reduce_sum(out=sqac[ot][:, bi:bi + 1],
                                 in_=dump[:, 0:nr * W],
                                 axis=mybir.AxisListType.X)
        return f

    def evac_v(ot):
        off = 0 if ot == 3 else VS

        def f(bi, r0, nr, psv, mw):
            dst = vall[0:mw, off + r0 * W:off + (r0 + nr) * W]
            nc.scalar.activation(
                out=dst, in_=psv[0:mw, 0:nr, 1:97],
                func=mybir.ActivationFunctionType.Relu,
                bias=sc1[ot][:, 1:2], scale=sc1[ot][:, 0:1])
        return f

    with tc.tile_pool(name="ps_c1", bufs=3, space="PSUM") as pp1, \
         tc.tile_pool(name="ps_tr", bufs=2, space="PSUM") as ppt, \
         tc.tile_pool(name="ps_s", bufs=1, space="PSUM") as pps:

        # conv1 qk row-blocks interleaved across the 3 out-tiles, with S'
        # transpose groups issued as soon as their pixel chunks are covered.
        qk0r = qk[0].rearrange("p (c k) -> p c k", k=128)
        qk1r = qk[1].rearrange("p (c k) -> p c k", k=128)
        qk2r = qk[2].rearrange("p (c k) -> p c k", k=128)
        sp = pps.tile([128, 2 * C], F32, tag="sp")

        def s_group(g):
            tq = ppt.tile([128, 3 * C], BF16, tag="tq")
            tk = ppt.tile([128, 3 * C], BF16, tag="tk")
            tqv = tq.rearrange("p (i c) -> p i c", c=C)
            tkv = tk.rearrange("p (i c) -> p i c", c=C)
            for i in range(3):
                ci = 3 * g + i
                nc.tensor.transpose(tqv[:, i, 0:128], qk0r[:, ci, :], ident[:])
                nc.tensor.transpose(tqv[:, i, 128:192], qk1r[0:64, ci, :],
                                    ident[0:64, 0:64])
                nc.tensor.transpose(tkv[:, i, 0:64], qk1r[64:128, ci, :],
                                    ident[64:128, 64:128])
                nc.tensor.transpose(tkv[:, i, 64:192], qk2r[:, ci, :], ident[:])
            qtc = ev.tile([128, 3 * C], BF16, tag="qtc")
            ktc = ev.tile([128, 3 * C], BF16, tag="ktc")
            nc.scalar.copy(out=qtc[:], in_=tq[:])
            nc.vector.tensor_copy(out=ktc[:], in_=tk[:])
            qcv = qtc.rearrange("p (i c) -> p i c", c=C)
            kcv = ktc.rearrange("p (i c) -> p i c", c=C)
            for i in range(3):
                nc.tensor.matmul(sp[:, 0:C], lhsT=kcv[:, i, 0:128],
                                 rhs=qcv[:, i, :],
                                 start=(g == 0 and i == 0),
                                 stop=(g == NCH // 3 - 1 and i == 2))
                nc.tensor.matmul(sp[0:64, C:2 * C], lhsT=kcv[:, i, 128:192],
                                 rhs=qcv[:, i, :],
                                 start=(g == 0 and i == 0),
                                 stop=(g == NCH // 3 - 1 and i == 2))

        next_g = 0
        for bi, (r0, nr) in enumerate(qk_blocks):
            for ot in range(3):
                conv1_block(ot, bi, r0, nr, evac_qk(ot), dense=(bi >= 4))
            px_done = (r0 + nr) * W
            while next_g < NCH // 3 and (3 * next_g + 3) * 128 <= px_done:
                s_group(next_g)
                next_g += 1
        while next_g < NCH // 3:
            s_group(next_g)
            next_g += 1

        # deferred zero-clears (needed only from P@V onward): ptall plane1
        # upper rows are a zero K-pad for the P@V DR weights; vall plane1
        # upper rows are junk read by that zero plane -- must be non-NaN
        # (0 x NaN = NaN).
        nc.vector.memset(ptall[64:128, C:2 * C], 0.0)
        nc.vector.memset(vall[64:128, VS:2 * VS], 0.0)

        # sumsq totals + AllReduce staging
        for ti in range(3):
            nc.vector.reduce_sum(out=accs[ti][:], in_=sqac[ti][:],
                                 axis=mybir.AxisListType.X)
        nc.vector.tensor_copy(out=ssa[:, 192:193], in_=accs[0][:])
        nc.vector.tensor_copy(out=ssb[:, 192:193], in_=accs[1][0:64, :])
        nc.vector.tensor_copy(out=accb[:, 0:1], in_=accs[1][:])
        nc.vector.tensor_copy(out=accb[:, 1:2], in_=accs[2][:])
        nc.sync.dma_start(out=ssa[0:64, 193:194], in_=accb[64:128, 0:1])
        nc.sync.dma_start(out=ssa[64:128, 193:194], in_=accb[0:64, 1:2])
        nc.sync.dma_start(out=ssb[0:64, 193:194], in_=accb[64:128, 1:2])
        nc.scalar.copy(out=ssa[:, 0:192], in_=sp[:, 0:C])
        nc.scalar.copy(out=ssb[:, 0:192], in_=sp[0:64, C:2 * C])
        nc.sync.dma_start(out=cc2i[0:64, :], in_=ssa[0:64, :])
        nc.scalar.dma_start(out=cc2i[64:128, :], in_=ssa[64:128, :])
        nc.gpsimd.dma_start(out=cc2i[128:192, :], in_=ssb[:])
        nc.gpsimd.collective_compute(
            "AllReduce", mybir.AluOpType.add, replica_groups=GROUPS,
            ins=[cc2i[:]], outs=[cc2o[:]])
        # sumsq columns first: the norm chain needs only these
        nc.gpsimd.dma_start(out=sfa[:, 192:194], in_=cc2o[0:128, 192:194])
        nc.gpsimd.dma_start(out=sfb[:, 192:194], in_=cc2o[128:192, 192:194])
        nc.sync.dma_start(out=sfa[0:64, 0:192], in_=cc2o[0:64, 0:192])
        nc.scalar.dma_start(out=sfa[64:128, 0:192], in_=cc2o[64:128, 0:192])
        nc.sync.dma_start(out=sfb[:, 0:192], in_=cc2o[128:192, 0:192])

        # conv1 v tiles overlap the collective
        conv1_tile(3, v_blocks, evac_v(3))
        conv1_tile(4, v_blocks, evac_v(4))

    # ---------------- transpose-free softmax -> fp8 P^T planes ------------
    with tc.tile_pool(name="ps_sm", bufs=1, space="PSUM") as ppm, \
         tc.tile_pool(name="ps_pv", bufs=2, space="PSUM") as ppv, \
         tc.tile_pool(name="ps_dw", bufs=2, space="PSUM") as ppd:
        psq = ppm.tile([1, C], BF16, tag="psq")
        pm = ppm.tile([128, 392], F32, tag="pm")
        bcast = pm[:, 0:192]
        pcs = pm[0:1, 196:388]

        nc.tensor.transpose(psq[0:1, 0:128], sfa[:, 192:193], ident[:])
        nc.tensor.transpose(psq[0:1, 128:192], sfb[0:64, 192:193],
                            ident[0:64, 0:64])
        nc.vector.tensor_scalar(out=qs[:], in0=psq[0:1, :], scalar1=1e-24,
                                scalar2=None, op0=mybir.AluOpType.max)
        nc.vector.tensor_scalar(out=rska[:], in0=sfa[:, 193:194],
                                scalar1=1e-24, scalar2=None,
                                op0=mybir.AluOpType.max)
        nc.vector.tensor_scalar(out=rskb[:], in0=sfb[0:64, 193:194],
                                scalar1=1e-24, scalar2=None,
                                op0=mybir.AluOpType.max)
        for r in (qs, rska, rskb):
            nc.vector.reciprocal(out=r[:], in_=r[:])
            nc.scalar.activation(out=r[:], in_=r[:],
                                 func=mybir.ActivationFunctionType.Sqrt)
        nc.tensor.matmul(bcast, lhsT=ones1[:, :], rhs=qs[:],
                         start=True, stop=True)
        nc.vector.scalar_tensor_tensor(
            out=ea[:], in0=sfa[:, 0:192], scalar=tmps[:, 0:1], in1=bcast,
            op0=mybir.AluOpType.mult, op1=mybir.AluOpType.mult)
        nc.vector.scalar_tensor_tensor(
            out=eb[:], in0=sfb[0:64, 0:192], scalar=tmps[0:64, 0:1],
            in1=bcast[0:64, :],
            op0=mybir.AluOpType.mult, op1=mybir.AluOpType.mult)
        # P^T planes (unnormalized): plane0 = keys 0:128, plane1 = keys 128:192
        nc.scalar.activation(out=ptall[:, 0:C], in_=ea[:],
                             func=mybir.ActivationFunctionType.Exp,
                             bias=0.0, scale=rska[:, 0:1])
        nc.scalar.activation(out=ptall[0:64, C:2 * C], in_=eb[:],
                             func=mybir.ActivationFunctionType.Exp,
                             bias=0.0, scale=rskb[:, 0:1])
        # out = P^T.T @ v (fp8 DoubleRow over the two key planes), interleaved
        # with depthwise-A blocks so dw matmuls hide the P@V evac latency.
        ptv = ptall.rearrange("p (two q) -> p two q", two=2)
        va_b = vall[:]
        yv = yout.rearrange("c (r w) -> c r w", w=W)

        def pv_rhs(r0, nr):
            return bass.AP(tensor=va_b.tensor, offset=va_b.offset + r0 * W,
                           ap=[[va_b.ap[0][0], 128], [VS, 2], [1, nr * W]])

        def pv_block(r0, nr):
            po = ppv.tile([128, 5 * W], F32, tag="po")
            po2 = ppv.tile([64, 5 * W], F32, tag="po2")
            nc.tensor.matmul(po[:, 0:nr * W], lhsT=ptv[:, :, 0:128],
                             rhs=pv_rhs(r0, nr), start=True, stop=True,
                             perf_mode=mybir.MatmulPerfMode.DoubleRow)
            nc.tensor.matmul(po2[0:64, 0:nr * W], lhsT=ptv[:, :, 128:192],
                             rhs=pv_rhs(r0, nr), start=True, stop=True,
                             perf_mode=mybir.MatmulPerfMode.DoubleRow)
            pov = po.rearrange("p (r w) -> p r w", w=W)
            po2v = po2.rearrange("p (r w) -> p r w", w=W)
            nc.scalar.copy(out=oav[:, r0 + 1:r0 + 1 + nr, 1:97],
                           in_=pov[:, 0:nr, :])
            nc.vector.tensor_copy(out=obv[0:64, r0 + 1:r0 + 1 + nr, 2:98],
                                  in_=po2v[0:64, 0:nr, :])

        def dwa_block(r0, nr):
            ps = ppd.tile([128, 5, W], F32, tag="dwps")
            for t in range(9):
                dy, dx = t // 3 - 1, t % 3 - 1
                nc.tensor.matmul(
                    ps[:, 0:nr, :],
                    lhsT=w2av[:, t, :],
                    rhs=oav[:, r0 + 1 + dy:r0 + 1 + dy + nr, 1 + dx:97 + dx],
                    start=(t == 0), stop=(t == 8))
            fo = ev.tile([128, 5, W], F32, tag="fo")
            nc.scalar.activation(out=fo[:, 0:nr, :], in_=ps[:, 0:nr, :],
                                 func=mybir.ActivationFunctionType.Identity,
                                 bias=b2a[:, 0:1], scale=rsA[:, 0:1])
            nc.sync.dma_start(out=yv[0:128, r0:r0 + nr, :],
                              in_=fo[:, 0:nr, :])

        def dwa_chain_dve(acc, r0, nr):
            for t in range(9):
                dy, dx = t // 3 - 1, t % 3 - 1
                rhs = oav[:, r0 + 1 + dy:r0 + 1 + dy + nr, 1 + dx:97 + dx]
                if t == 0:
                    nc.vector.tensor_scalar(
                        out=acc[:, 0:nr, :], in0=rhs, scalar1=w2ta[:, t:t + 1],
                        scalar2=None, op0=mybir.AluOpType.mult)
                else:
                    nc.vector.scalar_tensor_tensor(
                        out=acc[:, 0:nr, :], in0=rhs, scalar=w2ta[:, t:t + 1],
                        in1=acc[:, 0:nr, :], op0=mybir.AluOpType.mult,
                        op1=mybir.AluOpType.add)

        def dwa_evac_dve(acc, r0, nr):
            fo = ev.tile([128, 5, W], F32, tag="fo")
            nc.scalar.activation(out=fo[:, 0:nr, :], in_=acc[:, 0:nr, :],
                                 func=mybir.ActivationFunctionType.Identity,
                                 bias=b2a[:, 0:1], scale=rsA[:, 0:1])
            nc.sync.dma_start(out=yv[0:128, r0:r0 + nr, :],
                              in_=fo[:, 0:nr, :])

        dw_blocks = _row_blocks(OR_)
        pv_block(*v_blocks[0])
        pv_block(*v_blocks[1])
        # softmax denominator: colsum over keys via ones-matmul
        nc.tensor.matmul(pcs, lhsT=ones8[:, 0:1],
                         rhs=ptall[:, 0:C], start=True, stop=False)
        nc.tensor.matmul(pcs, lhsT=ones8[:, 0:1],
                         rhs=ptall[:, C:2 * C], start=False, stop=True)
        nc.vector.reciprocal(out=isr[:], in_=pcs)
        prt = pm[:, 192:194]
        nc.tensor.transpose(prt[0:128, 0:1], isr[0:1, 0:128],
                            identf[0:1, 0:1])
        nc.tensor.transpose(prt[0:64, 1:2], isr[0:1, 128:192],
                            identf[0:1, 0:1])
        nc.vector.tensor_copy(out=rsA[:], in_=prt[0:128, 0:1])
        nc.vector.tensor_copy(out=rsB[:], in_=prt[0:64, 1:2])

        for bi in range(2, len(v_blocks)):
            dwa_block(*dw_blocks[bi - 2])
            pv_block(*v_blocks[bi])
            if bi == 5:
                # replicate ob rows 1..25 (needs P@V-B evacs through block 4)
                nc.sync.dma_start(out=obv[64:128, 1:26, 1:97],
                                  in_=obv[0:64, 1:26, 2:98])
        nc.sync.dma_start(out=obv[64:128, 26:50, 1:97],
                          in_=obv[0:64, 26:50, 2:98])
        dwa_chain_dve(dwacc[0][:], *dw_blocks[8])
        dwa_chain_dve(dwacc[1][:], *dw_blocks[9])
        for (r0, nr) in dw_blocks:
            ps = ppd.tile([128, 5, W], F32, tag="dwps")
            for dy in range(3):
                nc.tensor.matmul(
                    ps[0:64, 0:nr, :], lhsT=w2fv[:, dy, :],
                    rhs=obv[:, r0 + dy:r0 + dy + nr, 1:97],
                    start=(dy == 0), stop=False)
                nc.tensor.matmul(
                    ps[0:64, 0:nr, :], lhsT=w2gv[64:128, dy, :],
                    rhs=obv[64:128, r0 + dy:r0 + dy + nr, 2:98],
                    start=False, stop=(dy == 2))
            fo = ev.tile([128, 5, W], F32, tag="fo")
            nc.scalar.activation(out=fo[0:64, 0:nr, :], in_=ps[0:64, 0:nr, :],
                                 func=mybir.ActivationFunctionType.Identity,
                                 bias=b2b[:, 0:1], scale=rsB[:, 0:1])
            nc.sync.dma_start(out=yv[128:192, r0:r0 + nr, :],
                              in_=fo[0:64, 0:nr, :])
        dwa_evac_dve(dwacc[0][:], *dw_blocks[8])
        dwa_evac_dve(dwacc[1][:], *dw_blocks[9])
    ctx.close()


# ---------------- host side ----------------
_NC_CACHE = None


def _get_nc():
    global _NC_CACHE
    if _NC_CACHE is None:
        _NC_CACHE = build_nc()
    return _NC_CACHE


def _pack_weights(inp, flip):
    bf = ml_dtypes.bfloat16
    w0 = inp["w0"][:, :, ::-1, :] if flip else inp["w0"]
    w1 = inp["w1"][:, :, ::-1, :] if flip else inp["w1"]
    w2 = inp["w2"][:, :, ::-1, :] if flip else inp["w2"]
    w0 = np.asarray(w0, np.float32)
    w1 = np.asarray(w1, np.float32)
    w2 = np.asarray(w2, np.float32)

    # conv0: out-channel order [x1(0:64), x3(128:192), x2(64:128)]
    cho = np.concatenate([np.arange(0, 64), np.arange(128, 192),
                          np.arange(64, 128)])
    wt = w0[cho].transpose(1, 2, 3, 0)        # [64c, 3ky, 3kx, 192m]
    w0p = np.zeros((128, 3, 2, C), np.float32)
    w0p[0:64, 0, 0] = wt[:, 0, 0]
    w0p[0:64, 0, 1] = wt[:, 0, 2]
    w0p[0:64, 1, 0] = wt[:, 0, 1]
    w0p[64:128, 0, 0] = wt[:, 1, 0]
    w0p[64:128, 0, 1] = wt[:, 1, 2]
    w0p[64:128, 1, 0] = wt[:, 1, 1]
    w0p[64:128, 1, 1] = wt[:, 2, 1]
    w0p[64:128, 2, 0] = wt[:, 2, 0]
    w0p[64:128, 2, 1] = wt[:, 2, 2]
    s0 = inp["g0"] / np.sqrt(inp["v0"] + BN_EPS)
    t0 = inp["be0"] + (inp["b0"] - inp["m0"]) * s0
    sb0 = np.stack([s0 / W0S, t0], axis=1).astype(np.float32)[cho]

    # conv1 DoubleRow packs.  Slab k-partition -> (ky, c) maps:
    def slab_map(s):
        k = np.arange(128 if s < 4 else 64)
        if s == 0:
            return np.zeros_like(k), k
        if s == 1:
            return np.where(k < 64, 0, 1), np.where(k < 64, 128 + k, k - 64)
        if s == 2:
            return np.ones_like(k), 64 + k
        if s == 3:
            return np.full_like(k, 2), k
        return np.full_like(k, 2), 128 + k

    wt1 = w1.transpose(1, 2, 3, 0)            # [192c, 3ky, 3kx, 576m]
    w1p = np.zeros((128, 8, 2, C3), np.float32)
    for pi, (p0, p1) in enumerate(PAIRS_C1):
        for pl, spec in enumerate((p0, p1)):
            if spec is None:
                continue
            s, dx = spec
            ky, cc = slab_map(s)
            w1p[:, pi, pl] = wt1[cc, ky, dx]
    w1px = np.zeros((128, 6, 2, C3), np.float32)
    for pi, (p0, p1) in enumerate(PAIRS_C1X):
        for pl, (s, dx) in enumerate((p0, p1)):
            ky, cc = slab_map(s)
            w1px[:, pi, pl] = wt1[cc, ky, dx]
    w1p4 = np.zeros((64, 2, 2, C3), np.float32)
    ky4, cc4 = slab_map(4)
    w1p4[:, 0, 0] = wt1[cc4, ky4, 0]
    w1p4[:, 0, 1] = wt1[cc4, ky4, 2]
    w1p4[:, 1, 0] = wt1[cc4, ky4, 1]
    s1 = inp["g1"] / np.sqrt(inp["v1"] + BN_EPS)
    t1 = inp["be1"] + (inp["b1"] - inp["m1"]) * s1
    sb1 = np.stack([s1 / W1S, t1], axis=1).astype(np.float32)

    w2ta = np.zeros((128, 9), np.float32)
    for t in range(9):
        w2ta[:, t] = w2[0:128, 0, t // 3, t % 3]
    w2da = np.zeros((128, 9, 128), np.float32)
    w2fb = np.zeros((128, 3, 64), np.float32)
    w2gb = np.zeros((128, 3, 64), np.float32)
    r64, r128 = np.arange(64), np.arange(128)
    for t in range(9):
        d = w2[:, 0, t // 3, t % 3]
        w2da[r128, t, r128] = d[0:128]
    for dy in range(3):
        db = w2[128:192, 0, dy, :]
        w2fb[r64, dy, r64] = db[:, 0]
        w2fb[64 + r64, dy, r64] = db[:, 1]
        w2gb[64 + r64, dy, r64] = db[:, 2]

    out = {
        "w0p": np.ascontiguousarray(
            (w0p * W0S).reshape(128, 3 * 2 * C)).astype(f8np),
        "sb0p": sb0,
        "w1p": np.ascontiguousarray(
            (w1p * W1S).reshape(128, 8 * 2 * C3)).astype(f8np),
        "w1px": np.ascontiguousarray(
            (w1px * W1S).reshape(128, 6 * 2 * C3)).astype(f8np),
        "w1p4": np.ascontiguousarray(
            (w1p4 * W1S).reshape(64, 2 * 2 * C3)).astype(f8np),
        "sb1": sb1,
        "w2da": np.ascontiguousarray(w2da.reshape(128, 9 * 128)).astype(bf),
        "w2ta": w2ta,
        "w2fb": np.ascontiguousarray(w2fb.reshape(128, 3 * 64)).astype(bf),
        "w2gb": np.ascontiguousarray(w2gb.reshape(128, 3 * 64)).astype(bf),
        "b2v": np.asarray(inp["b2"], np.float32).reshape(C, 1),
    }
    return out


def kernel(**inputs):
    inputs = {k: np.asarray(v) for k, v in inputs.items()}
    x = inputs["x"]
    B = x.shape[0]
    packs = [_pack_weights(inputs, flip) for flip in (False, True)]
    tempv = np.asarray(inputs["temp"], np.float32).reshape(1, 1)

    in_maps = []
    for core in range(8):
        s, h = core // 2, core % 2
        xi = np.asarray(x[s], np.float32)
        if h:
            xi = xi[:, ::-1, :]
        slab = np.zeros((64, XR, WPD), np.float32)
        slab[:, 1:54, 1:97] = xi[:, 0:53, :]
        m = dict(packs[h])
        m["xs"] = np.ascontiguousarray(slab.reshape(64, XR * WPD)).astype(f8np)
        m["tempv"] = tempv
        m["zpad8"] = np.zeros((1, 512), f8np)
        m["zpad16"] = np.zeros((1, 512), ml_dtypes.bfloat16)
        in_maps.append(m)

    nc = _get_nc()
    res = run_bass_kernel_spmd(nc, in_maps, list(range(8)))
    out = np.zeros((B, C, 96, 96), np.float32)
    for core in range(8):
        s, h = core // 2, core % 2
        yc = res.results[core]["yout"].reshape(C, OR_, W)
        if h:
            out[s, :, 48:96] = yc[:, ::-1, :]
        else:
            out[s, :, 0:48] = yc
    return out
